# revision 1
# baseline (speedup 1.0000x reference)
"""HAN kernel v2 — dma_gather-based edge aggregation, bf16 tables.

Strategy (dst-partitioned, batched-gather + one-hot-matmul aggregation):
  - Each core owns dst rows [c*12500, (c+1)*12500) of the mat nodes.
  - P1 projects all mat nodes into 4 bf16 window tables Tm0..Tm3 of 32768
    rows each (dma_gather indices are int16), elem nodes into Te, and the
    core's local dsts' attention-dot values into ADl [12544, 128] bf16
    (row = [aD_em(8) | aD_dd(8) | pad]); h-dot-a_src is recomputed on-chip
    from gathered rows, so tables store h only (256B rows, the dma_gather
    minimum).
  - P2: edges are host-bucketed by (core, dst_tile, src_window); each
    group is padded to 128-edge chunks with idx=-1 (skipped by the DMA) and
    dst_local=300 sentinel (excluded by the one-hot). Per tile: one int16
    load carries all gather indices (wrapped 16-partition layout) plus
    bf16-bitcast dst_local; 5 dma_gathers fetch h rows, 1 fetches aD rows.
    alpha = <h_src,a_src> (mult+reduce) + aD[dst]; leakyrelu; exp lands in
    the [128:136] cols of the weighted tile so each chunk's one-hot matmul
    scatter-adds numerator|denominator into PSUM at once.
  - o = relu(num/den), PE-transpose, bf16 oT tables; tanh/semantic partial
    sums accumulate; P3 as before (1KB AllReduce, softmax over 2 metapaths,
    weighted combine in bf16, final linear to fp32).
"""

import numpy as np
import ml_dtypes

import concourse.bacc as bacc
import concourse.bass as bass
import concourse.mybir as mybir
import concourse.tile as tile
from concourse.bass_utils import run_bass_kernel_spmd
from concourse.masks import make_identity

P = 128
N_MAT = 100000
N_ELEM = 118
F_MAT = 128
F_ELEM = 64
HID = 128
H = 8
D = 16
OUT = 64
NCORES = 8
ND = N_MAT // NCORES          # 12500 dst rows per core
NT = (ND + P - 1) // P        # 98 dst tiles per core
NDP = NT * P                  # 12544 padded dst rows
NEG = 0.2
WIN = 32768                   # int16 index window
NWIN = 4
SLABS_PER_WIN = WIN // 1024   # 32
NTM_FULL = N_MAT // 1024      # 97 full 1024-row projection slabs
LAST_SLAB = N_MAT - NTM_FULL * 1024  # 672
NDL_FULL = ND // 1024         # 12 full local-AD slabs
LAST_DL = ND - NDL_FULL * 1024       # 212
F32 = mybir.dt.float32
BF16 = mybir.dt.bfloat16
I16 = mybir.dt.int16

BF = ml_dtypes.bfloat16


def _bf(a):
    return np.asarray(a, dtype=BF)


def _blockdiag(a):
    """a [H, D] -> [HID, H] block diagonal so h @ A = per-head <h, a>."""
    A = np.zeros((HID, H), np.float32)
    for h in range(H):
        A[h * D:(h + 1) * D, h] = a[h]
    return A


def build_host_tensors(inputs):
    x_mat = inputs["x_mat"]
    WpT = np.ascontiguousarray(inputs["W_proj_mat"].T)       # [128f, 128k]
    WpeT = np.ascontiguousarray(inputs["W_proj_elem"].T)     # [64f, 128k]
    A_dem = _blockdiag(inputs["a_dst_em"])
    A_dmm = _blockdiag(inputs["a_dst_mm"])
    wad = np.concatenate([WpT @ A_dem, WpT @ A_dmm], axis=1)  # [128, 16]
    bad_row = np.concatenate(
        [inputs["b_proj_mat"] @ A_dem, inputs["b_proj_mat"] @ A_dmm])
    # a_src replicated per partition for the on-chip <h, a_src> dot
    asem = np.tile(inputs["a_src_em"].reshape(1, HID), (P, 1))
    asmm = np.tile(inputs["a_src_mm"].reshape(1, HID), (P, 1))

    host = dict(
        xT=_bf(x_mat.T),                                     # [128, 100000]
        xeT=_bf(inputs["x_elem"].T),                         # [64, 118]
        whT=_bf(WpT),                                        # [128, 128]
        bh=np.tile(inputs["b_proj_mat"].astype(np.float32), (P, 1)),
        weT=_bf(WpeT),                                       # [64, 128]
        be=np.tile(inputs["b_proj_elem"].astype(np.float32), (P, 1)),
        wad=_bf(wad),                                        # [128, 16]
        bad=np.tile(bad_row.astype(np.float32), (P, 1)),     # [128, 16]
        asem=_bf(asem),
        asmm=_bf(asmm),
        wkT=_bf(np.ascontiguousarray(inputs["Wk"].T)),
        bkc=inputs["bk"].astype(np.float32)[:, None],
        qc=(inputs["q"] / float(N_MAT)).astype(np.float32)[:, None],
        wlT=_bf(np.ascontiguousarray(inputs["Wl"].T)),
        blb=np.tile(inputs["bl"].astype(np.float32), (P, 1)),
        iot=_bf(np.tile(np.arange(P, dtype=np.float32), (P, 1))),
        ones=np.ones((1, P), np.float32),
    )
    return host


def bucket_edges_v2(inputs):
    """Joint bucketing of em+mm edges by (core, dst_tile, group) where
    group 0 = em (Te table), 1+w = mm window w. Returns per-core int16
    packs [NCORES, NT, P, 17*CH] and the static chunk counts."""
    NG = 1 + NWIN
    src_em = inputs["src_em"].astype(np.int64)
    dst_em = inputs["dst_em"].astype(np.int64)
    src_mm = inputs["src_mm"].astype(np.int64)
    dst_mm = inputs["dst_mm"].astype(np.int64)

    g = np.concatenate([np.zeros(len(src_em), np.int64), 1 + src_mm // WIN])
    val = np.concatenate([src_em, src_mm % WIN]).astype(np.int64)
    dst = np.concatenate([dst_em, dst_mm])
    core = dst // ND
    rem = dst % ND
    tl = rem // P
    dl = rem % P
    key = (core * NT + tl) * NG + g
    order = np.argsort(key, kind="stable")
    counts = np.bincount(key, minlength=NCORES * NT * NG)
    c3 = counts.reshape(NCORES, NT, NG)
    CH_EM = int(-(-c3[:, :, 0].max() // P))
    CHR = int(-(-c3[:, :, 1:].max() // P))
    CH = CH_EM + NWIN * CHR

    starts = np.zeros(NCORES * NT * NG, np.int64)
    starts[1:] = np.cumsum(counts)[:-1]
    ks = key[order]
    rank = np.arange(len(order)) - starts[ks]
    lane = rank % P
    chunk = rank // P
    gs = ks % NG
    ct = ks // NG                      # core*NT + tl
    gbase = np.where(gs == 0, 0, CH_EM + (gs - 1) * CHR)
    chunkpos = gbase + chunk           # chunk slot within [0, CH)

    # gather-idx arrays are consumed as flat position i = chunk*128 + lane
    flat_main = (ct * CH + chunkpos) * P + lane
    # pad slots gather row 0 (valid; excluded from aggregation by the
    # dl=300 sentinel) — the gather ucode requires non-negative indices
    # except as a contiguous tail, and num_idxs_reg == count(idx >= 0)
    main_idx = np.zeros(NCORES * NT * CH * P, np.int16)
    main_idx[flat_main] = val[order].astype(np.int16)
    ad_idx = np.zeros(NCORES * NT * CH * P, np.int16)
    ad_idx[flat_main] = rem[order].astype(np.int16)
    # dl is consumed as [lane, chunkpos]
    flat_dl = (ct * P + lane) * CH + chunkpos
    dlv = np.full(NCORES * NT * P * CH, 300.0, np.float32)
    dlv[flat_dl] = dl[order]

    main_idx = main_idx.reshape(NCORES, NT, CH * P)
    ad_idx = ad_idx.reshape(NCORES, NT, CH * P)
    dlb = _bf(dlv).view(np.int16).reshape(NCORES, NT, P, CH)

    def wrap(a, ncols):
        # [..., n] -> [..., 128, n//16] (16-part wrap, replicated x8)
        sh = a.shape[:-1]
        w = a.reshape(*sh, ncols // 16, 16)
        w = np.swapaxes(w, -1, -2)                   # [..., 16, n//16]
        w = np.broadcast_to(w[..., None, :, :],
                            (*sh, 8, 16, ncols // 16))
        return w.reshape(*sh, P, ncols // 16)

    packs = []
    em_w = wrap(main_idx[:, :, 0:CH_EM * P], CH_EM * P)
    mm_w = [wrap(main_idx[:, :, (CH_EM + r * CHR) * P:
                          (CH_EM + (r + 1) * CHR) * P], CHR * P)
            for r in range(NWIN)]
    ad_w = wrap(ad_idx, CH * P)
    e16 = np.concatenate([em_w, *mm_w, ad_w, dlb], axis=3)
    assert e16.shape == (NCORES, NT, P, 17 * CH), e16.shape
    return np.ascontiguousarray(e16), CH_EM, CHR


def build_program(ch_em, chr_, dbg=False, variant="full"):
    vparts = variant.split("-")
    vflags = set(vparts[1:])
    variant = vparts[0]
    CH = ch_em + NWIN * chr_
    W16 = 17 * CH
    nc = bacc.Bacc(
        "TRN2",
        target_bir_lowering=False,
        debug=False,
        enable_asserts=False,
        num_devices=NCORES,
    )

    inp = {}
    def din(name, shape, dt=F32):
        inp[name] = nc.dram_tensor(name, list(shape), dt, kind="ExternalInput").ap()
        return inp[name]

    xT = din("xT", [F_MAT, N_MAT], BF16)
    xeT = din("xeT", [F_ELEM, N_ELEM], BF16)
    xlT = din("xlT", [F_MAT, ND], BF16)
    whT = din("whT", [F_MAT, HID], BF16)
    bh = din("bh", [P, HID])
    weT = din("weT", [F_ELEM, HID], BF16)
    be = din("be", [P, HID])
    wad = din("wad", [F_MAT, 16], BF16)
    bad = din("bad", [P, 16])
    asem = din("asem", [P, HID], BF16)
    asmm = din("asmm", [P, HID], BF16)
    e16 = din("e16", [NT, P, W16], I16)
    wkT = din("wkT", [HID, HID], BF16)
    bkc = din("bkc", [HID, 1])
    qc = din("qc", [HID, 1])
    wlT = din("wlT", [HID, OUT], BF16)
    blb = din("blb", [P, OUT])
    iot = din("iot", [P, P], BF16)
    ones = din("ones", [1, P])
    y = nc.dram_tensor("y", [NDP, OUT], F32, kind="ExternalOutput").ap()

    with tile.TileContext(nc) as tc:
        with (
            tc.tile_pool(name="const", bufs=1) as cp,
            tc.tile_pool(name="dram", bufs=1, space="DRAM") as dp,
        ):
            # ---- persistent DRAM tables ----
            Tm = [dp.tile([WIN, HID], BF16, name=f"Tm{r}")
                  for r in range(NWIN)]
            Te = dp.tile([P, HID], BF16)
            ADl = dp.tile([NDP, P], BF16)
            oemT = dp.tile([HID, NDP], BF16)
            ommT = dp.tile([HID, NDP], BF16)
            Sin_d = dp.tile([HID, 2], F32)
            Sout_d = dp.tile([HID, 2], F32)

            # ---- constants in SBUF ----
            def lc(ap_in, shape, tag, dt=F32):
                t = cp.tile(list(shape), dt, tag=tag)
                nc.sync.dma_start(out=t[:], in_=ap_in[:])
                return t

            whT_sb = lc(whT, [F_MAT, HID], "whT", BF16)
            bh_sb = lc(bh, [P, HID], "bh")
            weT_sb = lc(weT, [F_ELEM, HID], "weT", BF16)
            be_sb = lc(be, [P, HID], "be")
            wad_sb = lc(wad, [F_MAT, 16], "wad", BF16)
            bad_sb = lc(bad, [P, 16], "bad")
            asem_sb = lc(asem, [P, HID], "asem", BF16)
            asmm_sb = lc(asmm, [P, HID], "asmm", BF16)
            wkT_sb = lc(wkT, [HID, HID], "wkT", BF16)
            bkc_sb = lc(bkc, [HID, 1], "bkc")
            qc_sb = lc(qc, [HID, 1], "qc")
            wlT_sb = lc(wlT, [HID, OUT], "wlT", BF16)
            blb_sb = lc(blb, [P, OUT], "blb")
            iot_sb = lc(iot, [P, P], "iot", BF16)
            ones_sb = lc(ones, [1, P], "ones")
            ident = cp.tile([P, P], F32, tag="ident")
            make_identity(nc, ident[:])
            S_sb = cp.tile([HID, 2], F32, tag="S")
            nc.gpsimd.memset(S_sb[:], 0.0)

            # ================= P1: projections =================
            do_mat_proj = variant != "s_gem2"
            with (
                tc.tile_pool(name="p1s", bufs=3) as p1s,
                tc.tile_pool(name="p1p", bufs=2, space="PSUM") as p1p,
            ):
                for s in range(NTM_FULL + 1 if do_mat_proj else 0):
                    w = 1024 if s < NTM_FULL else LAST_SLAB
                    win = s // SLABS_PER_WIN
                    wrow = (s % SLABS_PER_WIN) * 1024
                    xsl = p1s.tile([P, 1024], BF16, tag="xsl")
                    nc.sync.dma_start(
                        out=xsl[:, 0:w], in_=xT[:, s * 1024: s * 1024 + w]
                    )
                    ev = p1s.tile([P, 8, HID], BF16, tag="ev")
                    ntile = (w + P - 1) // P
                    for j in range(ntile):
                        m = min(P, w - j * P)
                        ps = p1p.tile([P, HID], F32, tag="ps")
                        nc.tensor.matmul(
                            out=ps[0:m, :],
                            lhsT=xsl[:, j * P: j * P + m],
                            rhs=whT_sb[:],
                            start=True,
                            stop=True,
                        )
                        nc.vector.tensor_add(
                            out=ev[0:m, j, :], in0=ps[0:m, :], in1=bh_sb[0:m, :]
                        )
                    if s < NTM_FULL:
                        nc.sync.dma_start(
                            out=Tm[win][wrow: wrow + 1024, :].rearrange(
                                "(a p) e -> p a e", p=P
                            ),
                            in_=ev[:, :, :],
                        )
                    else:
                        for j in range(ntile):
                            m = min(P, w - j * P)
                            r0 = wrow + j * P
                            nc.sync.dma_start(
                                out=Tm[win][r0: r0 + m, :], in_=ev[0:m, j, :]
                            )
                # elem projection
                xe_sb = p1s.tile([F_ELEM, N_ELEM], BF16, tag="xe")
                nc.sync.dma_start(out=xe_sb[:], in_=xeT[:])
                pse = p1p.tile([P, HID], F32, tag="ps")
                nc.tensor.matmul(
                    out=pse[0:N_ELEM, :],
                    lhsT=xe_sb[:],
                    rhs=weT_sb[:],
                    start=True,
                    stop=True,
                )
                eve = p1s.tile([P, 8, HID], BF16, tag="ev")
                nc.vector.tensor_add(
                    out=eve[0:N_ELEM, 0, :],
                    in0=pse[0:N_ELEM, :],
                    in1=be_sb[0:N_ELEM, :],
                )
                nc.sync.dma_start(
                    out=Te[0:N_ELEM, :], in_=eve[0:N_ELEM, 0, :]
                )
                if "tm" in vflags and not do_mat_proj:
                    nc.sync.dma_start(
                        out=Tm[0][0:N_ELEM, :], in_=eve[0:N_ELEM, 0, :]
                    )
                # local-dst attention dots -> ADl[:, 0:16]
                for s in range(NDL_FULL + 1 if do_mat_proj else 0):
                    w = 1024 if s < NDL_FULL else LAST_DL
                    xsl = p1s.tile([P, 1024], BF16, tag="xsl")
                    nc.sync.dma_start(
                        out=xsl[:, 0:w], in_=xlT[:, s * 1024: s * 1024 + w]
                    )
                    ev2 = p1s.tile([P, 8, 16], BF16, tag="ev2")
                    ntile = (w + P - 1) // P
                    for j in range(ntile):
                        m = min(P, w - j * P)
                        ps = p1p.tile([P, HID], F32, tag="ps")
                        nc.tensor.matmul(
                            out=ps[0:m, 0:16],
                            lhsT=xsl[:, j * P: j * P + m],
                            rhs=wad_sb[:],
                            start=True,
                            stop=True,
                        )
                        nc.vector.tensor_add(
                            out=ev2[0:m, j, :], in0=ps[0:m, 0:16],
                            in1=bad_sb[0:m, :]
                        )
                    rows = s * 1024
                    if s < NDL_FULL:
                        nc.sync.dma_start(
                            out=ADl[rows: rows + 1024, 0:16].rearrange(
                                "(a p) e -> p a e", p=P
                            ),
                            in_=ev2[:, :, :],
                        )
                    else:
                        for j in range(ntile):
                            m = min(P, w - j * P)
                            r0 = rows + j * P
                            nc.sync.dma_start(
                                out=ADl[r0: r0 + m, 0:16], in_=ev2[0:m, j, :]
                            )

            # ================= P2: edge aggregation =================
            do_p2 = variant != "nop2"
            strip = variant.startswith("s_")
            g_em = variant in ("full", "gem", "s_gem", "s_gem2", "s_all",
                               "noad", "nomm")
            g_mm = variant in ("full", "gmm", "s_all", "noad", "noem")
            g_ad = variant in ("full", "gad", "s_all", "nomm", "noem")
            if do_p2:
              with (
                tc.tile_pool(name="p2s", bufs=2) as p2s,
                tc.tile_pool(name="p2p", bufs=2, space="PSUM") as p2p,
                tc.tile_pool(name="p2t", bufs=2, space="PSUM") as p2t,
                tc.tile_pool(name="p2k", bufs=2, space="PSUM") as p2k,
              ):
                for t in range(4 if "nt4" in vflags else NT):
                    i16 = p2s.tile([P, W16], I16, tag="i16")
                    nc.sync.dma_start(out=i16[:], in_=e16[t, :, :])
                    G = p2s.tile([P, CH, HID], BF16, tag="G")
                    ADg = p2s.tile([P, CH, HID], BF16, tag="ADg")
                    if not (g_em and g_mm):
                        nc.gpsimd.memset(G[:], 0.0)
                    if not g_ad:
                        nc.gpsimd.memset(ADg[:], 0.0)
                    if g_em:
                        src_tab = Tm[0] if "tm" in vflags else Te
                        nc.gpsimd.dma_gather(
                            G[:, 0:ch_em, :], src_tab[:], i16[:, 0:8 * ch_em],
                            ch_em * P, ch_em * P, HID, single_packet=False,
                        )
                    if g_mm:
                        for r in range(NWIN):
                            c0 = ch_em + r * chr_
                            o16 = 8 * (ch_em + r * chr_)
                            nc.gpsimd.dma_gather(
                                G[:, c0:c0 + chr_, :], Tm[r][:],
                                i16[:, o16:o16 + 8 * chr_],
                                chr_ * P, chr_ * P, HID, single_packet=False,
                            )
                    if g_ad:
                        nc.gpsimd.dma_gather(
                            ADg[:, :, :], ADl[:],
                            i16[:, 8 * CH:16 * CH], CH * P, CH * P, HID,
                            single_packet=False,
                        )
                    if strip:
                        sG = p2s.tile([P, 2, HID], BF16, tag="sG")
                        nc.vector.tensor_copy(out=sG[:], in_=G[:, 0:2, :])
                        if "nostore" not in vflags:
                            nc.vector.tensor_copy(out=sG[:, 0:1, :],
                                                  in_=ADg[:, 0:1, :])
                            nc.sync.dma_start(
                                out=oemT[:, t * P:(t + 1) * P],
                                in_=sG[:, 0, :])
                        continue
                    dlb = i16[:, 16 * CH:17 * CH].bitcast(BF16)
                    # alpha = <h_src, a_src> + aD[dst]
                    AS = p2s.tile([P, CH, HID], BF16, tag="AS")
                    nc.vector.tensor_mul(
                        out=AS[:, 0:ch_em, :], in0=G[:, 0:ch_em, :],
                        in1=asem_sb[:, None, :].to_broadcast([P, ch_em, HID]),
                    )
                    nc.vector.tensor_mul(
                        out=AS[:, ch_em:CH, :], in0=G[:, ch_em:CH, :],
                        in1=asmm_sb[:, None, :].to_broadcast(
                            [P, CH - ch_em, HID]),
                    )
                    AL = p2s.tile([P, CH, H], BF16, tag="AL")
                    with nc.allow_low_precision(
                            reason="16-elem head dot, 2e-2 tolerance"):
                        nc.vector.tensor_reduce(
                            out=AL[:, :, :, None],
                            in_=AS[:].rearrange("p c (h d) -> p c h d", d=D),
                            axis=mybir.AxisListType.X, op=mybir.AluOpType.add,
                        )
                    nc.vector.tensor_add(
                        out=AL[:, 0:ch_em, :], in0=AL[:, 0:ch_em, :],
                        in1=ADg[:, 0:ch_em, 0:8],
                    )
                    nc.vector.tensor_add(
                        out=AL[:, ch_em:CH, :], in0=AL[:, ch_em:CH, :],
                        in1=ADg[:, ch_em:CH, 8:16],
                    )
                    nc.vector.scalar_tensor_tensor(
                        out=AL[:], in0=AL[:], scalar=NEG, in1=AL[:],
                        op0=mybir.AluOpType.mult, op1=mybir.AluOpType.max,
                    )
                    Gw = p2s.tile([P, CH, 136], BF16, tag="Gw")
                    nc.scalar.activation(
                        out=Gw[:, :, 128:136], in_=AL[:],
                        func=mybir.ActivationFunctionType.Exp,
                    )
                    nc.vector.tensor_mul(
                        out=Gw[:, :, 0:128].rearrange(
                            "p c (e s) -> p c e s", s=D),
                        in0=G[:].rearrange("p c (e s) -> p c e s", s=D),
                        in1=Gw[:, :, 128:136, None].to_broadcast(
                            [P, CH, H, D]),
                    )
                    OH = p2s.tile([P, CH, P], BF16, tag="OH")
                    nc.vector.tensor_tensor(
                        out=OH[:],
                        in0=iot_sb[:, None, :].to_broadcast([P, CH, P]),
                        in1=dlb[:, :, None].to_broadcast([P, CH, P]),
                        op=mybir.AluOpType.is_equal,
                    )
                    pem = p2p.tile([P, 136], F32, tag="em")
                    pmm = p2p.tile([P, 136], F32, tag="mm")
                    for c in range(CH):
                        tgt = pem if c < ch_em else pmm
                        nc.tensor.matmul(
                            out=tgt[:],
                            lhsT=OH[:, c, :],
                            rhs=Gw[:, c, :],
                            start=(c == 0 or c == ch_em),
                            stop=(c == ch_em - 1 or c == CH - 1),
                        )
                    for mp, ps_, oTd in ((0, pem, oemT), (1, pmm, ommT)):
                        den = p2s.tile([P, 8], F32, tag=f"den{mp}")
                        nc.vector.tensor_scalar_add(
                            out=den[:], in0=ps_[:, 128:136], scalar1=1e-16
                        )
                        nc.vector.reciprocal(out=den[:], in_=den[:])
                        o_sb = p2s.tile([P, 128], F32, tag=f"o{mp}")
                        nc.vector.tensor_mul(
                            out=o_sb[:].rearrange("p (e s) -> p e s", s=D),
                            in0=ps_[:, 0:128].rearrange(
                                "p (e s) -> p e s", s=D),
                            in1=den[:, :, None].to_broadcast([P, H, D]),
                        )
                        ptr = p2t.tile([P, P], F32, tag="tr")
                        nc.tensor.transpose(
                            out=ptr[:], in_=o_sb[:], identity=ident[:]
                        )
                        oT_sb = p2s.tile([P, P], BF16, tag=f"oT{mp}")
                        nc.scalar.activation(
                            out=oT_sb[:],
                            in_=ptr[:],
                            func=mybir.ActivationFunctionType.Relu,
                        )
                        nc.sync.dma_start(
                            out=oTd[:, t * P:(t + 1) * P], in_=oT_sb[:]
                        )
                        nw = ND - (NT - 1) * P if t == NT - 1 else P
                        pk = p2k.tile([P, P], F32, tag="k")
                        nc.tensor.matmul(
                            out=pk[:, 0:nw], lhsT=wkT_sb[:], rhs=oT_sb[:, 0:nw],
                            start=True, stop=True,
                        )
                        tanh_sb = p2s.tile([P, P], F32, tag="tanh")
                        s_col = p2s.tile([P, 1], F32, tag="scol")
                        nc.scalar.activation(
                            out=tanh_sb[:, 0:nw],
                            in_=pk[:, 0:nw],
                            func=mybir.ActivationFunctionType.Tanh,
                            bias=bkc_sb[:, 0:1],
                            accum_out=s_col[:],
                        )
                        nc.vector.tensor_add(
                            out=S_sb[:, mp:mp + 1],
                            in0=S_sb[:, mp:mp + 1],
                            in1=s_col[:],
                        )

            # ================= P3: semantic attention + final =================
            if variant in ("full", "nogather", "nop2"):
              with (
                tc.tile_pool(name="p3s", bufs=3) as p3s,
                tc.tile_pool(name="p3p", bufs=2, space="PSUM") as p3p,
              ):
                nc.sync.dma_start(out=Sin_d[:], in_=S_sb[:])
                nc.gpsimd.collective_compute(
                    "AllReduce",
                    mybir.AluOpType.add,
                    replica_groups=[list(range(NCORES))],
                    ins=[Sin_d.opt()],
                    outs=[Sout_d.opt()],
                )
                Sr_sb = p3s.tile([HID, 2], F32, tag="Sr")
                nc.sync.dma_start(out=Sr_sb[:], in_=Sout_d[:])
                ps_s = p3p.tile([P, 2], F32, tag="s")
                nc.tensor.matmul(
                    out=ps_s[0:1, :], lhsT=qc_sb[:, 0:1], rhs=Sr_sb[:],
                    start=True, stop=True,
                )
                es = p3s.tile([P, 2], F32, tag="es")
                nc.scalar.activation(
                    out=es[0:1, :], in_=ps_s[0:1, :],
                    func=mybir.ActivationFunctionType.Exp,
                )
                ds = p3s.tile([P, 1], F32, tag="ds")
                nc.vector.tensor_reduce(
                    out=ds[0:1, :], in_=es[0:1, :],
                    axis=mybir.AxisListType.X, op=mybir.AluOpType.add,
                )
                nc.vector.reciprocal(out=ds[0:1, :], in_=ds[0:1, :])
                at = p3s.tile([P, 2], F32, tag="at")
                nc.vector.tensor_scalar_mul(
                    out=at[0:1, :], in0=es[0:1, :], scalar1=ds[0:1, 0:1]
                )
                pb = p3p.tile([P, 2], F32, tag="b")
                nc.tensor.matmul(
                    out=pb[:], lhsT=ones_sb[:], rhs=at[0:1, :],
                    start=True, stop=True,
                )
                ab = p3s.tile([P, 2], F32, tag="ab")
                nc.vector.tensor_copy(out=ab[:], in_=pb[:])
                for t in range(NT):
                    oe = p3s.tile([P, P], BF16, tag="oe")
                    om = p3s.tile([P, P], BF16, tag="om")
                    nc.sync.dma_start(out=oe[:], in_=oemT[:, t * P:(t + 1) * P])
                    nc.sync.dma_start(out=om[:], in_=ommT[:, t * P:(t + 1) * P])
                    comb = p3s.tile([P, P], BF16, tag="comb")
                    nc.vector.tensor_scalar_mul(
                        out=comb[:], in0=oe[:], scalar1=ab[:, 0:1]
                    )
                    nc.vector.scalar_tensor_tensor(
                        out=comb[:],
                        in0=om[:],
                        scalar=ab[:, 1:2],
                        in1=comb[:],
                        op0=mybir.AluOpType.mult,
                        op1=mybir.AluOpType.add,
                    )
                    py_ = p3p.tile([P, OUT], F32, tag="y")
                    nc.tensor.matmul(
                        out=py_[:], lhsT=comb[:], rhs=wlT_sb[:],
                        start=True, stop=True,
                    )
                    y_sb = p3s.tile([P, OUT], F32, tag="ysb")
                    nc.vector.tensor_add(out=y_sb[:], in0=py_[:], in1=blb_sb[:])
                    nc.sync.dma_start(out=y[t * P:(t + 1) * P, :], in_=y_sb[:])

    nc.compile()
    return nc


_CACHE = {}


def prep_all(inputs):
    host = build_host_tensors(inputs)
    e16, ch_em, chr_ = bucket_edges_v2(inputs)
    xTb = host["xT"]
    in_maps = []
    for c in range(NCORES):
        m = dict(host)
        m["e16"] = np.ascontiguousarray(e16[c])
        m["xlT"] = np.ascontiguousarray(xTb[:, c * ND:(c + 1) * ND])
        in_maps.append(m)
    return in_maps, ch_em, chr_


def kernel(**inputs):
    in_maps, ch_em, chr_ = prep_all(inputs)
    key = (ch_em, chr_)
    if key not in _CACHE:
        _CACHE[key] = build_program(ch_em, chr_)
    nc = _CACHE[key]
    res = run_bass_kernel_spmd(nc, in_maps, core_ids=list(range(NCORES)))
    out = np.empty((N_MAT, OUT), np.float32)
    for c in range(NCORES):
        out[c * ND:(c + 1) * ND] = res.results[c]["y"][:ND]
    return out



# revision 4
# speedup vs baseline: 2.7023x; 2.7023x over previous
"""HAN kernel v3 — dense-em + matmul-AD, mm-only gathers, dst-partitioned.

Strategy (dst-partitioned across 8 cores, core c owns mat rows
[c*12500, (c+1)*12500)):
  - P1 projects all mat nodes into 4 bf16 window tables Tm0..3 (dma_gather
    indices are int16); elem nodes (118) stay SBUF-resident. Per-dst
    attention dots: aD_mm lands directly in an SBUF table ADm_sb[128,98,8];
    aD_em is PE-transposed per 128-dst block and written to DRAM rows
    ADemR[98, 8*128] (f32) for later broadcast-loads. S_em[118,8] =
    <h_elem, a_src_em> via transpose+matmul; Rem[118,8,17] packs h_elem
    head-columns plus a ones column (for the softmax denominator).
  - P2 mm (edge-parallel): edges bucketed host-side by (core, dst_tile,
    src_window); per (tile,window) the gather count is the max over cores
    (static, SPMD-shared), shorter cores padded with idx=0 / dl=300
    sentinel edges; slots beyond the count are skipped by the DMA
    (descriptor savings). Per tile: 4 dma_gathers fetch h_src rows;
    alpha = <h,a_src_mm> (DVE mult+reduce) + aD_mm[dst] where the dst
    lookup is a one-hot-transpose matmul: OHT[j,c,p] = (dl[p,c]==j) built
    from a broadcast-DMA'd dl row, then OHT_c^T @ ADm_sb[:,t,:] in PSUM.
    leakyrelu+exp; weighted rows + exp cols scatter-add into PSUM via the
    one-hot matmul per chunk (numerator | denominator at once).
  - P2 em (dense): only 118 sources, so per tile the full [118 src x 128
    dst] attention matrix is computed densely: alpha = S_em + aD_em
    (broadcast-DMA row), lrelu, exp, times the host-built edge-count
    matrix CmT[t]; num|den via 8 per-head matmuls against Rem.
  - o = relu(num/den), PE-transpose, bf16 oT tables; tanh/semantic partial
    sums accumulate; P3: 1KB AllReduce, softmax over 2 metapaths, weighted
    combine, final linear to fp32.
"""

import numpy as np
import ml_dtypes

import concourse.bacc as bacc
import concourse.bass as bass
import concourse.mybir as mybir
import concourse.tile as tile
from concourse.bass_utils import run_bass_kernel_spmd
from concourse.masks import make_identity

P = 128
N_MAT = 100000
N_ELEM = 118
F_MAT = 128
F_ELEM = 64
HID = 128
H = 8
D = 16
OUT = 64
NCORES = 8
ND = N_MAT // NCORES          # 12500 dst rows per core
NT = (ND + P - 1) // P        # 98 dst tiles per core
NDP = NT * P                  # 12544 padded dst rows
NEG = 0.2
WIN = 32768                   # int16 index window
NWIN = 4
SLABS_PER_WIN = WIN // 1024   # 32
NTM_FULL = N_MAT // 1024      # 97 full 1024-row projection slabs
LAST_SLAB = N_MAT - NTM_FULL * 1024  # 672
NDL_FULL = ND // 1024         # 12 full local slabs
LAST_DL = ND - NDL_FULL * 1024       # 212
F32 = mybir.dt.float32
BF16 = mybir.dt.bfloat16
I16 = mybir.dt.int16

BF = ml_dtypes.bfloat16


def _bf(a):
    return np.asarray(a, dtype=BF)


def _blockdiag(a):
    """a [H, D] -> [HID, H] block diagonal so h @ A = per-head <h, a>."""
    A = np.zeros((HID, H), np.float32)
    for h in range(H):
        A[h * D:(h + 1) * D, h] = a[h]
    return A


def build_host_tensors(inputs):
    x_mat = inputs["x_mat"]
    WpT = np.ascontiguousarray(inputs["W_proj_mat"].T)       # [128f, 128k]
    WpeT = np.ascontiguousarray(inputs["W_proj_elem"].T)     # [64f, 128k]
    A_dem = _blockdiag(inputs["a_dst_em"])
    A_dmm = _blockdiag(inputs["a_dst_mm"])
    wad = np.concatenate([WpT @ A_dem, WpT @ A_dmm], axis=1)  # [128, 16]
    bad_row = np.concatenate(
        [inputs["b_proj_mat"] @ A_dem, inputs["b_proj_mat"] @ A_dmm])
    asmm = np.tile(inputs["a_src_mm"].reshape(1, HID), (P, 1))

    iotc = np.arange(P, dtype=np.float32)[:, None]           # column iota

    host = dict(
        xT=_bf(x_mat.T),                                     # [128, 100000]
        xeT=_bf(inputs["x_elem"].T),                         # [64, 118]
        whT=_bf(WpT),                                        # [128, 128]
        bh=np.tile(inputs["b_proj_mat"].astype(np.float32), (P, 1)),
        weT=_bf(WpeT),                                       # [64, 128]
        be=np.tile(inputs["b_proj_elem"].astype(np.float32), (P, 1)),
        wad=_bf(wad),                                        # [128, 16]
        bad=np.tile(bad_row.astype(np.float32), (P, 1)),     # [128, 16]
        asmm=_bf(asmm),
        aembd=_bf(_blockdiag(inputs["a_src_em"])),           # [128, 8]
        wkT=_bf(np.ascontiguousarray(inputs["Wk"].T)),
        bkc=inputs["bk"].astype(np.float32)[:, None],
        qc=(inputs["q"] / float(N_MAT)).astype(np.float32)[:, None],
        wlT=_bf(np.ascontiguousarray(inputs["Wl"].T)),
        blb=np.tile(inputs["bl"].astype(np.float32), (P, 1)),
        iot=_bf(np.tile(np.arange(P, dtype=np.float32), (P, 1))),
        iotc=_bf(iotc),
        ones=np.ones((1, P), np.float32),
    )
    return host


def bucket_edges_v3(inputs):
    """mm edges only: bucket by (core, dst_tile, src_window). Static per
    (tile, window) gather count = max over cores (>=16); shorter cores are
    padded with idx=0 / dl=300. Returns the int16 pack [NCORES, NT, P,
    9*CHM], the dl row tensor [NCORES, NT, CHM*128], counts c16 [NT, 4],
    and CHR. Also builds the em count matrices CmT [NCORES, NT, 118, 128]."""
    src = inputs["src_mm"].astype(np.int64)
    dst = inputs["dst_mm"].astype(np.int64)
    core = dst // ND
    rem = dst - core * ND
    tl = rem // P
    dl = rem % P
    w = src // WIN
    key = ((core * NT + tl) * NWIN + w)
    order = np.argsort(key, kind="stable")
    counts = np.bincount(key, minlength=NCORES * NT * NWIN)
    c4 = counts.reshape(NCORES, NT, NWIN)
    c16 = np.maximum(c4.max(axis=0), 16)                     # [NT, NWIN]
    CHR = int(-(-c16.max() // P))
    CHM = NWIN * CHR

    starts = np.zeros(NCORES * NT * NWIN, np.int64)
    starts[1:] = np.cumsum(counts)[:-1]
    ks = key[order]
    rank = np.arange(len(order)) - starts[ks]

    # valid-chunk repacking: window w of tile t occupies chunk slots
    # [voff[t][w], voff[t][w]+vc[t][w]) so downstream per-tile work only
    # covers vtot[t] = sum_w vc chunks (instead of the global max CHM).
    vc = -(-c16 // P)                                # [NT, NWIN]
    voff = np.zeros((NT, NWIN), np.int64)
    voff[:, 1:] = np.cumsum(vc, axis=1)[:, :-1]
    vtot = vc.sum(axis=1)                            # [NT]

    ct_ = ks // NWIN                       # core*NT + tl
    ws = ks % NWIN
    tls = ct_ % NT
    slot = (ct_ * CHM + voff[tls, ws]) * P + rank

    idxv = np.full(NCORES * NT * CHM * P, -1, np.int16)
    idxv[slot] = (src[order] - ws * WIN).astype(np.int16)
    dlv = np.full(NCORES * NT * CHM * P, 300.0, np.float32)
    dlv[slot] = dl[order]

    # pad [count, c16) with idx=0 (valid row, excluded via dl sentinel)
    idxv = idxv.reshape(NCORES, NT, CHM * P)
    for t in range(NT):
        for wi in range(NWIN):
            base = int(voff[t, wi]) * P
            tgt = int(c16[t, wi])
            for c in range(NCORES):
                n = int(c4[c, t, wi])
                if n < tgt:
                    idxv[c, t, base + n:base + tgt] = 0
    dlv = dlv.reshape(NCORES, NT, CHM * P)

    def wrap(a):
        # [..., n] -> [..., 128, n//16] (16-part wrap, replicated x8)
        sh = a.shape[:-1]
        n = a.shape[-1]
        w_ = a.reshape(*sh, n // 16, 16)
        w_ = np.swapaxes(w_, -1, -2)                 # [..., 16, n//16]
        w_ = np.broadcast_to(w_[..., None, :, :], (*sh, 8, 16, n // 16))
        return w_.reshape(*sh, P, n // 16)

    # per-window wrapped idx blocks at col offset 8*voff (width 8*vc)
    iw = np.full((NCORES, NT, P, 8 * CHM), -1, np.int16)
    for t in range(NT):
        for wi in range(NWIN):
            b, n = int(voff[t, wi]), int(vc[t, wi])
            iw[:, t, :, 8 * b:8 * (b + n)] = wrap(
                idxv[:, t, b * P:(b + n) * P])
    # dlb: [p, CHM] with col = global chunk slot
    dlb = _bf(dlv).view(np.int16).reshape(NCORES, NT, CHM, P)
    dlb = np.swapaxes(dlb, 2, 3)                          # [.., 128, CHM]
    e16 = np.concatenate([iw, dlb], axis=3)
    assert e16.shape == (NCORES, NT, P, 9 * CHM)

    dlr = _bf(dlv).reshape(NCORES, NT, CHM * P)

    # em dense count matrices: CmT[c, t, s, p] = #em edges (s -> dst c*ND+t*128+p)
    src_e = inputs["src_em"].astype(np.int64)
    dst_e = inputs["dst_em"].astype(np.int64)
    core_e = dst_e // ND
    rem_e = dst_e - core_e * ND
    flat = (core_e * NT + rem_e // P) * (N_ELEM * P) + src_e * P + rem_e % P
    cm = np.bincount(flat, minlength=NCORES * NT * N_ELEM * P)
    CmT = _bf(cm.reshape(NCORES, NT, N_ELEM, P))

    c16_t = tuple(tuple(int(x) for x in row) for row in c16)
    return (np.ascontiguousarray(e16), np.ascontiguousarray(dlr),
            np.ascontiguousarray(CmT), c16_t, CHR)


def build_program(c16, chr_, dbg=False, variant="full"):
    vparts = variant.split("-")
    vflags = set(vparts[1:])
    variant = vparts[0]
    CHM = NWIN * chr_
    nc = bacc.Bacc(
        "TRN2",
        target_bir_lowering=False,
        debug=False,
        enable_asserts=False,
        num_devices=NCORES,
    )

    inp = {}
    def din(name, shape, dt=F32):
        inp[name] = nc.dram_tensor(name, list(shape), dt, kind="ExternalInput").ap()
        return inp[name]

    xT = din("xT", [F_MAT, N_MAT], BF16)
    xeT = din("xeT", [F_ELEM, N_ELEM], BF16)
    xlT = din("xlT", [F_MAT, ND], BF16)
    whT = din("whT", [F_MAT, HID], BF16)
    bh = din("bh", [P, HID])
    weT = din("weT", [F_ELEM, HID], BF16)
    be = din("be", [P, HID])
    wad = din("wad", [F_MAT, 16], BF16)
    bad = din("bad", [P, 16])
    asmm = din("asmm", [P, HID], BF16)
    aembd = din("aembd", [P, H], BF16)
    e16 = din("e16", [NT, P, 9 * CHM], I16)
    dlr = din("dlr", [NT, CHM * P], BF16)
    CmT = din("CmT", [NT, N_ELEM, P], BF16)
    wkT = din("wkT", [HID, HID], BF16)
    bkc = din("bkc", [HID, 1])
    qc = din("qc", [HID, 1])
    wlT = din("wlT", [HID, OUT], BF16)
    blb = din("blb", [P, OUT])
    iot = din("iot", [P, P], BF16)
    iotc = din("iotc", [P, 1], BF16)
    ones = din("ones", [1, P])
    y = nc.dram_tensor("y", [NDP, OUT], F32, kind="ExternalOutput").ap()

    with tile.TileContext(nc) as tc:
        with (
            tc.tile_pool(name="const", bufs=1) as cp,
            tc.tile_pool(name="dram", bufs=1, space="DRAM") as dp,
        ):
            # ---- persistent DRAM tables ----
            Tm = [dp.tile([WIN, HID], BF16, name=f"Tm{r}")
                  for r in range(NWIN)]
            ADemR = dp.tile([NT, H * P], F32)
            oemT = dp.tile([HID, NDP], BF16)
            ommT = dp.tile([HID, NDP], BF16)
            Sin_d = dp.tile([HID, 2], F32)
            Sout_d = dp.tile([HID, 2], F32)

            # ---- constants in SBUF ----
            def lc(ap_in, shape, tag, dt=F32):
                t = cp.tile(list(shape), dt, tag=tag)
                nc.sync.dma_start(out=t[:], in_=ap_in[:])
                return t

            whT_sb = lc(whT, [F_MAT, HID], "whT", BF16)
            bh_sb = lc(bh, [P, HID], "bh")
            weT_sb = lc(weT, [F_ELEM, HID], "weT", BF16)
            be_sb = lc(be, [P, HID], "be")
            wad_sb = lc(wad, [F_MAT, 16], "wad", BF16)
            bad_sb = lc(bad, [P, 16], "bad")
            asmm_sb = lc(asmm, [P, HID], "asmm", BF16)
            aembd_sb = lc(aembd, [P, H], "aembd", BF16)
            wkT_sb = lc(wkT, [HID, HID], "wkT", BF16)
            bkc_sb = lc(bkc, [HID, 1], "bkc")
            qc_sb = lc(qc, [HID, 1], "qc")
            wlT_sb = lc(wlT, [HID, OUT], "wlT", BF16)
            blb_sb = lc(blb, [P, OUT], "blb")
            iot_sb = lc(iot, [P, P], "iot", BF16)
            iotc_sb = lc(iotc, [P, 1], "iotc", BF16)
            ones_sb = lc(ones, [1, P], "ones")
            ident = cp.tile([P, P], F32, tag="ident")
            make_identity(nc, ident[:])
            identb = cp.tile([P, P], BF16, tag="identb")
            nc.vector.tensor_copy(out=identb[:], in_=ident[:])
            S_sb = cp.tile([HID, 2], F32, tag="S")
            nc.gpsimd.memset(S_sb[:], 0.0)
            # persistent SBUF tables
            ADm_sb = cp.tile([P, NT, H], BF16, tag="ADm")
            nc.gpsimd.memset(ADm_sb[:], 0.0)
            Sem_sb = cp.tile([N_ELEM, H], F32, tag="Sem")
            Rem_sb = cp.tile([N_ELEM, H, 17], BF16, tag="Rem")

            # ================= P1: projections =================
            do_p1 = "nop1" not in vflags
            with (
                tc.tile_pool(name="p1s", bufs=3) as p1s,
                tc.tile_pool(name="p1p", bufs=2, space="PSUM") as p1p,
            ):
                for s in range(NTM_FULL + 1 if do_p1 else 0):
                    w = 1024 if s < NTM_FULL else LAST_SLAB
                    win = s // SLABS_PER_WIN
                    wrow = (s % SLABS_PER_WIN) * 1024
                    xsl = p1s.tile([P, 1024], BF16, tag="xsl")
                    nc.sync.dma_start(
                        out=xsl[:, 0:w], in_=xT[:, s * 1024: s * 1024 + w]
                    )
                    ev = p1s.tile([P, 8, HID], BF16, tag="ev")
                    ntile = (w + P - 1) // P
                    for j in range(ntile):
                        m = min(P, w - j * P)
                        ps = p1p.tile([P, HID], F32, tag="ps")
                        nc.tensor.matmul(
                            out=ps[0:m, :],
                            lhsT=xsl[:, j * P: j * P + m],
                            rhs=whT_sb[:],
                            start=True,
                            stop=True,
                        )
                        nc.vector.tensor_add(
                            out=ev[0:m, j, :], in0=ps[0:m, :], in1=bh_sb[0:m, :]
                        )
                    if s < NTM_FULL:
                        nc.sync.dma_start(
                            out=Tm[win][wrow: wrow + 1024, :].rearrange(
                                "(a p) e -> p a e", p=P
                            ),
                            in_=ev[:, :, :],
                        )
                    else:
                        for j in range(ntile):
                            m = min(P, w - j * P)
                            r0 = wrow + j * P
                            nc.sync.dma_start(
                                out=Tm[win][r0: r0 + m, :], in_=ev[0:m, j, :]
                            )
                # ---- elem projection + S_em + Rem ----
                xe_sb = p1s.tile([F_ELEM, N_ELEM], BF16, tag="xe")
                nc.sync.dma_start(out=xe_sb[:], in_=xeT[:])
                pse = p1p.tile([P, HID], F32, tag="ps")
                nc.tensor.matmul(
                    out=pse[0:N_ELEM, :],
                    lhsT=xe_sb[:],
                    rhs=weT_sb[:],
                    start=True,
                    stop=True,
                )
                eve = p1s.tile([P, HID], BF16, tag="eve")
                nc.vector.tensor_add(
                    out=eve[0:N_ELEM, :],
                    in0=pse[0:N_ELEM, :],
                    in1=be_sb[0:N_ELEM, :],
                )
                # Rem[s, h, 0:16] = h_elem head cols; [s, h, 16] = 1
                nc.vector.tensor_copy(
                    out=Rem_sb[:, :, 0:16],
                    in_=eve[0:N_ELEM, :].rearrange("s (h k) -> s h k", k=D),
                )
                nc.gpsimd.memset(Rem_sb[:, :, 16:17], 1.0)
                # S_em = h_elem @ blockdiag(a_src_em): transpose h_elem first
                ptr_e = p1p.tile([P, N_ELEM], BF16, tag="ptre")
                nc.tensor.transpose(
                    out=ptr_e[:, 0:N_ELEM], in_=eve[0:N_ELEM, 0:P],
                    identity=identb[0:N_ELEM, 0:N_ELEM],
                )
                heT = p1s.tile([P, N_ELEM], BF16, tag="heT")
                nc.vector.tensor_copy(out=heT[:], in_=ptr_e[:, 0:N_ELEM])
                ps_s = p1p.tile([P, H], F32, tag="pss")
                nc.tensor.matmul(
                    out=ps_s[0:N_ELEM, :], lhsT=heT[:], rhs=aembd_sb[:],
                    start=True, stop=True,
                )
                nc.vector.tensor_copy(
                    out=Sem_sb[:], in_=ps_s[0:N_ELEM, :]
                )
                # ---- local-dst attention dots ----
                for s in range(NDL_FULL + 1 if do_p1 else 0):
                    w = 1024 if s < NDL_FULL else LAST_DL
                    xsl = p1s.tile([P, 1024], BF16, tag="xsl")
                    nc.sync.dma_start(
                        out=xsl[:, 0:w], in_=xlT[:, s * 1024: s * 1024 + w]
                    )
                    evT = p1s.tile([H, 8, P], F32, tag="evT")
                    ntile = (w + P - 1) // P
                    for j in range(ntile):
                        m = min(P, w - j * P)
                        t_abs = s * 8 + j
                        ps = p1p.tile([P, HID], F32, tag="ps")
                        nc.tensor.matmul(
                            out=ps[0:m, 0:16],
                            lhsT=xsl[:, j * P: j * P + m],
                            rhs=wad_sb[:],
                            start=True,
                            stop=True,
                        )
                        ev2 = p1s.tile([P, 16], F32, tag="ev2")
                        nc.vector.tensor_add(
                            out=ev2[0:m, :], in0=ps[0:m, 0:16],
                            in1=bad_sb[0:m, :]
                        )
                        # mm half -> SBUF table (bf16)
                        nc.vector.tensor_copy(
                            out=ADm_sb[0:m, t_abs, :], in_=ev2[0:m, 8:16]
                        )
                        # em half -> transpose -> ADemR rows
                        ptr = p1p.tile([P, P], F32, tag="ptr")
                        nc.tensor.transpose(
                            out=ptr[0:16, 0:m], in_=ev2[0:m, 0:16],
                            identity=ident[0:m, 0:m],
                        )
                        nc.vector.tensor_copy(
                            out=evT[:, j, 0:m], in_=ptr[0:H, 0:m]
                        )
                        if m < P:
                            nc.gpsimd.memset(evT[:, j, m:P], 0.0)
                    nc.sync.dma_start(
                        out=ADemR[s * 8: s * 8 + ntile, :].rearrange(
                            "t (h d) -> h t d", h=H),
                        in_=evT[:, 0:ntile, :],
                    )

            # ================= P2: edge aggregation =================
            do_p2 = variant != "nop2"
            g_mm = variant in ("full", "nomem") or "gonly" in vflags
            do_em = variant in ("full", "nomm_g")
            if do_p2:
              with (
                tc.tile_pool(name="p2s", bufs=2) as p2s,
                tc.tile_pool(name="p2g", bufs=2) as p2g,
                tc.tile_pool(name="p2p", bufs=2, space="PSUM") as p2p,
                tc.tile_pool(name="p2a", bufs=2, space="PSUM") as p2a,
                tc.tile_pool(name="p2t", bufs=1, space="PSUM") as p2t,
              ):
                # per-tile valid-chunk geometry from the static counts
                vc_ = [[-(-c16[t][w] // P) for w in range(NWIN)]
                       for t in range(NT)]
                # zero both G rotation buffers once: slots skipped by the
                # gather read stale SBUF; stale must be finite bf16
                for _ in range(2):
                    Gz = p2g.tile([P, CHM, HID], BF16, tag="G")
                    nc.gpsimd.memset(Gz[:], 0.0)
                for t in range(4 if "nt4" in vflags else NT):
                    offs = [0]
                    for w in range(NWIN):
                        offs.append(offs[-1] + vc_[t][w])
                    VT = offs[-1]
                    i16 = p2s.tile([P, 9 * CHM], I16, tag="i16")
                    nc.sync.dma_start(out=i16[:], in_=e16[t, :, :])
                    dlb = i16[:, 8 * CHM:9 * CHM].bitcast(BF16)
                    dlR = p2s.tile([P, CHM * P], BF16, tag="dlR")
                    nc.sync.dma_start(
                        out=dlR[:, 0:VT * P],
                        in_=dlr[t:t + 1, 0:VT * P].to_broadcast([P, VT * P]),
                    )
                    G = p2g.tile([P, CHM, HID], BF16, tag="G")
                    if g_mm:
                        for r in range(NWIN):
                            b, n = offs[r], vc_[t][r]
                            nc.gpsimd.dma_gather(
                                G[:, b:b + n, :], Tm[r][:],
                                i16[:, 8 * b:8 * (b + n)],
                                n * P, c16[t][r], HID, single_packet=False,
                            )
                    else:
                        nc.gpsimd.memset(G[:], 0.0)
                    # alpha = <h_src, a_src_mm> + aD_mm[dst]
                    AS = p2g.tile([P, CHM, HID], BF16, tag="AS")
                    nc.vector.tensor_mul(
                        out=AS[:, 0:VT, :], in0=G[:, 0:VT, :],
                        in1=asmm_sb[:, None, :].to_broadcast([P, VT, HID]),
                    )
                    AL = p2s.tile([P, CHM, H], BF16, tag="AL")
                    with nc.allow_low_precision(
                            reason="16-elem head dot, 2e-2 tolerance"):
                        nc.vector.tensor_reduce(
                            out=AL[:, 0:VT, :, None],
                            in_=AS[:, 0:VT, :].rearrange(
                                "p c (h d) -> p c h d", d=D),
                            axis=mybir.AxisListType.X, op=mybir.AluOpType.add,
                        )
                    # OHT[j, c, p] = (dl[p, c] == j); AD lookup via matmul
                    OHT = p2g.tile([P, CHM, P], BF16, tag="OHT")
                    nc.vector.tensor_tensor(
                        out=OHT[:, 0:VT, :],
                        in0=iotc_sb[:, :, None].to_broadcast([P, VT, P]),
                        in1=dlR[:, 0:VT * P].rearrange("j (c p) -> j c p", p=P),
                        op=mybir.AluOpType.is_equal,
                    )
                    adp = p2a.tile([P, CHM, H], F32, tag="adp")
                    for c in range(VT):
                        nc.tensor.matmul(
                            out=adp[:, c, :], lhsT=OHT[:, c, :],
                            rhs=ADm_sb[:, t, :], start=True, stop=True,
                        )
                    nc.vector.tensor_add(
                        out=AL[:, 0:VT, :], in0=AL[:, 0:VT, :],
                        in1=adp[:, 0:VT, :])
                    nc.vector.scalar_tensor_tensor(
                        out=AL[:, 0:VT, :], in0=AL[:, 0:VT, :], scalar=NEG,
                        in1=AL[:, 0:VT, :],
                        op0=mybir.AluOpType.mult, op1=mybir.AluOpType.max,
                    )
                    Gw = p2g.tile([P, CHM, 136], BF16, tag="Gw")
                    nc.scalar.activation(
                        out=Gw[:, 0:VT, 128:136], in_=AL[:, 0:VT, :],
                        func=mybir.ActivationFunctionType.Exp,
                    )
                    nc.vector.tensor_mul(
                        out=Gw[:, 0:VT, 0:128].rearrange(
                            "p c (e s) -> p c e s", s=D),
                        in0=G[:, 0:VT, :].rearrange("p c (e s) -> p c e s", s=D),
                        in1=Gw[:, 0:VT, 128:136, None].to_broadcast(
                            [P, VT, H, D]),
                    )
                    OH = p2g.tile([P, CHM, P], BF16, tag="OH")
                    nc.vector.tensor_tensor(
                        out=OH[:, 0:VT, :],
                        in0=iot_sb[:, None, :].to_broadcast([P, VT, P]),
                        in1=dlb[:, 0:VT, None].to_broadcast([P, VT, P]),
                        op=mybir.AluOpType.is_equal,
                    )
                    pmm = p2p.tile([P, 136], F32, tag="mm")
                    for c in range(VT):
                        nc.tensor.matmul(
                            out=pmm[:],
                            lhsT=OH[:, c, :],
                            rhs=Gw[:, c, :],
                            start=(c == 0),
                            stop=(c == VT - 1),
                        )
                    # ---- em dense ----
                    pem = p2p.tile([P, H, 32], F32, tag="em")
                    if do_em:
                        ct = p2s.tile([N_ELEM, P], BF16, tag="ct")
                        nc.sync.dma_start(out=ct[:], in_=CmT[t, :, :])
                        adE = p2s.tile([N_ELEM, H, P], F32, tag="adE")
                        nc.sync.dma_start(
                            out=adE[:],
                            in_=ADemR[t:t + 1, :].to_broadcast(
                                [N_ELEM, H * P]).rearrange(
                                "s (h d) -> s h d", h=H),
                        )
                        alE = p2s.tile([N_ELEM, H, P], F32, tag="alE")
                        nc.vector.tensor_add(
                            out=alE[:], in0=adE[:],
                            in1=Sem_sb[:, :, None].to_broadcast([N_ELEM, H, P]),
                        )
                        nc.vector.scalar_tensor_tensor(
                            out=alE[:], in0=alE[:], scalar=NEG, in1=alE[:],
                            op0=mybir.AluOpType.mult, op1=mybir.AluOpType.max,
                        )
                        WE = p2s.tile([N_ELEM, H, P], BF16, tag="WE")
                        nc.scalar.activation(
                            out=WE[:], in_=alE[:],
                            func=mybir.ActivationFunctionType.Exp,
                        )
                        nc.vector.tensor_mul(
                            out=WE[:], in0=WE[:],
                            in1=ct[:, None, :].to_broadcast([N_ELEM, H, P]),
                        )
                        for h in range(H):
                            nc.tensor.matmul(
                                out=pem[:, h, 0:17], lhsT=WE[:, h, :],
                                rhs=Rem_sb[:, h, :], start=True, stop=True,
                            )
                    # ---- per-metapath epilogue ----
                    for mp, oTd in ((0, oemT), (1, ommT)):
                        den = p2s.tile([P, 8], F32, tag=f"den{mp}")
                        if mp == 0:
                            if not do_em:
                                continue
                            nc.vector.tensor_scalar_add(
                                out=den[:], in0=pem[:, :, 16], scalar1=1e-16
                            )
                        else:
                            nc.vector.tensor_scalar_add(
                                out=den[:], in0=pmm[:, 128:136], scalar1=1e-16
                            )
                        nc.vector.reciprocal(out=den[:], in_=den[:])
                        o_sb = p2s.tile([P, 128], F32, tag=f"o{mp}")
                        num_ap = (pem[:, :, 0:16] if mp == 0
                                  else pmm[:, 0:128].rearrange(
                                      "p (e s) -> p e s", s=D))
                        nc.vector.tensor_mul(
                            out=o_sb[:].rearrange("p (e s) -> p e s", s=D),
                            in0=num_ap,
                            in1=den[:, :, None].to_broadcast([P, H, D]),
                        )
                        ptr = p2t.tile([P, P], F32, tag="tr")
                        nc.tensor.transpose(
                            out=ptr[:], in_=o_sb[:], identity=ident[:]
                        )
                        oT_sb = p2s.tile([P, P], BF16, tag=f"oT{mp}")
                        nc.scalar.activation(
                            out=oT_sb[:],
                            in_=ptr[:],
                            func=mybir.ActivationFunctionType.Relu,
                        )
                        nc.sync.dma_start(
                            out=oTd[:, t * P:(t + 1) * P], in_=oT_sb[:]
                        )
                        nw = ND - (NT - 1) * P if t == NT - 1 else P
                        pk = p2t.tile([P, P], F32, tag="k")
                        nc.tensor.matmul(
                            out=pk[:, 0:nw], lhsT=wkT_sb[:], rhs=oT_sb[:, 0:nw],
                            start=True, stop=True,
                        )
                        tanh_sb = p2s.tile([P, P], F32, tag="tanh")
                        s_col = p2s.tile([P, 1], F32, tag="scol")
                        nc.scalar.activation(
                            out=tanh_sb[:, 0:nw],
                            in_=pk[:, 0:nw],
                            func=mybir.ActivationFunctionType.Tanh,
                            bias=bkc_sb[:, 0:1],
                            accum_out=s_col[:],
                        )
                        nc.vector.tensor_add(
                            out=S_sb[:, mp:mp + 1],
                            in0=S_sb[:, mp:mp + 1],
                            in1=s_col[:],
                        )

            # ================= P3: semantic attention + final =================
            if variant in ("full", "nop2"):
              with (
                tc.tile_pool(name="p3s", bufs=3) as p3s,
                tc.tile_pool(name="p3p", bufs=2, space="PSUM") as p3p,
              ):
                nc.sync.dma_start(out=Sin_d[:], in_=S_sb[:])
                nc.gpsimd.collective_compute(
                    "AllReduce",
                    mybir.AluOpType.add,
                    replica_groups=[list(range(NCORES))],
                    ins=[Sin_d.opt()],
                    outs=[Sout_d.opt()],
                )
                Sr_sb = p3s.tile([HID, 2], F32, tag="Sr")
                nc.sync.dma_start(out=Sr_sb[:], in_=Sout_d[:])
                ps_s = p3p.tile([P, 2], F32, tag="s")
                nc.tensor.matmul(
                    out=ps_s[0:1, :], lhsT=qc_sb[:, 0:1], rhs=Sr_sb[:],
                    start=True, stop=True,
                )
                es = p3s.tile([P, 2], F32, tag="es")
                nc.scalar.activation(
                    out=es[0:1, :], in_=ps_s[0:1, :],
                    func=mybir.ActivationFunctionType.Exp,
                )
                ds = p3s.tile([P, 1], F32, tag="ds")
                nc.vector.tensor_reduce(
                    out=ds[0:1, :], in_=es[0:1, :],
                    axis=mybir.AxisListType.X, op=mybir.AluOpType.add,
                )
                nc.vector.reciprocal(out=ds[0:1, :], in_=ds[0:1, :])
                at = p3s.tile([P, 2], F32, tag="at")
                nc.vector.tensor_scalar_mul(
                    out=at[0:1, :], in0=es[0:1, :], scalar1=ds[0:1, 0:1]
                )
                pb = p3p.tile([P, 2], F32, tag="b")
                nc.tensor.matmul(
                    out=pb[:], lhsT=ones_sb[:], rhs=at[0:1, :],
                    start=True, stop=True,
                )
                ab = p3s.tile([P, 2], F32, tag="ab")
                nc.vector.tensor_copy(out=ab[:], in_=pb[:])
                for t in range(NT):
                    oe = p3s.tile([P, P], BF16, tag="oe")
                    om = p3s.tile([P, P], BF16, tag="om")
                    nc.sync.dma_start(out=oe[:], in_=oemT[:, t * P:(t + 1) * P])
                    nc.sync.dma_start(out=om[:], in_=ommT[:, t * P:(t + 1) * P])
                    comb = p3s.tile([P, P], BF16, tag="comb")
                    nc.vector.tensor_scalar_mul(
                        out=comb[:], in0=oe[:], scalar1=ab[:, 0:1]
                    )
                    nc.vector.scalar_tensor_tensor(
                        out=comb[:],
                        in0=om[:],
                        scalar=ab[:, 1:2],
                        in1=comb[:],
                        op0=mybir.AluOpType.mult,
                        op1=mybir.AluOpType.add,
                    )
                    py_ = p3p.tile([P, OUT], F32, tag="y")
                    nc.tensor.matmul(
                        out=py_[:], lhsT=comb[:], rhs=wlT_sb[:],
                        start=True, stop=True,
                    )
                    y_sb = p3s.tile([P, OUT], F32, tag="ysb")
                    nc.vector.tensor_add(out=y_sb[:], in0=py_[:], in1=blb_sb[:])
                    nc.sync.dma_start(out=y[t * P:(t + 1) * P, :], in_=y_sb[:])

    nc.compile()
    return nc


_CACHE = {}


def prep_all(inputs):
    host = build_host_tensors(inputs)
    e16, dlr, CmT, c16, chr_ = bucket_edges_v3(inputs)
    xTb = host["xT"]
    in_maps = []
    for c in range(NCORES):
        m = dict(host)
        m["e16"] = np.ascontiguousarray(e16[c])
        m["dlr"] = np.ascontiguousarray(dlr[c])
        m["CmT"] = np.ascontiguousarray(CmT[c])
        m["xlT"] = np.ascontiguousarray(xTb[:, c * ND:(c + 1) * ND])
        in_maps.append(m)
    return in_maps, c16, chr_


def kernel(**inputs):
    in_maps, c16, chr_ = prep_all(inputs)
    key = (c16, chr_)
    if key not in _CACHE:
        _CACHE[key] = build_program(c16, chr_)
    nc = _CACHE[key]
    res = run_bass_kernel_spmd(nc, in_maps, core_ids=list(range(NCORES)))
    out = np.empty((N_MAT, OUT), np.float32)
    for c in range(NCORES):
        out[c * ND:(c + 1) * ND] = res.results[c]["y"][:ND]
    return out


# revision 16
# speedup vs baseline: 2.8512x; 1.0551x over previous
"""HAN kernel v3 — dense-em + matmul-AD, mm-only gathers, dst-partitioned.

Strategy (dst-partitioned across 8 cores, core c owns mat rows
[c*12500, (c+1)*12500)):
  - P1 projects all mat nodes into 4 bf16 window tables Tm0..3 (dma_gather
    indices are int16); elem nodes (118) stay SBUF-resident. Per-dst
    attention dots: aD_mm lands directly in an SBUF table ADm_sb[128,98,8];
    aD_em is PE-transposed per 128-dst block and written to DRAM rows
    ADemR[98, 8*128] (f32) for later broadcast-loads. S_em[118,8] =
    <h_elem, a_src_em> via transpose+matmul; Rem[118,8,17] packs h_elem
    head-columns plus a ones column (for the softmax denominator).
  - P2 mm (edge-parallel): edges bucketed host-side by (core, dst_tile,
    src_window); per (tile,window) the gather count is the max over cores
    (static, SPMD-shared), shorter cores padded with idx=0 / dl=300
    sentinel edges; slots beyond the count are skipped by the DMA
    (descriptor savings). Per tile: 4 dma_gathers fetch h_src rows;
    alpha = <h,a_src_mm> (DVE mult+reduce) + aD_mm[dst] where the dst
    lookup is a one-hot-transpose matmul: OHT[j,c,p] = (dl[p,c]==j) built
    from a broadcast-DMA'd dl row, then OHT_c^T @ ADm_sb[:,t,:] in PSUM.
    leakyrelu+exp; weighted rows + exp cols scatter-add into PSUM via the
    one-hot matmul per chunk (numerator | denominator at once).
  - P2 em (dense): only 118 sources, so per tile the full [118 src x 128
    dst] attention matrix is computed densely: alpha = S_em + aD_em
    (broadcast-DMA row), lrelu, exp, times the host-built edge-count
    matrix CmT[t]; num|den via 8 per-head matmuls against Rem.
  - o = relu(num/den), PE-transpose, bf16 oT tables; tanh/semantic partial
    sums accumulate; P3: 1KB AllReduce, softmax over 2 metapaths, weighted
    combine, final linear to fp32.
"""

import numpy as np
import ml_dtypes

import concourse.bacc as bacc
import concourse.bass as bass
import concourse.mybir as mybir
import concourse.tile as tile
from concourse.bass_utils import run_bass_kernel_spmd
from concourse.masks import make_identity

P = 128
N_MAT = 100000
N_ELEM = 118
F_MAT = 128
F_ELEM = 64
HID = 128
H = 8
D = 16
OUT = 64
NCORES = 8
ND = N_MAT // NCORES          # 12500 dst rows per core
NT = (ND + P - 1) // P        # 98 dst tiles per core
NDP = NT * P                  # 12544 padded dst rows
NEG = 0.2
WIN = 32768                   # int16 index window
NWIN = 4
SLABS_PER_WIN = WIN // 1024   # 32
NTM_FULL = N_MAT // 1024      # 97 full 1024-row projection slabs
LAST_SLAB = N_MAT - NTM_FULL * 1024  # 672
NDL_FULL = ND // 1024         # 12 full local slabs
LAST_DL = ND - NDL_FULL * 1024       # 212
F32 = mybir.dt.float32
BF16 = mybir.dt.bfloat16
I16 = mybir.dt.int16

BF = ml_dtypes.bfloat16


def _bf(a):
    return np.asarray(a, dtype=BF)


def _blockdiag(a):
    """a [H, D] -> [HID, H] block diagonal so h @ A = per-head <h, a>."""
    A = np.zeros((HID, H), np.float32)
    for h in range(H):
        A[h * D:(h + 1) * D, h] = a[h]
    return A


def build_host_tensors(inputs):
    x_mat = inputs["x_mat"]
    WpT = np.ascontiguousarray(inputs["W_proj_mat"].T)       # [128f, 128k]
    WpeT = np.ascontiguousarray(inputs["W_proj_elem"].T)     # [64f, 128k]
    A_dem = _blockdiag(inputs["a_dst_em"])
    A_dmm = _blockdiag(inputs["a_dst_mm"])
    wad = np.concatenate([WpT @ A_dem, WpT @ A_dmm], axis=1)  # [128, 16]
    bad_row = np.concatenate(
        [inputs["b_proj_mat"] @ A_dem, inputs["b_proj_mat"] @ A_dmm])
    asmm = np.tile(inputs["a_src_mm"].reshape(1, HID), (P, 1))

    iotc = np.arange(P, dtype=np.float32)[:, None]           # column iota

    host = dict(
        xT=_bf(x_mat.T),                                     # [128, 100000]
        xeT=_bf(inputs["x_elem"].T),                         # [64, 118]
        whT=_bf(WpT),                                        # [128, 128]
        bh=np.tile(inputs["b_proj_mat"].astype(np.float32), (P, 1)),
        weT=_bf(WpeT),                                       # [64, 128]
        be=np.tile(inputs["b_proj_elem"].astype(np.float32), (P, 1)),
        wad=_bf(wad),                                        # [128, 16]
        bad=np.tile(bad_row.astype(np.float32), (P, 1)),     # [128, 16]
        asmm=_bf(asmm),
        aembd=_bf(_blockdiag(inputs["a_src_em"])),           # [128, 8]
        wkT=_bf(np.ascontiguousarray(inputs["Wk"].T)),
        bkc=inputs["bk"].astype(np.float32)[:, None],
        qc=(inputs["q"] / float(N_MAT)).astype(np.float32)[:, None],
        wlT=_bf(np.ascontiguousarray(inputs["Wl"].T)),
        blb=np.tile(inputs["bl"].astype(np.float32), (P, 1)),
        iot=_bf(np.tile(np.arange(P, dtype=np.float32), (P, 1))),
        iotc=_bf(iotc),
        ones=np.ones((1, P), np.float32),
    )
    return host


def bucket_edges_v3(inputs):
    """mm edges only: bucket by (core, dst_tile, src_window). Static per
    (tile, window) gather count = max over cores (>=16); shorter cores are
    padded with idx=0 / dl=300. Returns the int16 pack [NCORES, NT, P,
    9*CHM], the dl row tensor [NCORES, NT, CHM*128], counts c16 [NT, 4],
    and CHR. Also builds the em count matrices CmT [NCORES, NT, 118, 128]."""
    src = inputs["src_mm"].astype(np.int64)
    dst = inputs["dst_mm"].astype(np.int64)
    core = dst // ND
    rem = dst - core * ND
    tl = rem // P
    dl = rem % P
    # gather-table row in the AllGather'd padded layout (NDP rows per core)
    src = (src // ND) * NDP + (src % ND)
    w = src // WIN
    key = ((core * NT + tl) * NWIN + w)
    order = np.argsort(key, kind="stable")
    counts = np.bincount(key, minlength=NCORES * NT * NWIN)
    c4 = counts.reshape(NCORES, NT, NWIN)
    c16 = np.maximum(c4.max(axis=0), 16)                     # [NT, NWIN]
    CHR = int(-(-c16.max() // P))
    CHM = NWIN * CHR

    starts = np.zeros(NCORES * NT * NWIN, np.int64)
    starts[1:] = np.cumsum(counts)[:-1]
    ks = key[order]
    rank = np.arange(len(order)) - starts[ks]

    # valid-chunk repacking: window w of tile t occupies chunk slots
    # [voff[t][w], voff[t][w]+vc[t][w]) so downstream per-tile work only
    # covers vtot[t] = sum_w vc chunks (instead of the global max CHM).
    vc = -(-c16 // P)                                # [NT, NWIN]
    voff = np.zeros((NT, NWIN), np.int64)
    voff[:, 1:] = np.cumsum(vc, axis=1)[:, :-1]
    vtot = vc.sum(axis=1)                            # [NT]

    ct_ = ks // NWIN                       # core*NT + tl
    ws = ks % NWIN
    tls = ct_ % NT
    slot = (ct_ * CHM + voff[tls, ws]) * P + rank

    idxv = np.full(NCORES * NT * CHM * P, -1, np.int16)
    idxv[slot] = (src[order] - ws * WIN).astype(np.int16)
    dlv = np.full(NCORES * NT * CHM * P, 300.0, np.float32)
    dlv[slot] = dl[order]

    # pad [count, c16) with idx=0 (valid row, excluded via dl sentinel)
    idxv = idxv.reshape(NCORES, NT, CHM * P)
    for t in range(NT):
        for wi in range(NWIN):
            base = int(voff[t, wi]) * P
            tgt = int(c16[t, wi])
            for c in range(NCORES):
                n = int(c4[c, t, wi])
                if n < tgt:
                    idxv[c, t, base + n:base + tgt] = 0
    dlv = dlv.reshape(NCORES, NT, CHM * P)

    def wrap(a):
        # [..., n] -> [..., 128, n//16] (16-part wrap, replicated x8)
        sh = a.shape[:-1]
        n = a.shape[-1]
        w_ = a.reshape(*sh, n // 16, 16)
        w_ = np.swapaxes(w_, -1, -2)                 # [..., 16, n//16]
        w_ = np.broadcast_to(w_[..., None, :, :], (*sh, 8, 16, n // 16))
        return w_.reshape(*sh, P, n // 16)

    # per-window wrapped idx blocks at col offset 8*voff (width 8*vc)
    iw = np.full((NCORES, NT, P, 8 * CHM), -1, np.int16)
    for t in range(NT):
        for wi in range(NWIN):
            b, n = int(voff[t, wi]), int(vc[t, wi])
            iw[:, t, :, 8 * b:8 * (b + n)] = wrap(
                idxv[:, t, b * P:(b + n) * P])
    # dlb: [p, CHM] with col = global chunk slot
    dlb = _bf(dlv).view(np.int16).reshape(NCORES, NT, CHM, P)
    dlb = np.swapaxes(dlb, 2, 3)                          # [.., 128, CHM]
    e16 = np.concatenate([iw, dlb], axis=3)
    assert e16.shape == (NCORES, NT, P, 9 * CHM)

    dlr = _bf(dlv).reshape(NCORES, NT, CHM * P)

    # em dense count matrices: CmT[c, t, s, p] = #em edges (s -> dst c*ND+t*128+p)
    src_e = inputs["src_em"].astype(np.int64)
    dst_e = inputs["dst_em"].astype(np.int64)
    core_e = dst_e // ND
    rem_e = dst_e - core_e * ND
    flat = (core_e * NT + rem_e // P) * (N_ELEM * P) + src_e * P + rem_e % P
    cm = np.bincount(flat, minlength=NCORES * NT * N_ELEM * P)
    CmT = _bf(cm.reshape(NCORES, NT, N_ELEM, P))

    c16_t = tuple(tuple(int(x) for x in row) for row in c16)
    return (np.ascontiguousarray(e16), np.ascontiguousarray(dlr),
            np.ascontiguousarray(CmT), c16_t, CHR)


def build_program(c16, chr_, dbg=False, variant="full"):
    vparts = variant.split("-")
    vflags = set(vparts[1:])
    variant = vparts[0]
    CHM = NWIN * chr_
    nc = bacc.Bacc(
        "TRN2",
        target_bir_lowering=False,
        debug=False,
        enable_asserts=False,
        num_devices=NCORES,
    )

    inp = {}
    def din(name, shape, dt=F32):
        inp[name] = nc.dram_tensor(name, list(shape), dt, kind="ExternalInput").ap()
        return inp[name]

    xeT = din("xeT", [F_ELEM, N_ELEM], BF16)
    xlT = din("xlT", [F_MAT, ND], BF16)
    whT = din("whT", [F_MAT, HID], BF16)
    bh = din("bh", [P, HID])
    weT = din("weT", [F_ELEM, HID], BF16)
    be = din("be", [P, HID])
    wad = din("wad", [F_MAT, 16], BF16)
    bad = din("bad", [P, 16])
    asmm = din("asmm", [P, HID], BF16)
    aembd = din("aembd", [P, H], BF16)
    e16 = din("e16", [NT, P, 9 * CHM], I16)
    dlr = din("dlr", [NT, CHM * P], BF16)
    CmT = din("CmT", [NT, N_ELEM, P], BF16)
    wkT = din("wkT", [HID, HID], BF16)
    bkc = din("bkc", [HID, 1])
    qc = din("qc", [HID, 1])
    wlT = din("wlT", [HID, OUT], BF16)
    blb = din("blb", [P, OUT])
    iot = din("iot", [P, P], BF16)
    iotc = din("iotc", [P, 1], BF16)
    ones = din("ones", [1, P])
    y = nc.dram_tensor("y", [NDP, OUT], F32, kind="ExternalOutput").ap()

    with tile.TileContext(nc) as tc:
        with (
            tc.tile_pool(name="const", bufs=1) as cp,
            tc.tile_pool(name="dram", bufs=1, space="DRAM") as dp,
        ):
            # ---- persistent DRAM tables ----
            AGin = dp.tile([NDP, HID], BF16)
            TmAll = dp.tile([NCORES * NDP, HID], BF16, addr_space="Shared")
            # gather windows are int16-addressable row-slices of TmAll
            Tm = [TmAll[WIN * r: min(WIN * (r + 1), NCORES * NDP), :]
                  for r in range(NWIN)]
            ADemR = dp.tile([NT, H * P], F32)
            Sin_d = dp.tile([HID, 2], F32)
            Sout_d = dp.tile([HID, 2], F32)

            # ---- constants in SBUF ----
            def lc(ap_in, shape, tag, dt=F32):
                t = cp.tile(list(shape), dt, tag=tag)
                nc.sync.dma_start(out=t[:], in_=ap_in[:])
                return t

            whT_sb = lc(whT, [F_MAT, HID], "whT", BF16)
            bh_sb = lc(bh, [P, HID], "bh")
            weT_sb = lc(weT, [F_ELEM, HID], "weT", BF16)
            be_sb = lc(be, [P, HID], "be")
            wad_sb = lc(wad, [F_MAT, 16], "wad", BF16)
            bad_sb = lc(bad, [P, 16], "bad")
            asmm_sb = lc(asmm, [P, HID], "asmm", BF16)
            aembd_sb = lc(aembd, [P, H], "aembd", BF16)
            wkT_sb = lc(wkT, [HID, HID], "wkT", BF16)
            bkc_sb = lc(bkc, [HID, 1], "bkc")
            qc_sb = lc(qc, [HID, 1], "qc")
            wlT_sb = lc(wlT, [HID, OUT], "wlT", BF16)
            blb_sb = lc(blb, [P, OUT], "blb")
            iot_sb = lc(iot, [P, P], "iot", BF16)
            iotc_sb = lc(iotc, [P, 1], "iotc", BF16)
            ones_sb = lc(ones, [1, P], "ones")
            ident = cp.tile([P, P], F32, tag="ident")
            make_identity(nc, ident[:])
            identb = cp.tile([P, P], BF16, tag="identb")
            nc.vector.tensor_copy(out=identb[:], in_=ident[:])
            S_sb = cp.tile([HID, 2], F32, tag="S")
            nc.gpsimd.memset(S_sb[:], 0.0)
            # persistent SBUF tables
            ADm_sb = cp.tile([P, NT, H], BF16, tag="ADm")
            nc.gpsimd.memset(ADm_sb[:], 0.0)
            Sem_sb = cp.tile([N_ELEM, H], F32, tag="Sem")
            Rem_sb = cp.tile([N_ELEM, H, 17], BF16, tag="Rem")
            # per-metapath outputs, transposed [HID, dst], SBUF-resident
            oem_sb = cp.tile([HID, NDP], BF16, tag="oem")
            omm_sb = cp.tile([HID, NDP], BF16, tag="omm")

            # ================= P1: projections =================
            # Distributed: each core projects only its local ND rows into
            # AGin, then an AllGather assembles the full node table TmAll
            # [NCORES*NDP, HID] every core gathers from. The per-dst
            # attention dots share the same xlT slab pass (one load, two
            # matmuls into one psum tile).
            do_p1 = "nop1" not in vflags
            with (
                tc.tile_pool(name="p1s", bufs=3) as p1s,
                tc.tile_pool(name="p1p", bufs=2, space="PSUM") as p1p,
            ):
                for s in range(NDL_FULL + 1 if do_p1 else 0):
                    w = 1024 if s < NDL_FULL else LAST_DL
                    xsl = p1s.tile([P, 1024], BF16, tag="xsl")
                    nc.sync.dma_start(
                        out=xsl[:, 0:w], in_=xlT[:, s * 1024: s * 1024 + w]
                    )
                    ev = p1s.tile([P, 8, HID], BF16, tag="ev")
                    evT = p1s.tile([H, 8, P], F32, tag="evT")
                    ntile = (w + P - 1) // P
                    for j in range(ntile):
                        m = min(P, w - j * P)
                        t_abs = s * 8 + j
                        ps = p1p.tile([P, HID + 16], F32, tag="ps")
                        nc.tensor.matmul(
                            out=ps[0:m, 0:HID],
                            lhsT=xsl[:, j * P: j * P + m],
                            rhs=whT_sb[:],
                            start=True,
                            stop=True,
                        )
                        nc.tensor.matmul(
                            out=ps[0:m, HID:HID + 16],
                            lhsT=xsl[:, j * P: j * P + m],
                            rhs=wad_sb[:],
                            start=True,
                            stop=True,
                        )
                        nc.vector.tensor_add(
                            out=ev[0:m, j, :], in0=ps[0:m, 0:HID],
                            in1=bh_sb[0:m, :]
                        )
                        ev2 = p1s.tile([P, 16], F32, tag="ev2")
                        nc.vector.tensor_add(
                            out=ev2[0:m, :], in0=ps[0:m, HID:HID + 16],
                            in1=bad_sb[0:m, :]
                        )
                        # mm half -> SBUF table (bf16)
                        nc.vector.tensor_copy(
                            out=ADm_sb[0:m, t_abs, :], in_=ev2[0:m, 8:16]
                        )
                        # em half -> transpose -> ADemR rows
                        ptr = p1p.tile([P, P], F32, tag="ptr")
                        nc.tensor.transpose(
                            out=ptr[0:16, 0:m], in_=ev2[0:m, 0:16],
                            identity=ident[0:m, 0:m],
                        )
                        nc.vector.tensor_copy(
                            out=evT[:, j, 0:m], in_=ptr[0:H, 0:m]
                        )
                        if m < P:
                            nc.gpsimd.memset(evT[:, j, m:P], 0.0)
                    if s < NDL_FULL:
                        nc.sync.dma_start(
                            out=AGin[s * 1024:(s + 1) * 1024, :].rearrange(
                                "(a p) e -> p a e", p=P
                            ),
                            in_=ev[:, :, :],
                        )
                    else:
                        for j in range(ntile):
                            m = min(P, w - j * P)
                            r0 = s * 1024 + j * P
                            nc.sync.dma_start(
                                out=AGin[r0: r0 + m, :], in_=ev[0:m, j, :]
                            )
                    nc.sync.dma_start(
                        out=ADemR[s * 8: s * 8 + ntile, :].rearrange(
                            "t (h d) -> h t d", h=H),
                        in_=evT[:, 0:ntile, :],
                    )
                if do_p1:
                    nc.gpsimd.collective_compute(
                        "AllGather",
                        mybir.AluOpType.bypass,
                        replica_groups=[list(range(NCORES))],
                        ins=[AGin.opt()],
                        outs=[TmAll.opt()],
                    )
                # ---- elem projection + S_em + Rem ----
                xe_sb = p1s.tile([F_ELEM, N_ELEM], BF16, tag="xe")
                nc.sync.dma_start(out=xe_sb[:], in_=xeT[:])
                pse = p1p.tile([P, HID], F32, tag="ps")
                nc.tensor.matmul(
                    out=pse[0:N_ELEM, :],
                    lhsT=xe_sb[:],
                    rhs=weT_sb[:],
                    start=True,
                    stop=True,
                )
                eve = p1s.tile([P, HID], BF16, tag="eve")
                nc.vector.tensor_add(
                    out=eve[0:N_ELEM, :],
                    in0=pse[0:N_ELEM, :],
                    in1=be_sb[0:N_ELEM, :],
                )
                # Rem[s, h, 0:16] = h_elem head cols; [s, h, 16] = 1
                nc.vector.tensor_copy(
                    out=Rem_sb[:, :, 0:16],
                    in_=eve[0:N_ELEM, :].rearrange("s (h k) -> s h k", k=D),
                )
                nc.gpsimd.memset(Rem_sb[:, :, 16:17], 1.0)
                # S_em = h_elem @ blockdiag(a_src_em): transpose h_elem first
                ptr_e = p1p.tile([P, N_ELEM], BF16, tag="ptre")
                nc.tensor.transpose(
                    out=ptr_e[:, 0:N_ELEM], in_=eve[0:N_ELEM, 0:P],
                    identity=identb[0:N_ELEM, 0:N_ELEM],
                )
                heT = p1s.tile([P, N_ELEM], BF16, tag="heT")
                nc.vector.tensor_copy(out=heT[:], in_=ptr_e[:, 0:N_ELEM])
                ps_s = p1p.tile([P, H], F32, tag="pss")
                nc.tensor.matmul(
                    out=ps_s[0:N_ELEM, :], lhsT=heT[:], rhs=aembd_sb[:],
                    start=True, stop=True,
                )
                nc.vector.tensor_copy(
                    out=Sem_sb[:], in_=ps_s[0:N_ELEM, :]
                )

            # ================= P2: edge aggregation =================
            do_p2 = variant != "nop2"
            g_mm = variant in ("full", "nomem") or "gonly" in vflags
            do_em = variant in ("full", "nomm_g")
            if do_p2:
              with (
                tc.tile_pool(name="p2s", bufs=2) as p2s,
                tc.tile_pool(name="p2g", bufs=2) as p2g,
                tc.tile_pool(name="p2p", bufs=2, space="PSUM") as p2p,
                tc.tile_pool(name="p2a", bufs=2, space="PSUM") as p2a,
                tc.tile_pool(name="p2t", bufs=1, space="PSUM") as p2t,
              ):
                # per-tile valid-chunk geometry from the static counts
                vc_ = [[-(-c16[t][w] // P) for w in range(NWIN)]
                       for t in range(NT)]
                # zero both G rotation buffers once: slots skipped by the
                # gather read stale SBUF; stale must be finite bf16
                for _ in range(2):
                    Gz = p2g.tile([P, CHM, HID], BF16, tag="G")
                    nc.gpsimd.memset(Gz[:], 0.0)
                for t in range(4 if "nt4" in vflags else NT):
                    offs = [0]
                    for w in range(NWIN):
                        offs.append(offs[-1] + vc_[t][w])
                    VT = offs[-1]
                    i16 = p2s.tile([P, 9 * CHM], I16, tag="i16")
                    nc.sync.dma_start(out=i16[:], in_=e16[t, :, :])
                    dlb = i16[:, 8 * CHM:9 * CHM].bitcast(BF16)
                    dlR = p2s.tile([P, CHM * P], BF16, tag="dlR")
                    nc.sync.dma_start(
                        out=dlR[:, 0:VT * P],
                        in_=dlr[t:t + 1, 0:VT * P].to_broadcast([P, VT * P]),
                    )
                    G = p2g.tile([P, CHM, HID], BF16, tag="G")
                    if g_mm:
                        for r in range(NWIN):
                            b, n = offs[r], vc_[t][r]
                            nc.gpsimd.dma_gather(
                                G[:, b:b + n, :], Tm[r],
                                i16[:, 8 * b:8 * (b + n)],
                                n * P, c16[t][r], HID, single_packet=False,
                            )
                    else:
                        nc.gpsimd.memset(G[:], 0.0)
                    # alpha = <h_src, a_src_mm> + aD_mm[dst]
                    AS = p2g.tile([P, CHM, HID], BF16, tag="AS")
                    nc.vector.tensor_mul(
                        out=AS[:, 0:VT, :], in0=G[:, 0:VT, :],
                        in1=asmm_sb[:, None, :].to_broadcast([P, VT, HID]),
                    )
                    AL = p2s.tile([P, CHM, H], BF16, tag="AL")
                    with nc.allow_low_precision(
                            reason="16-elem head dot, 2e-2 tolerance"):
                        nc.vector.tensor_reduce(
                            out=AL[:, 0:VT, :, None],
                            in_=AS[:, 0:VT, :].rearrange(
                                "p c (h d) -> p c h d", d=D),
                            axis=mybir.AxisListType.X, op=mybir.AluOpType.add,
                        )
                    # OHT[j, c, p] = (dl[p, c] == j); AD lookup via matmul
                    OHT = p2g.tile([P, CHM, P], BF16, tag="OHT")
                    nc.vector.tensor_tensor(
                        out=OHT[:, 0:VT, :],
                        in0=iotc_sb[:, :, None].to_broadcast([P, VT, P]),
                        in1=dlR[:, 0:VT * P].rearrange("j (c p) -> j c p", p=P),
                        op=mybir.AluOpType.is_equal,
                    )
                    adp = p2a.tile([P, CHM, H], F32, tag="adp")
                    for c in range(VT):
                        nc.tensor.matmul(
                            out=adp[:, c, :], lhsT=OHT[:, c, :],
                            rhs=ADm_sb[:, t, :], start=True, stop=True,
                        )
                    nc.vector.tensor_add(
                        out=AL[:, 0:VT, :], in0=AL[:, 0:VT, :],
                        in1=adp[:, 0:VT, :])
                    nc.vector.scalar_tensor_tensor(
                        out=AL[:, 0:VT, :], in0=AL[:, 0:VT, :], scalar=NEG,
                        in1=AL[:, 0:VT, :],
                        op0=mybir.AluOpType.mult, op1=mybir.AluOpType.max,
                    )
                    Gw = p2g.tile([P, CHM, 136], BF16, tag="Gw")
                    nc.scalar.activation(
                        out=Gw[:, 0:VT, 128:136], in_=AL[:, 0:VT, :],
                        func=mybir.ActivationFunctionType.Exp,
                    )
                    nc.vector.tensor_mul(
                        out=Gw[:, 0:VT, 0:128].rearrange(
                            "p c (e s) -> p c e s", s=D),
                        in0=G[:, 0:VT, :].rearrange("p c (e s) -> p c e s", s=D),
                        in1=Gw[:, 0:VT, 128:136, None].to_broadcast(
                            [P, VT, H, D]),
                    )
                    OH = p2g.tile([P, CHM, P], BF16, tag="OH")
                    nc.vector.tensor_tensor(
                        out=OH[:, 0:VT, :],
                        in0=iot_sb[:, None, :].to_broadcast([P, VT, P]),
                        in1=dlb[:, 0:VT, None].to_broadcast([P, VT, P]),
                        op=mybir.AluOpType.is_equal,
                    )
                    pmm = p2p.tile([P, 136], F32, tag="mm")
                    for c in range(VT):
                        nc.tensor.matmul(
                            out=pmm[:],
                            lhsT=OH[:, c, :],
                            rhs=Gw[:, c, :],
                            start=(c == 0),
                            stop=(c == VT - 1),
                        )
                    # ---- em dense ----
                    pem = p2p.tile([P, H, 32], F32, tag="em")
                    if do_em:
                        ct = p2s.tile([N_ELEM, P], BF16, tag="ct")
                        nc.sync.dma_start(out=ct[:], in_=CmT[t, :, :])
                        adE = p2s.tile([N_ELEM, H, P], F32, tag="adE")
                        nc.sync.dma_start(
                            out=adE[:],
                            in_=ADemR[t:t + 1, :].to_broadcast(
                                [N_ELEM, H * P]).rearrange(
                                "s (h d) -> s h d", h=H),
                        )
                        alE = p2s.tile([N_ELEM, H, P], F32, tag="alE")
                        nc.vector.tensor_add(
                            out=alE[:], in0=adE[:],
                            in1=Sem_sb[:, :, None].to_broadcast([N_ELEM, H, P]),
                        )
                        nc.vector.scalar_tensor_tensor(
                            out=alE[:], in0=alE[:], scalar=NEG, in1=alE[:],
                            op0=mybir.AluOpType.mult, op1=mybir.AluOpType.max,
                        )
                        WE = p2s.tile([N_ELEM, H, P], BF16, tag="WE")
                        nc.scalar.activation(
                            out=WE[:], in_=alE[:],
                            func=mybir.ActivationFunctionType.Exp,
                        )
                        nc.vector.tensor_mul(
                            out=WE[:], in0=WE[:],
                            in1=ct[:, None, :].to_broadcast([N_ELEM, H, P]),
                        )
                        for h in range(H):
                            nc.tensor.matmul(
                                out=pem[:, h, 0:17], lhsT=WE[:, h, :],
                                rhs=Rem_sb[:, h, :], start=True, stop=True,
                            )
                    # ---- per-metapath epilogue ----
                    for mp, oTd in ((0, oem_sb), (1, omm_sb)):
                        den = p2s.tile([P, 8], F32, tag=f"den{mp}")
                        if mp == 0:
                            if not do_em:
                                continue
                            nc.vector.tensor_scalar_add(
                                out=den[:], in0=pem[:, :, 16], scalar1=1e-16
                            )
                        else:
                            nc.vector.tensor_scalar_add(
                                out=den[:], in0=pmm[:, 128:136], scalar1=1e-16
                            )
                        nc.vector.reciprocal(out=den[:], in_=den[:])
                        o_sb = p2s.tile([P, 128], F32, tag=f"o{mp}")
                        num_ap = (pem[:, :, 0:16] if mp == 0
                                  else pmm[:, 0:128].rearrange(
                                      "p (e s) -> p e s", s=D))
                        nc.vector.tensor_mul(
                            out=o_sb[:].rearrange("p (e s) -> p e s", s=D),
                            in0=num_ap,
                            in1=den[:, :, None].to_broadcast([P, H, D]),
                        )
                        ptr = p2t.tile([P, P], F32, tag="tr")
                        nc.tensor.transpose(
                            out=ptr[:], in_=o_sb[:], identity=ident[:]
                        )
                        nc.scalar.activation(
                            out=oTd[:, t * P:(t + 1) * P],
                            in_=ptr[:],
                            func=mybir.ActivationFunctionType.Relu,
                        )
                        nw = ND - (NT - 1) * P if t == NT - 1 else P
                        pk = p2t.tile([P, P], F32, tag="k")
                        nc.tensor.matmul(
                            out=pk[:, 0:nw], lhsT=wkT_sb[:],
                            rhs=oTd[:, t * P:t * P + nw],
                            start=True, stop=True,
                        )
                        tanh_sb = p2s.tile([P, P], F32, tag="tanh")
                        s_col = p2s.tile([P, 1], F32, tag="scol")
                        nc.scalar.activation(
                            out=tanh_sb[:, 0:nw],
                            in_=pk[:, 0:nw],
                            func=mybir.ActivationFunctionType.Tanh,
                            bias=bkc_sb[:, 0:1],
                            accum_out=s_col[:],
                        )
                        nc.vector.tensor_add(
                            out=S_sb[:, mp:mp + 1],
                            in0=S_sb[:, mp:mp + 1],
                            in1=s_col[:],
                        )

            # ================= P3: semantic attention + final =================
            if variant in ("full", "nop2"):
              with (
                tc.tile_pool(name="p3s", bufs=3) as p3s,
                tc.tile_pool(name="p3p", bufs=2, space="PSUM") as p3p,
              ):
                nc.sync.dma_start(out=Sin_d[:], in_=S_sb[:])
                nc.gpsimd.collective_compute(
                    "AllReduce",
                    mybir.AluOpType.add,
                    replica_groups=[list(range(NCORES))],
                    ins=[Sin_d.opt()],
                    outs=[Sout_d.opt()],
                )
                Sr_sb = p3s.tile([HID, 2], F32, tag="Sr")
                nc.sync.dma_start(out=Sr_sb[:], in_=Sout_d[:])
                ps_s = p3p.tile([P, 2], F32, tag="s")
                nc.tensor.matmul(
                    out=ps_s[0:1, :], lhsT=qc_sb[:, 0:1], rhs=Sr_sb[:],
                    start=True, stop=True,
                )
                es = p3s.tile([P, 2], F32, tag="es")
                nc.scalar.activation(
                    out=es[0:1, :], in_=ps_s[0:1, :],
                    func=mybir.ActivationFunctionType.Exp,
                )
                ds = p3s.tile([P, 1], F32, tag="ds")
                nc.vector.tensor_reduce(
                    out=ds[0:1, :], in_=es[0:1, :],
                    axis=mybir.AxisListType.X, op=mybir.AluOpType.add,
                )
                nc.vector.reciprocal(out=ds[0:1, :], in_=ds[0:1, :])
                at = p3s.tile([P, 2], F32, tag="at")
                nc.vector.tensor_scalar_mul(
                    out=at[0:1, :], in0=es[0:1, :], scalar1=ds[0:1, 0:1]
                )
                pb = p3p.tile([P, 2], F32, tag="b")
                nc.tensor.matmul(
                    out=pb[:], lhsT=ones_sb[:], rhs=at[0:1, :],
                    start=True, stop=True,
                )
                ab = p3s.tile([P, 2], F32, tag="ab")
                nc.vector.tensor_copy(out=ab[:], in_=pb[:])
                # process tiles in groups of 4: one combine over [128, 512],
                # 4 matmuls, one batched y store
                TG = 4
                for g in range(0, NT, TG):
                    ng = min(TG, NT - g)
                    gw = ng * P
                    comb = p3s.tile([P, TG * P], BF16, tag="comb")
                    nc.vector.tensor_scalar_mul(
                        out=comb[:, 0:gw],
                        in0=oem_sb[:, g * P:g * P + gw], scalar1=ab[:, 0:1]
                    )
                    nc.vector.scalar_tensor_tensor(
                        out=comb[:, 0:gw],
                        in0=omm_sb[:, g * P:g * P + gw],
                        scalar=ab[:, 1:2],
                        in1=comb[:, 0:gw],
                        op0=mybir.AluOpType.mult,
                        op1=mybir.AluOpType.add,
                    )
                    y_sb = p3s.tile([P, TG, OUT], F32, tag="ysb")
                    for j in range(ng):
                        py_ = p3p.tile([P, OUT], F32, tag="y")
                        nc.tensor.matmul(
                            out=py_[:], lhsT=comb[:, j * P:(j + 1) * P],
                            rhs=wlT_sb[:], start=True, stop=True,
                        )
                        nc.vector.tensor_add(
                            out=y_sb[:, j, :], in0=py_[:], in1=blb_sb[:])
                    nc.sync.dma_start(
                        out=y[g * P:g * P + gw, :].rearrange(
                            "(a p) o -> p a o", p=P),
                        in_=y_sb[:, 0:ng, :])

    nc.compile()
    return nc


_CACHE = {}


def prep_all(inputs):
    host = build_host_tensors(inputs)
    e16, dlr, CmT, c16, chr_ = bucket_edges_v3(inputs)
    xTb = host.pop("xT")
    in_maps = []
    for c in range(NCORES):
        m = dict(host)
        m["e16"] = np.ascontiguousarray(e16[c])
        m["dlr"] = np.ascontiguousarray(dlr[c])
        m["CmT"] = np.ascontiguousarray(CmT[c])
        m["xlT"] = np.ascontiguousarray(xTb[:, c * ND:(c + 1) * ND])
        in_maps.append(m)
    return in_maps, c16, chr_


def kernel(**inputs):
    in_maps, c16, chr_ = prep_all(inputs)
    key = (c16, chr_)
    if key not in _CACHE:
        _CACHE[key] = build_program(c16, chr_)
    nc = _CACHE[key]
    res = run_bass_kernel_spmd(nc, in_maps, core_ids=list(range(NCORES)))
    out = np.empty((N_MAT, OUT), np.float32)
    for c in range(NCORES):
        out[c * ND:(c + 1) * ND] = res.results[c]["y"][:ND]
    return out


# revision 29
# speedup vs baseline: 3.3524x; 1.1758x over previous
"""HAN kernel v3 — dense-em + matmul-AD, mm-only gathers, dst-partitioned.

Strategy (dst-partitioned across 8 cores, core c owns mat rows
[c*12500, (c+1)*12500)):
  - P1 projects all mat nodes into 4 bf16 window tables Tm0..3 (dma_gather
    indices are int16); elem nodes (118) stay SBUF-resident. Per-dst
    attention dots: aD_mm lands directly in an SBUF table ADm_sb[128,98,8];
    aD_em is PE-transposed per 128-dst block and written to DRAM rows
    ADemR[98, 8*128] (f32) for later broadcast-loads. S_em[118,8] =
    <h_elem, a_src_em> via transpose+matmul; Rem[118,8,17] packs h_elem
    head-columns plus a ones column (for the softmax denominator).
  - P2 mm (edge-parallel): edges bucketed host-side by (core, dst_tile,
    src_window); per (tile,window) the gather count is the max over cores
    (static, SPMD-shared), shorter cores padded with idx=0 / dl=300
    sentinel edges; slots beyond the count are skipped by the DMA
    (descriptor savings). Per tile: 4 dma_gathers fetch h_src rows;
    alpha = <h,a_src_mm> (DVE mult+reduce) + aD_mm[dst] where the dst
    lookup is a one-hot-transpose matmul: OHT[j,c,p] = (dl[p,c]==j) built
    from a broadcast-DMA'd dl row, then OHT_c^T @ ADm_sb[:,t,:] in PSUM.
    leakyrelu+exp; weighted rows + exp cols scatter-add into PSUM via the
    one-hot matmul per chunk (numerator | denominator at once).
  - P2 em (dense): only 118 sources, so per tile the full [118 src x 128
    dst] attention matrix is computed densely: alpha = S_em + aD_em
    (broadcast-DMA row), lrelu, exp, times the host-built edge-count
    matrix CmT[t]; num|den via 8 per-head matmuls against Rem.
  - o = relu(num/den), PE-transpose, bf16 oT tables; tanh/semantic partial
    sums accumulate; P3: 1KB AllReduce, softmax over 2 metapaths, weighted
    combine, final linear to fp32.
"""

import numpy as np
import ml_dtypes

import concourse.bacc as bacc
import concourse.bass as bass
import concourse.mybir as mybir
import concourse.tile as tile
from concourse.bass_utils import run_bass_kernel_spmd
from concourse.masks import make_identity

P = 128
N_MAT = 100000
N_ELEM = 118
F_MAT = 128
F_ELEM = 64
HID = 128
H = 8
D = 16
OUT = 64
NCORES = 8
ND = N_MAT // NCORES          # 12500 dst rows per core
NT = (ND + P - 1) // P        # 98 dst tiles per core
NDP = NT * P                  # 12544 padded dst rows
NEG = 0.2
WIN = 32768                   # int16 index window
NWIN = 4
SLABS_PER_WIN = WIN // 1024   # 32
NTM_FULL = N_MAT // 1024      # 97 full 1024-row projection slabs
LAST_SLAB = N_MAT - NTM_FULL * 1024  # 672
NDL_FULL = ND // 1024         # 12 full local slabs
LAST_DL = ND - NDL_FULL * 1024       # 212
F32 = mybir.dt.float32
BF16 = mybir.dt.bfloat16
I16 = mybir.dt.int16

BF = ml_dtypes.bfloat16

# consolidated-constant column layouts (shared by host packing and program)
def _mkseg(widths):
    seg, c = {}, 0
    for n, w in widths:
        seg[n] = (c, c + w)
        c += w
    return seg, c


CB_SEG, CB_COLS = _mkseg([
    ("whT", HID), ("weT", HID), ("wad", 16), ("asmm", HID), ("aembd", H),
    ("wkT", HID), ("wlT", OUT), ("iot", P), ("iotc", 1), ("xeT", N_ELEM),
])
CF_SEG, CF_COLS = _mkseg([
    ("bh", HID), ("be", HID), ("bad", 16), ("bkc", 1), ("qc", 1),
    ("blb", OUT),
])


def pk_offsets(CHM):
    E0 = 0
    M0 = E0 + NT * P * 9 * CHM
    D0 = M0 + NT * N_ELEM * P
    X0 = D0 + NT * CHM * P
    TOT = X0 + F_MAT * ND
    return E0, M0, D0, X0, TOT


def _bf(a):
    return np.asarray(a, dtype=BF)


def _blockdiag(a):
    """a [H, D] -> [HID, H] block diagonal so h @ A = per-head <h, a>."""
    A = np.zeros((HID, H), np.float32)
    for h in range(H):
        A[h * D:(h + 1) * D, h] = a[h]
    return A


def build_host_tensors(inputs):
    x_mat = inputs["x_mat"]
    WpT = np.ascontiguousarray(inputs["W_proj_mat"].T)       # [128f, 128k]
    WpeT = np.ascontiguousarray(inputs["W_proj_elem"].T)     # [64f, 128k]
    A_dem = _blockdiag(inputs["a_dst_em"])
    A_dmm = _blockdiag(inputs["a_dst_mm"])
    wad = np.concatenate([WpT @ A_dem, WpT @ A_dmm], axis=1)  # [128, 16]
    bad_row = np.concatenate(
        [inputs["b_proj_mat"] @ A_dem, inputs["b_proj_mat"] @ A_dmm])
    asmm = np.tile(inputs["a_src_mm"].reshape(1, HID), (P, 1))

    bfp = dict(
        whT=_bf(WpT),                                        # [128, 128]
        weT=_bf(WpeT),                                       # [64, 128]
        wad=_bf(wad),                                        # [128, 16]
        asmm=_bf(asmm),
        aembd=_bf(_blockdiag(inputs["a_src_em"])),           # [128, 8]
        wkT=_bf(np.ascontiguousarray(inputs["Wk"].T)),
        wlT=_bf(np.ascontiguousarray(inputs["Wl"].T)),
        iot=_bf(np.tile(np.arange(P, dtype=np.float32), (P, 1))),
        iotc=_bf(np.arange(P, dtype=np.float32)[:, None]),
        xeT=_bf(inputs["x_elem"].T),                         # [64, 118]
    )
    f32p = dict(
        bh=np.tile(inputs["b_proj_mat"].astype(np.float32), (P, 1)),
        be=np.tile(inputs["b_proj_elem"].astype(np.float32), (P, 1)),
        bad=np.tile(bad_row.astype(np.float32), (P, 1)),     # [128, 16]
        bkc=inputs["bk"].astype(np.float32)[:, None],
        qc=(inputs["q"] / float(N_MAT)).astype(np.float32)[:, None],
        blb=np.tile(inputs["bl"].astype(np.float32), (P, 1)),
    )
    cb_arr = np.zeros((P, CB_COLS), BF)
    for n, a in bfp.items():
        c0, c1 = CB_SEG[n]
        cb_arr[0:a.shape[0], c0:c1] = a
    cf_arr = np.zeros((P, CF_COLS), np.float32)
    for n, a in f32p.items():
        c0, c1 = CF_SEG[n]
        cf_arr[0:a.shape[0], c0:c1] = a
    return dict(cb=cb_arr, cf=cf_arr, xT=_bf(x_mat.T))


def bucket_edges_v3(inputs):
    """mm edges only: bucket by (core, dst_tile, src_window). Static per
    (tile, window) gather count = max over cores (>=16); shorter cores are
    padded with idx=0 / dl=300. Returns the int16 pack [NCORES, NT, P,
    9*CHM], the dl row tensor [NCORES, NT, CHM*128], counts c16 [NT, 4],
    and CHR. Also builds the em count matrices CmT [NCORES, NT, 118, 128]."""
    src = inputs["src_mm"].astype(np.int64)
    dst = inputs["dst_mm"].astype(np.int64)
    core = dst // ND
    rem = dst - core * ND
    tl = rem // P
    dl = rem % P
    # gather-table row in the AllGather'd padded layout (NDP rows per core)
    src = (src // ND) * NDP + (src % ND)
    w = src // WIN
    key = ((core * NT + tl) * NWIN + w)
    order = np.argsort(key, kind="stable")
    counts = np.bincount(key, minlength=NCORES * NT * NWIN)
    c4 = counts.reshape(NCORES, NT, NWIN)
    c16 = np.maximum(c4.max(axis=0), 16)                     # [NT, NWIN]
    CHR = int(-(-c16.max() // P))
    CHM = NWIN * CHR

    starts = np.zeros(NCORES * NT * NWIN, np.int64)
    starts[1:] = np.cumsum(counts)[:-1]
    ks = key[order]
    rank = np.arange(len(order)) - starts[ks]

    # valid-chunk repacking: window w of tile t occupies chunk slots
    # [voff[t][w], voff[t][w]+vc[t][w]) so downstream per-tile work only
    # covers vtot[t] = sum_w vc chunks (instead of the global max CHM).
    vc = -(-c16 // P)                                # [NT, NWIN]
    voff = np.zeros((NT, NWIN), np.int64)
    voff[:, 1:] = np.cumsum(vc, axis=1)[:, :-1]
    vtot = vc.sum(axis=1)                            # [NT]

    ct_ = ks // NWIN                       # core*NT + tl
    ws = ks % NWIN
    tls = ct_ % NT
    slot = (ct_ * CHM + voff[tls, ws]) * P + rank

    idxv = np.full(NCORES * NT * CHM * P, -1, np.int16)
    idxv[slot] = (src[order] - ws * WIN).astype(np.int16)
    dlv = np.full(NCORES * NT * CHM * P, 300.0, np.float32)
    dlv[slot] = dl[order]

    # pad [count, c16) with idx=0 (valid row, excluded via dl sentinel)
    idxv = idxv.reshape(NCORES, NT, CHM * P)
    for t in range(NT):
        for wi in range(NWIN):
            base = int(voff[t, wi]) * P
            tgt = int(c16[t, wi])
            for c in range(NCORES):
                n = int(c4[c, t, wi])
                if n < tgt:
                    idxv[c, t, base + n:base + tgt] = 0
    dlv = dlv.reshape(NCORES, NT, CHM * P)

    def wrap(a):
        # [..., n] -> [..., 128, n//16] (16-part wrap, replicated x8)
        sh = a.shape[:-1]
        n = a.shape[-1]
        w_ = a.reshape(*sh, n // 16, 16)
        w_ = np.swapaxes(w_, -1, -2)                 # [..., 16, n//16]
        w_ = np.broadcast_to(w_[..., None, :, :], (*sh, 8, 16, n // 16))
        return w_.reshape(*sh, P, n // 16)

    # per-window wrapped idx blocks at col offset 8*voff (width 8*vc)
    iw = np.full((NCORES, NT, P, 8 * CHM), -1, np.int16)
    for t in range(NT):
        for wi in range(NWIN):
            b, n = int(voff[t, wi]), int(vc[t, wi])
            iw[:, t, :, 8 * b:8 * (b + n)] = wrap(
                idxv[:, t, b * P:(b + n) * P])
    # dlb: [p, CHM] with col = global chunk slot
    dlb = _bf(dlv).view(np.int16).reshape(NCORES, NT, CHM, P)
    dlb = np.swapaxes(dlb, 2, 3)                          # [.., 128, CHM]
    e16 = np.concatenate([iw, dlb], axis=3)
    assert e16.shape == (NCORES, NT, P, 9 * CHM)

    dlr = _bf(dlv).reshape(NCORES, NT, CHM * P)

    # em dense count matrices: CmT[c, t, s, p] = #em edges (s -> dst c*ND+t*128+p)
    src_e = inputs["src_em"].astype(np.int64)
    dst_e = inputs["dst_em"].astype(np.int64)
    core_e = dst_e // ND
    rem_e = dst_e - core_e * ND
    flat = (core_e * NT + rem_e // P) * (N_ELEM * P) + src_e * P + rem_e % P
    cm = np.bincount(flat, minlength=NCORES * NT * N_ELEM * P)
    CmT = _bf(cm.reshape(NCORES, NT, N_ELEM, P))

    c16_t = tuple(tuple(int(x) for x in row) for row in c16)
    return (np.ascontiguousarray(e16), np.ascontiguousarray(dlr),
            np.ascontiguousarray(CmT), c16_t, CHR)


def build_program(c16, chr_, dbg=False, variant="full"):
    vparts = variant.split("-")
    vflags = set(vparts[1:])
    variant = vparts[0]
    CHM = NWIN * chr_
    nc = bacc.Bacc(
        "TRN2",
        target_bir_lowering=False,
        debug=False,
        enable_asserts=False,
        num_devices=NCORES,
    )

    # three consolidated inputs (per-execution buffer binding costs ~55us
    # per input through the axon PJRT path, so pack everything):
    #   cb: shared bf16 constants [128, CB_COLS], column segments
    #   cf: shared f32 constants [128, CF_COLS]
    #   pk: per-core flat i16 blob (e16 | CmT | dlr | xlT bit patterns)
    cb = nc.dram_tensor("cb", [P, CB_COLS], BF16, kind="ExternalInput").ap()
    cf = nc.dram_tensor("cf", [P, CF_COLS], F32, kind="ExternalInput").ap()
    E0, M0, D0, X0, PK_TOT = pk_offsets(CHM)
    pkin = nc.dram_tensor("pk", [1, PK_TOT], I16, kind="ExternalInput").ap()
    y = nc.dram_tensor("y", [NDP, OUT], F32, kind="ExternalOutput").ap()

    def pk_seg(a, n):
        return pkin[0, a:a + n]

    xlT = pk_seg(X0, F_MAT * ND).rearrange("(f n) -> f n", n=ND).bitcast(BF16)

    with tile.TileContext(nc) as tc:
        with (
            tc.tile_pool(name="const", bufs=1) as cp,
            tc.tile_pool(name="dram", bufs=1, space="DRAM") as dp,
        ):
            # ---- persistent DRAM tables ----
            AGin = dp.tile([NDP, HID], BF16)
            TmAll = dp.tile([NCORES * NDP, HID], BF16, addr_space="Shared")
            # gather windows are int16-addressable row-slices of TmAll
            Tm = [TmAll[WIN * r: min(WIN * (r + 1), NCORES * NDP), :]
                  for r in range(NWIN)]
            ADemR = dp.tile([NT, H * P], F32)
            Sin_d = dp.tile([HID, 2], F32)
            Sout_d = dp.tile([HID, 2], F32)

            # ---- constants in SBUF: two bulk loads, then slices ----
            cb_sb = cp.tile([P, CB_COLS], BF16, tag="cb")
            nc.sync.dma_start(out=cb_sb[:], in_=cb[:])
            cf_sb = cp.tile([P, CF_COLS], F32, tag="cf")
            nc.sync.dma_start(out=cf_sb[:], in_=cf[:])

            def cbs(name, rows=P):
                a, b = CB_SEG[name]
                return cb_sb[0:rows, a:b]

            def cfs(name, rows=P):
                a, b = CF_SEG[name]
                return cf_sb[0:rows, a:b]

            whT_sb = cbs("whT")
            weT_sb = cbs("weT", F_ELEM)
            wad_sb = cbs("wad")
            asmm_sb = cbs("asmm")
            aembd_sb = cbs("aembd")
            wkT_sb = cbs("wkT")
            wlT_sb = cbs("wlT")
            iot_sb = cbs("iot")
            iotc_sb = cbs("iotc")
            xeT_sb = cbs("xeT", F_ELEM)
            bh_sb = cfs("bh")
            be_sb = cfs("be")
            bad_sb = cfs("bad")
            bkc_sb = cfs("bkc")
            qc_sb = cfs("qc")
            blb_sb = cfs("blb")
            ones_sb = cp.tile([1, P], F32, tag="ones")
            nc.vector.memset(ones_sb[:], 1.0)
            ident = cp.tile([P, P], F32, tag="ident")
            make_identity(nc, ident[:])
            identb = cp.tile([P, P], BF16, tag="identb")
            nc.vector.tensor_copy(out=identb[:], in_=ident[:])
            S_sb = cp.tile([HID, 2], F32, tag="S")
            nc.gpsimd.memset(S_sb[:], 0.0)
            # persistent SBUF tables
            ADm_sb = cp.tile([P, NT, H], BF16, tag="ADm")
            nc.gpsimd.memset(ADm_sb[:], 0.0)
            Sem_sb = cp.tile([N_ELEM, H], F32, tag="Sem")
            Rem_sb = cp.tile([N_ELEM, H, 17], BF16, tag="Rem")
            # per-metapath outputs, transposed [HID, dst], SBUF-resident
            oem_sb = cp.tile([HID, NDP], BF16, tag="oem")
            omm_sb = cp.tile([HID, NDP], BF16, tag="omm")
            nc.vector.memset(oem_sb[:], 0.0)
            nc.vector.memset(omm_sb[:], 0.0)

            # ================= P1: projections =================
            # Distributed: each core projects only its local ND rows into
            # AGin, then an AllGather assembles the full node table TmAll
            # [NCORES*NDP, HID] every core gathers from. The per-dst
            # attention dots share the same xlT slab pass (one load, two
            # matmuls into one psum tile).
            do_p1 = "nop1" not in vflags
            with (
                tc.tile_pool(name="p1s", bufs=3) as p1s,
                tc.tile_pool(name="p1p", bufs=2, space="PSUM") as p1p,
            ):
                for s in range(NDL_FULL + 1 if do_p1 else 0):
                    w = 1024 if s < NDL_FULL else LAST_DL
                    xsl = p1s.tile([P, 1024], BF16, tag="xsl")
                    nc.sync.dma_start(
                        out=xsl[:, 0:w], in_=xlT[:, s * 1024: s * 1024 + w]
                    )
                    ev = p1s.tile([P, 8, HID], BF16, tag="ev")
                    evT = p1s.tile([H, 8, P], F32, tag="evT")
                    ntile = (w + P - 1) // P
                    for j in range(ntile):
                        m = min(P, w - j * P)
                        t_abs = s * 8 + j
                        ps = p1p.tile([P, HID + 16], F32, tag="ps")
                        nc.tensor.matmul(
                            out=ps[0:m, 0:HID],
                            lhsT=xsl[:, j * P: j * P + m],
                            rhs=whT_sb,
                            start=True,
                            stop=True,
                        )
                        nc.tensor.matmul(
                            out=ps[0:m, HID:HID + 16],
                            lhsT=xsl[:, j * P: j * P + m],
                            rhs=wad_sb,
                            start=True,
                            stop=True,
                        )
                        nc.vector.tensor_add(
                            out=ev[0:m, j, :], in0=ps[0:m, 0:HID],
                            in1=bh_sb[0:m, :]
                        )
                        ev2 = p1s.tile([P, 16], F32, tag="ev2")
                        nc.vector.tensor_add(
                            out=ev2[0:m, :], in0=ps[0:m, HID:HID + 16],
                            in1=bad_sb[0:m, :]
                        )
                        # mm half -> SBUF table (bf16)
                        nc.vector.tensor_copy(
                            out=ADm_sb[0:m, t_abs, :], in_=ev2[0:m, 8:16]
                        )
                        # em half -> transpose -> ADemR rows
                        ptr = p1p.tile([P, P], F32, tag="ptr")
                        nc.tensor.transpose(
                            out=ptr[0:16, 0:m], in_=ev2[0:m, 0:16],
                            identity=ident[0:m, 0:m],
                        )
                        nc.vector.tensor_copy(
                            out=evT[:, j, 0:m], in_=ptr[0:H, 0:m]
                        )
                        if m < P:
                            nc.gpsimd.memset(evT[:, j, m:P], 0.0)
                    if s < NDL_FULL:
                        nc.sync.dma_start(
                            out=AGin[s * 1024:(s + 1) * 1024, :].rearrange(
                                "(a p) e -> p a e", p=P
                            ),
                            in_=ev[:, :, :],
                        )
                    else:
                        for j in range(ntile):
                            m = min(P, w - j * P)
                            r0 = s * 1024 + j * P
                            nc.sync.dma_start(
                                out=AGin[r0: r0 + m, :], in_=ev[0:m, j, :]
                            )
                    nc.sync.dma_start(
                        out=ADemR[s * 8: s * 8 + ntile, :].rearrange(
                            "t (h d) -> h t d", h=H),
                        in_=evT[:, 0:ntile, :],
                    )
                if do_p1:
                    nc.gpsimd.collective_compute(
                        "AllGather",
                        mybir.AluOpType.bypass,
                        replica_groups=[list(range(NCORES))],
                        ins=[AGin.opt()],
                        outs=[TmAll.opt()],
                    )
                # ---- elem projection + S_em + Rem ----
                pse = p1p.tile([P, HID], F32, tag="ps")
                nc.tensor.matmul(
                    out=pse[0:N_ELEM, :],
                    lhsT=xeT_sb,
                    rhs=weT_sb,
                    start=True,
                    stop=True,
                )
                eve = p1s.tile([P, HID], BF16, tag="eve")
                nc.vector.tensor_add(
                    out=eve[0:N_ELEM, :],
                    in0=pse[0:N_ELEM, :],
                    in1=be_sb[0:N_ELEM, :],
                )
                # Rem[s, h, 0:16] = h_elem head cols; [s, h, 16] = 1
                nc.vector.tensor_copy(
                    out=Rem_sb[:, :, 0:16],
                    in_=eve[0:N_ELEM, :].rearrange("s (h k) -> s h k", k=D),
                )
                nc.gpsimd.memset(Rem_sb[:, :, 16:17], 1.0)
                # S_em = h_elem @ blockdiag(a_src_em): transpose h_elem first
                ptr_e = p1p.tile([P, N_ELEM], BF16, tag="ptre")
                nc.tensor.transpose(
                    out=ptr_e[:, 0:N_ELEM], in_=eve[0:N_ELEM, 0:P],
                    identity=identb[0:N_ELEM, 0:N_ELEM],
                )
                heT = p1s.tile([P, N_ELEM], BF16, tag="heT")
                nc.vector.tensor_copy(out=heT[:], in_=ptr_e[:, 0:N_ELEM])
                ps_s = p1p.tile([P, H], F32, tag="pss")
                nc.tensor.matmul(
                    out=ps_s[0:N_ELEM, :], lhsT=heT[:], rhs=aembd_sb,
                    start=True, stop=True,
                )
                nc.vector.tensor_copy(
                    out=Sem_sb[:], in_=ps_s[0:N_ELEM, :]
                )

            # ================= P2: edge aggregation =================
            do_p2 = variant != "nop2"
            g_mm = variant in ("full", "nomem") or "gonly" in vflags
            do_em = variant in ("full", "nomm_g")
            if do_p2:
              with (
                tc.tile_pool(name="p2s", bufs=2) as p2s,
                tc.tile_pool(name="p2g", bufs=2) as p2g,
                tc.tile_pool(name="p2p", bufs=2, space="PSUM") as p2p,
                tc.tile_pool(name="p2a", bufs=2, space="PSUM") as p2a,
                tc.tile_pool(name="p2t", bufs=1, space="PSUM") as p2t,
              ):
                # per-tile valid-chunk geometry from the static counts
                vc_ = [[-(-c16[t][w] // P) for w in range(NWIN)]
                       for t in range(NT)]
                # zero both G rotation buffers once: slots skipped by the
                # gather read stale SBUF; stale must be finite bf16
                for _ in range(2):
                    Gz = p2g.tile([P, CHM, HID], BF16, tag="G")
                    nc.gpsimd.memset(Gz[:], 0.0)
                for t in range(4 if "nt4" in vflags else NT):
                    offs = [0]
                    for w in range(NWIN):
                        offs.append(offs[-1] + vc_[t][w])
                    VT = offs[-1]
                    i16 = p2s.tile([P, 9 * CHM], I16, tag="i16")
                    szE = P * 9 * CHM
                    nc.sync.dma_start(
                        out=i16[:],
                        in_=pk_seg(E0 + t * szE, szE).rearrange(
                            "(p w) -> p w", w=9 * CHM))
                    dlb = i16[:, 8 * CHM:9 * CHM].bitcast(BF16)
                    dlR = p2s.tile([P, CHM * P], BF16, tag="dlR")
                    nc.sync.dma_start(
                        out=dlR[:, 0:VT * P],
                        in_=pk_seg(D0 + t * CHM * P, VT * P).rearrange(
                            "(o n) -> o n", o=1).bitcast(BF16).to_broadcast(
                            [P, VT * P]),
                    )
                    G = p2g.tile([P, CHM, HID], BF16, tag="G")
                    if g_mm:
                        for r in range(NWIN):
                            b, n = offs[r], vc_[t][r]
                            nc.gpsimd.dma_gather(
                                G[:, b:b + n, :], Tm[r],
                                i16[:, 8 * b:8 * (b + n)],
                                n * P, c16[t][r], HID, single_packet=False,
                            )
                    else:
                        nc.gpsimd.memset(G[:], 0.0)
                    # alpha = <h_src, a_src_mm> + aD_mm[dst]
                    AS = p2g.tile([P, CHM, HID], BF16, tag="AS")
                    nc.vector.tensor_mul(
                        out=AS[:, 0:VT, :], in0=G[:, 0:VT, :],
                        in1=asmm_sb[:, None, :].to_broadcast([P, VT, HID]),
                    )
                    AL = p2s.tile([P, CHM, H], BF16, tag="AL")
                    with nc.allow_low_precision(
                            reason="16-elem head dot, 2e-2 tolerance"):
                        nc.vector.tensor_reduce(
                            out=AL[:, 0:VT, :, None],
                            in_=AS[:, 0:VT, :].rearrange(
                                "p c (h d) -> p c h d", d=D),
                            axis=mybir.AxisListType.X, op=mybir.AluOpType.add,
                        )
                    # OHT[j, c, p] = (dl[p, c] == j); AD lookup via matmul
                    OHT = p2g.tile([P, CHM, P], BF16, tag="OHT")
                    nc.vector.tensor_tensor(
                        out=OHT[:, 0:VT, :],
                        in0=iotc_sb[:, :, None].to_broadcast([P, VT, P]),
                        in1=dlR[:, 0:VT * P].rearrange("j (c p) -> j c p", p=P),
                        op=mybir.AluOpType.is_equal,
                    )
                    adp = p2a.tile([P, CHM, H], F32, tag="adp")
                    for c in range(VT):
                        nc.tensor.matmul(
                            out=adp[:, c, :], lhsT=OHT[:, c, :],
                            rhs=ADm_sb[:, t, :], start=True, stop=True,
                        )
                    nc.vector.tensor_add(
                        out=AL[:, 0:VT, :], in0=AL[:, 0:VT, :],
                        in1=adp[:, 0:VT, :])
                    nc.vector.scalar_tensor_tensor(
                        out=AL[:, 0:VT, :], in0=AL[:, 0:VT, :], scalar=NEG,
                        in1=AL[:, 0:VT, :],
                        op0=mybir.AluOpType.mult, op1=mybir.AluOpType.max,
                    )
                    Gw = p2g.tile([P, CHM, 136], BF16, tag="Gw")
                    nc.scalar.activation(
                        out=Gw[:, 0:VT, 128:136], in_=AL[:, 0:VT, :],
                        func=mybir.ActivationFunctionType.Exp,
                    )
                    nc.vector.tensor_mul(
                        out=Gw[:, 0:VT, 0:128].rearrange(
                            "p c (e s) -> p c e s", s=D),
                        in0=G[:, 0:VT, :].rearrange("p c (e s) -> p c e s", s=D),
                        in1=Gw[:, 0:VT, 128:136, None].to_broadcast(
                            [P, VT, H, D]),
                    )
                    OH = p2g.tile([P, CHM, P], BF16, tag="OH")
                    nc.vector.tensor_tensor(
                        out=OH[:, 0:VT, :],
                        in0=iot_sb[:, None, :].to_broadcast([P, VT, P]),
                        in1=dlb[:, 0:VT, None].to_broadcast([P, VT, P]),
                        op=mybir.AluOpType.is_equal,
                    )
                    pmm = p2p.tile([P, 136], F32, tag="mm")
                    for c in range(VT):
                        nc.tensor.matmul(
                            out=pmm[:],
                            lhsT=OH[:, c, :],
                            rhs=Gw[:, c, :],
                            start=(c == 0),
                            stop=(c == VT - 1),
                        )
                    # ---- em dense ----
                    pem = p2p.tile([P, H, 32], F32, tag="em")
                    if do_em:
                        ct = p2s.tile([N_ELEM, P], BF16, tag="ct")
                        szM = N_ELEM * P
                        nc.sync.dma_start(
                            out=ct[:],
                            in_=pk_seg(M0 + t * szM, szM).rearrange(
                                "(s d) -> s d", d=P).bitcast(BF16))
                        adE = p2s.tile([N_ELEM, H, P], F32, tag="adE")
                        nc.sync.dma_start(
                            out=adE[:],
                            in_=ADemR[t:t + 1, :].to_broadcast(
                                [N_ELEM, H * P]).rearrange(
                                "s (h d) -> s h d", h=H),
                        )
                        alE = p2s.tile([N_ELEM, H, P], F32, tag="alE")
                        nc.vector.tensor_add(
                            out=alE[:], in0=adE[:],
                            in1=Sem_sb[:, :, None].to_broadcast([N_ELEM, H, P]),
                        )
                        nc.vector.scalar_tensor_tensor(
                            out=alE[:], in0=alE[:], scalar=NEG, in1=alE[:],
                            op0=mybir.AluOpType.mult, op1=mybir.AluOpType.max,
                        )
                        WE = p2s.tile([N_ELEM, H, P], BF16, tag="WE")
                        nc.scalar.activation(
                            out=WE[:], in_=alE[:],
                            func=mybir.ActivationFunctionType.Exp,
                        )
                        nc.vector.tensor_mul(
                            out=WE[:], in0=WE[:],
                            in1=ct[:, None, :].to_broadcast([N_ELEM, H, P]),
                        )
                        for h in range(H):
                            nc.tensor.matmul(
                                out=pem[:, h, 0:17], lhsT=WE[:, h, :],
                                rhs=Rem_sb[:, h, :], start=True, stop=True,
                            )
                    # ---- per-metapath epilogue ----
                    for mp, oTd in ((0, oem_sb), (1, omm_sb)):
                        den = p2s.tile([P, 8], F32, tag=f"den{mp}")
                        if mp == 0:
                            if not do_em:
                                continue
                            nc.vector.tensor_scalar_add(
                                out=den[:], in0=pem[:, :, 16], scalar1=1e-16
                            )
                        else:
                            nc.vector.tensor_scalar_add(
                                out=den[:], in0=pmm[:, 128:136], scalar1=1e-16
                            )
                        nc.vector.reciprocal(out=den[:], in_=den[:])
                        o_sb = p2s.tile([P, 128], F32, tag=f"o{mp}")
                        num_ap = (pem[:, :, 0:16] if mp == 0
                                  else pmm[:, 0:128].rearrange(
                                      "p (e s) -> p e s", s=D))
                        nc.vector.tensor_mul(
                            out=o_sb[:].rearrange("p (e s) -> p e s", s=D),
                            in0=num_ap,
                            in1=den[:, :, None].to_broadcast([P, H, D]),
                        )
                        ptr = p2t.tile([P, P], F32, tag="tr")
                        nc.tensor.transpose(
                            out=ptr[:], in_=o_sb[:], identity=ident[:]
                        )
                        nc.scalar.activation(
                            out=oTd[:, t * P:(t + 1) * P],
                            in_=ptr[:],
                            func=mybir.ActivationFunctionType.Relu,
                        )
                        nw = ND - (NT - 1) * P if t == NT - 1 else P
                        pk = p2t.tile([P, P], F32, tag="k")
                        nc.tensor.matmul(
                            out=pk[:, 0:nw], lhsT=wkT_sb,
                            rhs=oTd[:, t * P:t * P + nw],
                            start=True, stop=True,
                        )
                        tanh_sb = p2s.tile([P, P], F32, tag="tanh")
                        s_col = p2s.tile([P, 1], F32, tag="scol")
                        nc.scalar.activation(
                            out=tanh_sb[:, 0:nw],
                            in_=pk[:, 0:nw],
                            func=mybir.ActivationFunctionType.Tanh,
                            bias=bkc_sb[:, 0:1],
                            accum_out=s_col[:],
                        )
                        nc.vector.tensor_add(
                            out=S_sb[:, mp:mp + 1],
                            in0=S_sb[:, mp:mp + 1],
                            in1=s_col[:],
                        )

            # ================= P3: semantic attention + final =================
            if variant in ("full", "nop2"):
              with (
                tc.tile_pool(name="p3s", bufs=3) as p3s,
                tc.tile_pool(name="p3p", bufs=2, space="PSUM") as p3p,
              ):
                nc.sync.dma_start(out=Sin_d[:], in_=S_sb[:])
                nc.gpsimd.collective_compute(
                    "AllReduce",
                    mybir.AluOpType.add,
                    replica_groups=[list(range(NCORES))],
                    ins=[Sin_d.opt()],
                    outs=[Sout_d.opt()],
                )
                Sr_sb = p3s.tile([HID, 2], F32, tag="Sr")
                nc.sync.dma_start(out=Sr_sb[:], in_=Sout_d[:])
                ps_s = p3p.tile([P, 2], F32, tag="s")
                nc.tensor.matmul(
                    out=ps_s[0:1, :], lhsT=qc_sb[:, 0:1], rhs=Sr_sb[:],
                    start=True, stop=True,
                )
                es = p3s.tile([P, 2], F32, tag="es")
                nc.scalar.activation(
                    out=es[0:1, :], in_=ps_s[0:1, :],
                    func=mybir.ActivationFunctionType.Exp,
                )
                ds = p3s.tile([P, 1], F32, tag="ds")
                nc.vector.tensor_reduce(
                    out=ds[0:1, :], in_=es[0:1, :],
                    axis=mybir.AxisListType.X, op=mybir.AluOpType.add,
                )
                nc.vector.reciprocal(out=ds[0:1, :], in_=ds[0:1, :])
                at = p3s.tile([P, 2], F32, tag="at")
                nc.vector.tensor_scalar_mul(
                    out=at[0:1, :], in0=es[0:1, :], scalar1=ds[0:1, 0:1]
                )
                pb = p3p.tile([P, 2], F32, tag="b")
                nc.tensor.matmul(
                    out=pb[:], lhsT=ones_sb[:], rhs=at[0:1, :],
                    start=True, stop=True,
                )
                ab = p3s.tile([P, 2], F32, tag="ab")
                nc.vector.tensor_copy(out=ab[:], in_=pb[:])
                # process tiles in groups of 4: one combine over [128, 512],
                # 4 matmuls, one batched y store
                TG = 4
                for g in range(0, NT, TG):
                    ng = min(TG, NT - g)
                    gw = ng * P
                    comb = p3s.tile([P, TG * P], BF16, tag="comb")
                    nc.vector.tensor_scalar_mul(
                        out=comb[:, 0:gw],
                        in0=oem_sb[:, g * P:g * P + gw], scalar1=ab[:, 0:1]
                    )
                    nc.vector.scalar_tensor_tensor(
                        out=comb[:, 0:gw],
                        in0=omm_sb[:, g * P:g * P + gw],
                        scalar=ab[:, 1:2],
                        in1=comb[:, 0:gw],
                        op0=mybir.AluOpType.mult,
                        op1=mybir.AluOpType.add,
                    )
                    y_sb = p3s.tile([P, TG, OUT], F32, tag="ysb")
                    for j in range(ng):
                        py_ = p3p.tile([P, OUT], F32, tag="y")
                        nc.tensor.matmul(
                            out=py_[:], lhsT=comb[:, j * P:(j + 1) * P],
                            rhs=wlT_sb, start=True, stop=True,
                        )
                        nc.vector.tensor_add(
                            out=y_sb[:, j, :], in0=py_[:], in1=blb_sb)
                    nc.sync.dma_start(
                        out=y[g * P:g * P + gw, :].rearrange(
                            "(a p) o -> p a o", p=P),
                        in_=y_sb[:, 0:ng, :])

    nc.compile()
    return nc


_CACHE = {}


def prep_all(inputs):
    host = build_host_tensors(inputs)
    e16, dlr, CmT, c16, chr_ = bucket_edges_v3(inputs)
    xTb = host.pop("xT")
    CHM = NWIN * chr_
    _, _, _, _, PK_TOT = pk_offsets(CHM)
    in_maps = []
    for c in range(NCORES):
        xl = np.ascontiguousarray(xTb[:, c * ND:(c + 1) * ND])
        pk = np.concatenate([
            e16[c].ravel(),
            CmT[c].view(np.int16).ravel(),
            dlr[c].view(np.int16).ravel(),
            xl.view(np.int16).ravel(),
        ])[None, :]
        assert pk.shape[1] == PK_TOT, (pk.shape, PK_TOT)
        in_maps.append(dict(host, pk=np.ascontiguousarray(pk)))
    return in_maps, c16, chr_


def kernel(**inputs):
    in_maps, c16, chr_ = prep_all(inputs)
    key = (c16, chr_)
    if key not in _CACHE:
        _CACHE[key] = build_program(c16, chr_)
    nc = _CACHE[key]
    res = run_bass_kernel_spmd(nc, in_maps, core_ids=list(range(NCORES)))
    out = np.empty((N_MAT, OUT), np.float32)
    for c in range(NCORES):
        out[c * ND:(c + 1) * ND] = res.results[c]["y"][:ND]
    return out


# revision 30
# speedup vs baseline: 3.5599x; 1.0619x over previous
"""HAN kernel v3 — dense-em + matmul-AD, mm-only gathers, dst-partitioned.

Strategy (dst-partitioned across 8 cores, core c owns mat rows
[c*12500, (c+1)*12500)):
  - P1 projects all mat nodes into 4 bf16 window tables Tm0..3 (dma_gather
    indices are int16); elem nodes (118) stay SBUF-resident. Per-dst
    attention dots: aD_mm lands directly in an SBUF table ADm_sb[128,98,8];
    aD_em is PE-transposed per 128-dst block and written to DRAM rows
    ADemR[98, 8*128] (f32) for later broadcast-loads. S_em[118,8] =
    <h_elem, a_src_em> via transpose+matmul; Rem[118,8,17] packs h_elem
    head-columns plus a ones column (for the softmax denominator).
  - P2 mm (edge-parallel): edges bucketed host-side by (core, dst_tile,
    src_window); per (tile,window) the gather count is the max over cores
    (static, SPMD-shared), shorter cores padded with idx=0 / dl=300
    sentinel edges; slots beyond the count are skipped by the DMA
    (descriptor savings). Per tile: 4 dma_gathers fetch h_src rows;
    alpha = <h,a_src_mm> (DVE mult+reduce) + aD_mm[dst] where the dst
    lookup is a one-hot-transpose matmul: OHT[j,c,p] = (dl[p,c]==j) built
    from a broadcast-DMA'd dl row, then OHT_c^T @ ADm_sb[:,t,:] in PSUM.
    leakyrelu+exp; weighted rows + exp cols scatter-add into PSUM via the
    one-hot matmul per chunk (numerator | denominator at once).
  - P2 em (dense): only 118 sources, so per tile the full [118 src x 128
    dst] attention matrix is computed densely: alpha = S_em + aD_em
    (broadcast-DMA row), lrelu, exp, times the host-built edge-count
    matrix CmT[t]; num|den via 8 per-head matmuls against Rem.
  - o = relu(num/den), PE-transpose, bf16 oT tables; tanh/semantic partial
    sums accumulate; P3: 1KB AllReduce, softmax over 2 metapaths, weighted
    combine, final linear to fp32.
"""

import numpy as np
import ml_dtypes

import concourse.bacc as bacc
import concourse.bass as bass
import concourse.mybir as mybir
import concourse.tile as tile
from concourse.bass_utils import run_bass_kernel_spmd
from concourse.masks import make_identity

P = 128
N_MAT = 100000
N_ELEM = 118
F_MAT = 128
F_ELEM = 64
HID = 128
H = 8
D = 16
OUT = 64
NCORES = 8
ND = N_MAT // NCORES          # 12500 dst rows per core
NT = (ND + P - 1) // P        # 98 dst tiles per core
NDP = NT * P                  # 12544 padded dst rows
NEG = 0.2
WIN = 32768                   # int16 index window
NWIN = 4
SLABS_PER_WIN = WIN // 1024   # 32
NTM_FULL = N_MAT // 1024      # 97 full 1024-row projection slabs
LAST_SLAB = N_MAT - NTM_FULL * 1024  # 672
NDL_FULL = ND // 1024         # 12 full local slabs
LAST_DL = ND - NDL_FULL * 1024       # 212
F32 = mybir.dt.float32
BF16 = mybir.dt.bfloat16
I16 = mybir.dt.int16

BF = ml_dtypes.bfloat16

# consolidated-constant column layouts (shared by host packing and program)
def _mkseg(widths):
    seg, c = {}, 0
    for n, w in widths:
        seg[n] = (c, c + w)
        c += w
    return seg, c


CB_SEG, CB_COLS = _mkseg([
    ("whT", HID), ("weT", HID), ("wad", 16), ("asmm", HID), ("aembd", H),
    ("wkT", HID), ("wlT", OUT), ("iot", P), ("iotc", 1), ("xeT", N_ELEM),
])
CF_SEG, CF_COLS = _mkseg([
    ("bh", HID), ("be", HID), ("bad", 16), ("bkc", 1), ("qc", 1),
    ("blb", OUT),
])


def pk_offsets(CHM):
    E0 = 0
    M0 = E0 + NT * P * 9 * CHM
    D0 = M0 + NT * N_ELEM * P
    X0 = D0 + NT * CHM * P
    TOT = X0 + F_MAT * ND
    return E0, M0, D0, X0, TOT


def _bf(a):
    return np.asarray(a, dtype=BF)


def _blockdiag(a):
    """a [H, D] -> [HID, H] block diagonal so h @ A = per-head <h, a>."""
    A = np.zeros((HID, H), np.float32)
    for h in range(H):
        A[h * D:(h + 1) * D, h] = a[h]
    return A


def build_host_tensors(inputs):
    x_mat = inputs["x_mat"]
    WpT = np.ascontiguousarray(inputs["W_proj_mat"].T)       # [128f, 128k]
    WpeT = np.ascontiguousarray(inputs["W_proj_elem"].T)     # [64f, 128k]
    A_dem = _blockdiag(inputs["a_dst_em"])
    A_dmm = _blockdiag(inputs["a_dst_mm"])
    wad = np.concatenate([WpT @ A_dem, WpT @ A_dmm], axis=1)  # [128, 16]
    bad_row = np.concatenate(
        [inputs["b_proj_mat"] @ A_dem, inputs["b_proj_mat"] @ A_dmm])
    asmm = np.tile(inputs["a_src_mm"].reshape(1, HID), (P, 1))

    bfp = dict(
        whT=_bf(WpT),                                        # [128, 128]
        weT=_bf(WpeT),                                       # [64, 128]
        wad=_bf(wad),                                        # [128, 16]
        asmm=_bf(asmm),
        aembd=_bf(_blockdiag(inputs["a_src_em"])),           # [128, 8]
        wkT=_bf(np.ascontiguousarray(inputs["Wk"].T)),
        wlT=_bf(np.ascontiguousarray(inputs["Wl"].T)),
        iot=_bf(np.tile(np.arange(P, dtype=np.float32), (P, 1))),
        iotc=_bf(np.arange(P, dtype=np.float32)[:, None]),
        xeT=_bf(inputs["x_elem"].T),                         # [64, 118]
    )
    f32p = dict(
        bh=np.tile(inputs["b_proj_mat"].astype(np.float32), (P, 1)),
        be=np.tile(inputs["b_proj_elem"].astype(np.float32), (P, 1)),
        bad=np.tile(bad_row.astype(np.float32), (P, 1)),     # [128, 16]
        bkc=inputs["bk"].astype(np.float32)[:, None],
        qc=(inputs["q"] / float(N_MAT)).astype(np.float32)[:, None],
        blb=np.tile(inputs["bl"].astype(np.float32), (P, 1)),
    )
    cb_arr = np.zeros((P, CB_COLS), BF)
    for n, a in bfp.items():
        c0, c1 = CB_SEG[n]
        cb_arr[0:a.shape[0], c0:c1] = a
    cf_arr = np.zeros((P, CF_COLS), np.float32)
    for n, a in f32p.items():
        c0, c1 = CF_SEG[n]
        cf_arr[0:a.shape[0], c0:c1] = a
    return dict(cb=cb_arr, cf=cf_arr, xT=_bf(x_mat.T))


def bucket_edges_v3(inputs):
    """mm edges only: bucket by (core, dst_tile, src_window). Static per
    (tile, window) gather count = max over cores (>=16); shorter cores are
    padded with idx=0 / dl=300. Returns the int16 pack [NCORES, NT, P,
    9*CHM], the dl row tensor [NCORES, NT, CHM*128], counts c16 [NT, 4],
    and CHR. Also builds the em count matrices CmT [NCORES, NT, 118, 128]."""
    src = inputs["src_mm"].astype(np.int64)
    dst = inputs["dst_mm"].astype(np.int64)
    core = dst // ND
    rem = dst - core * ND
    tl = rem // P
    dl = rem % P
    # gather-table row in the AllGather'd padded layout (NDP rows per core)
    src = (src // ND) * NDP + (src % ND)
    w = src // WIN
    key = ((core * NT + tl) * NWIN + w)
    order = np.argsort(key, kind="stable")
    counts = np.bincount(key, minlength=NCORES * NT * NWIN)
    c4 = counts.reshape(NCORES, NT, NWIN)
    c16 = np.maximum(c4.max(axis=0), 16)                     # [NT, NWIN]
    CHR = int(-(-c16.max() // P))
    CHM = NWIN * CHR

    starts = np.zeros(NCORES * NT * NWIN, np.int64)
    starts[1:] = np.cumsum(counts)[:-1]
    ks = key[order]
    rank = np.arange(len(order)) - starts[ks]

    # valid-chunk repacking: window w of tile t occupies chunk slots
    # [voff[t][w], voff[t][w]+vc[t][w]) so downstream per-tile work only
    # covers vtot[t] = sum_w vc chunks (instead of the global max CHM).
    vc = -(-c16 // P)                                # [NT, NWIN]
    voff = np.zeros((NT, NWIN), np.int64)
    voff[:, 1:] = np.cumsum(vc, axis=1)[:, :-1]
    vtot = vc.sum(axis=1)                            # [NT]

    ct_ = ks // NWIN                       # core*NT + tl
    ws = ks % NWIN
    tls = ct_ % NT
    slot = (ct_ * CHM + voff[tls, ws]) * P + rank

    idxv = np.full(NCORES * NT * CHM * P, -1, np.int16)
    idxv[slot] = (src[order] - ws * WIN).astype(np.int16)
    dlv = np.full(NCORES * NT * CHM * P, 300.0, np.float32)
    dlv[slot] = dl[order]

    # pad [count, c16) with idx=0 (valid row, excluded via dl sentinel)
    idxv = idxv.reshape(NCORES, NT, CHM * P)
    for t in range(NT):
        for wi in range(NWIN):
            base = int(voff[t, wi]) * P
            tgt = int(c16[t, wi])
            for c in range(NCORES):
                n = int(c4[c, t, wi])
                if n < tgt:
                    idxv[c, t, base + n:base + tgt] = 0
    dlv = dlv.reshape(NCORES, NT, CHM * P)

    def wrap(a):
        # [..., n] -> [..., 128, n//16] (16-part wrap, replicated x8)
        sh = a.shape[:-1]
        n = a.shape[-1]
        w_ = a.reshape(*sh, n // 16, 16)
        w_ = np.swapaxes(w_, -1, -2)                 # [..., 16, n//16]
        w_ = np.broadcast_to(w_[..., None, :, :], (*sh, 8, 16, n // 16))
        return w_.reshape(*sh, P, n // 16)

    # per-window wrapped idx blocks at col offset 8*voff (width 8*vc)
    iw = np.full((NCORES, NT, P, 8 * CHM), -1, np.int16)
    for t in range(NT):
        for wi in range(NWIN):
            b, n = int(voff[t, wi]), int(vc[t, wi])
            iw[:, t, :, 8 * b:8 * (b + n)] = wrap(
                idxv[:, t, b * P:(b + n) * P])
    # dlb: [p, CHM] with col = global chunk slot
    dlb = _bf(dlv).view(np.int16).reshape(NCORES, NT, CHM, P)
    dlb = np.swapaxes(dlb, 2, 3)                          # [.., 128, CHM]
    e16 = np.concatenate([iw, dlb], axis=3)
    assert e16.shape == (NCORES, NT, P, 9 * CHM)

    dlr = _bf(dlv).reshape(NCORES, NT, CHM * P)

    # em dense count matrices: CmT[c, t, s, p] = #em edges (s -> dst c*ND+t*128+p)
    src_e = inputs["src_em"].astype(np.int64)
    dst_e = inputs["dst_em"].astype(np.int64)
    core_e = dst_e // ND
    rem_e = dst_e - core_e * ND
    flat = (core_e * NT + rem_e // P) * (N_ELEM * P) + src_e * P + rem_e % P
    cm = np.bincount(flat, minlength=NCORES * NT * N_ELEM * P)
    CmT = _bf(cm.reshape(NCORES, NT, N_ELEM, P))

    c16_t = tuple(tuple(int(x) for x in row) for row in c16)
    return (np.ascontiguousarray(e16), np.ascontiguousarray(dlr),
            np.ascontiguousarray(CmT), c16_t, CHR)


def build_program(c16, chr_, dbg=False, variant="full"):
    vparts = variant.split("-")
    vflags = set(vparts[1:])
    variant = vparts[0]
    CHM = NWIN * chr_
    nc = bacc.Bacc(
        "TRN2",
        target_bir_lowering=False,
        debug=False,
        enable_asserts=False,
        num_devices=NCORES,
    )

    # three consolidated inputs (per-execution buffer binding costs ~55us
    # per input through the axon PJRT path, so pack everything):
    #   cb: shared bf16 constants [128, CB_COLS], column segments
    #   cf: shared f32 constants [128, CF_COLS]
    #   pk: per-core flat i16 blob (e16 | CmT | dlr | xlT bit patterns)
    cb = nc.dram_tensor("cb", [P, CB_COLS], BF16, kind="ExternalInput").ap()
    cf = nc.dram_tensor("cf", [P, CF_COLS], F32, kind="ExternalInput").ap()
    E0, M0, D0, X0, PK_TOT = pk_offsets(CHM)
    pkin = nc.dram_tensor("pk", [1, PK_TOT], I16, kind="ExternalInput").ap()
    y = nc.dram_tensor("y", [NDP, OUT], F32, kind="ExternalOutput").ap()

    def pk_seg(a, n):
        return pkin[0, a:a + n]

    xlT = pk_seg(X0, F_MAT * ND).rearrange("(f n) -> f n", n=ND).bitcast(BF16)

    with tile.TileContext(nc) as tc:
        with (
            tc.tile_pool(name="const", bufs=1) as cp,
            tc.tile_pool(name="dram", bufs=1, space="DRAM") as dp,
        ):
            # ---- persistent DRAM tables ----
            AGin = dp.tile([NDP, HID], BF16)
            TmAll = dp.tile([NCORES * NDP, HID], BF16, addr_space="Shared")
            # gather windows are int16-addressable row-slices of TmAll
            Tm = [TmAll[WIN * r: min(WIN * (r + 1), NCORES * NDP), :]
                  for r in range(NWIN)]
            ADemR = dp.tile([NT, H * P], F32)
            Sin_d = dp.tile([HID, 2], F32)
            Sout_d = dp.tile([HID, 2], F32)

            # ---- constants in SBUF: two bulk loads, then slices ----
            cb_sb = cp.tile([P, CB_COLS], BF16, tag="cb")
            nc.sync.dma_start(out=cb_sb[:], in_=cb[:])
            cf_sb = cp.tile([P, CF_COLS], F32, tag="cf")
            nc.sync.dma_start(out=cf_sb[:], in_=cf[:])

            def cbs(name, rows=P):
                a, b = CB_SEG[name]
                return cb_sb[0:rows, a:b]

            def cfs(name, rows=P):
                a, b = CF_SEG[name]
                return cf_sb[0:rows, a:b]

            whT_sb = cbs("whT")
            weT_sb = cbs("weT", F_ELEM)
            wad_sb = cbs("wad")
            asmm_sb = cbs("asmm")
            aembd_sb = cbs("aembd")
            wkT_sb = cbs("wkT")
            wlT_sb = cbs("wlT")
            iot_sb = cbs("iot")
            iotc_sb = cbs("iotc")
            xeT_sb = cbs("xeT", F_ELEM)
            bh_sb = cfs("bh")
            be_sb = cfs("be")
            bad_sb = cfs("bad")
            bkc_sb = cfs("bkc")
            qc_sb = cfs("qc")
            blb_sb = cfs("blb")
            ones_sb = cp.tile([1, P], F32, tag="ones")
            nc.vector.memset(ones_sb[:], 1.0)
            ident = cp.tile([P, P], F32, tag="ident")
            make_identity(nc, ident[:])
            identb = cp.tile([P, P], BF16, tag="identb")
            nc.vector.tensor_copy(out=identb[:], in_=ident[:])
            S_sb = cp.tile([HID, 2], F32, tag="S")
            nc.gpsimd.memset(S_sb[:], 0.0)
            # persistent SBUF tables
            ADm_sb = cp.tile([P, NT, H], BF16, tag="ADm")
            nc.gpsimd.memset(ADm_sb[:], 0.0)
            Sem_sb = cp.tile([N_ELEM, H], F32, tag="Sem")
            Rem_sb = cp.tile([N_ELEM, H, 17], BF16, tag="Rem")
            # per-metapath outputs, transposed [HID, dst], SBUF-resident
            oem_sb = cp.tile([HID, NDP], BF16, tag="oem")
            omm_sb = cp.tile([HID, NDP], BF16, tag="omm")
            nc.vector.memset(oem_sb[:], 0.0)
            nc.vector.memset(omm_sb[:], 0.0)

            # ================= P1: projections =================
            # Distributed: each core projects only its local ND rows into
            # AGin, then an AllGather assembles the full node table TmAll
            # [NCORES*NDP, HID] every core gathers from. The per-dst
            # attention dots share the same xlT slab pass (one load, two
            # matmuls into one psum tile).
            do_p1 = "nop1" not in vflags
            with (
                tc.tile_pool(name="p1s", bufs=3) as p1s,
                tc.tile_pool(name="p1p", bufs=2, space="PSUM") as p1p,
            ):
                for s in range(NDL_FULL + 1 if do_p1 else 0):
                    w = 1024 if s < NDL_FULL else LAST_DL
                    xsl = p1s.tile([P, 1024], BF16, tag="xsl")
                    nc.sync.dma_start(
                        out=xsl[:, 0:w], in_=xlT[:, s * 1024: s * 1024 + w]
                    )
                    ev = p1s.tile([P, 8, HID], BF16, tag="ev")
                    evT = p1s.tile([H, 8, P], F32, tag="evT")
                    ntile = (w + P - 1) // P
                    for j in range(ntile):
                        m = min(P, w - j * P)
                        t_abs = s * 8 + j
                        ps = p1p.tile([P, HID + 16], F32, tag="ps")
                        nc.tensor.matmul(
                            out=ps[0:m, 0:HID],
                            lhsT=xsl[:, j * P: j * P + m],
                            rhs=whT_sb,
                            start=True,
                            stop=True,
                        )
                        nc.tensor.matmul(
                            out=ps[0:m, HID:HID + 16],
                            lhsT=xsl[:, j * P: j * P + m],
                            rhs=wad_sb,
                            start=True,
                            stop=True,
                        )
                        nc.vector.tensor_add(
                            out=ev[0:m, j, :], in0=ps[0:m, 0:HID],
                            in1=bh_sb[0:m, :]
                        )
                        ev2 = p1s.tile([P, 16], F32, tag="ev2")
                        nc.vector.tensor_add(
                            out=ev2[0:m, :], in0=ps[0:m, HID:HID + 16],
                            in1=bad_sb[0:m, :]
                        )
                        # mm half -> SBUF table (bf16)
                        nc.vector.tensor_copy(
                            out=ADm_sb[0:m, t_abs, :], in_=ev2[0:m, 8:16]
                        )
                        # em half -> transpose -> ADemR rows
                        ptr = p1p.tile([P, P], F32, tag="ptr")
                        nc.tensor.transpose(
                            out=ptr[0:16, 0:m], in_=ev2[0:m, 0:16],
                            identity=ident[0:m, 0:m],
                        )
                        nc.vector.tensor_copy(
                            out=evT[:, j, 0:m], in_=ptr[0:H, 0:m]
                        )
                        if m < P:
                            nc.gpsimd.memset(evT[:, j, m:P], 0.0)
                    if s < NDL_FULL:
                        nc.sync.dma_start(
                            out=AGin[s * 1024:(s + 1) * 1024, :].rearrange(
                                "(a p) e -> p a e", p=P
                            ),
                            in_=ev[:, :, :],
                        )
                    else:
                        for j in range(ntile):
                            m = min(P, w - j * P)
                            r0 = s * 1024 + j * P
                            nc.sync.dma_start(
                                out=AGin[r0: r0 + m, :], in_=ev[0:m, j, :]
                            )
                    nc.sync.dma_start(
                        out=ADemR[s * 8: s * 8 + ntile, :].rearrange(
                            "t (h d) -> h t d", h=H),
                        in_=evT[:, 0:ntile, :],
                    )
                if do_p1:
                    nc.gpsimd.collective_compute(
                        "AllGather",
                        mybir.AluOpType.bypass,
                        replica_groups=[list(range(NCORES))],
                        ins=[AGin.opt()],
                        outs=[TmAll.opt()],
                    )
                # ---- elem projection + S_em + Rem ----
                pse = p1p.tile([P, HID], F32, tag="ps")
                nc.tensor.matmul(
                    out=pse[0:N_ELEM, :],
                    lhsT=xeT_sb,
                    rhs=weT_sb,
                    start=True,
                    stop=True,
                )
                eve = p1s.tile([P, HID], BF16, tag="eve")
                nc.vector.tensor_add(
                    out=eve[0:N_ELEM, :],
                    in0=pse[0:N_ELEM, :],
                    in1=be_sb[0:N_ELEM, :],
                )
                # Rem[s, h, 0:16] = h_elem head cols; [s, h, 16] = 1
                nc.vector.tensor_copy(
                    out=Rem_sb[:, :, 0:16],
                    in_=eve[0:N_ELEM, :].rearrange("s (h k) -> s h k", k=D),
                )
                nc.gpsimd.memset(Rem_sb[:, :, 16:17], 1.0)
                # S_em = h_elem @ blockdiag(a_src_em): transpose h_elem first
                ptr_e = p1p.tile([P, N_ELEM], BF16, tag="ptre")
                nc.tensor.transpose(
                    out=ptr_e[:, 0:N_ELEM], in_=eve[0:N_ELEM, 0:P],
                    identity=identb[0:N_ELEM, 0:N_ELEM],
                )
                heT = p1s.tile([P, N_ELEM], BF16, tag="heT")
                nc.vector.tensor_copy(out=heT[:], in_=ptr_e[:, 0:N_ELEM])
                ps_s = p1p.tile([P, H], F32, tag="pss")
                nc.tensor.matmul(
                    out=ps_s[0:N_ELEM, :], lhsT=heT[:], rhs=aembd_sb,
                    start=True, stop=True,
                )
                nc.vector.tensor_copy(
                    out=Sem_sb[:], in_=ps_s[0:N_ELEM, :]
                )

            # ================= P2: edge aggregation =================
            do_p2 = variant != "nop2"
            g_mm = variant in ("full", "nomem") or "gonly" in vflags
            do_em = variant in ("full", "nomm_g")
            if do_p2:
              with (
                tc.tile_pool(name="p2s", bufs=2) as p2s,
                tc.tile_pool(name="p2g", bufs=2) as p2g,
                tc.tile_pool(name="p2p", bufs=2, space="PSUM") as p2p,
                tc.tile_pool(name="p2a", bufs=2, space="PSUM") as p2a,
                tc.tile_pool(name="p2t", bufs=1, space="PSUM") as p2t,
              ):
                # per-tile valid-chunk geometry from the static counts
                vc_ = [[-(-c16[t][w] // P) for w in range(NWIN)]
                       for t in range(NT)]
                # zero both G rotation buffers once: slots skipped by the
                # gather read stale SBUF; stale must be finite bf16
                for _ in range(2):
                    Gz = p2g.tile([P, CHM, HID], BF16, tag="G")
                    nc.gpsimd.memset(Gz[:], 0.0)
                for t in range(4 if "nt4" in vflags else NT):
                    offs = [0]
                    for w in range(NWIN):
                        offs.append(offs[-1] + vc_[t][w])
                    VT = offs[-1]
                    i16 = p2s.tile([P, 9 * CHM], I16, tag="i16")
                    szE = P * 9 * CHM
                    nc.sync.dma_start(
                        out=i16[:],
                        in_=pk_seg(E0 + t * szE, szE).rearrange(
                            "(p w) -> p w", w=9 * CHM))
                    dlb = i16[:, 8 * CHM:9 * CHM].bitcast(BF16)
                    dlR = p2s.tile([P, CHM * P], BF16, tag="dlR")
                    nc.sync.dma_start(
                        out=dlR[:, 0:VT * P],
                        in_=pk_seg(D0 + t * CHM * P, VT * P).rearrange(
                            "(o n) -> o n", o=1).bitcast(BF16).to_broadcast(
                            [P, VT * P]),
                    )
                    G = p2g.tile([P, CHM, HID], BF16, tag="G")
                    if g_mm:
                        for r in range(NWIN):
                            b, n = offs[r], vc_[t][r]
                            nc.gpsimd.dma_gather(
                                G[:, b:b + n, :], Tm[r],
                                i16[:, 8 * b:8 * (b + n)],
                                n * P, c16[t][r], HID, single_packet=True,
                            )
                    else:
                        nc.gpsimd.memset(G[:], 0.0)
                    # alpha = <h_src, a_src_mm> + aD_mm[dst]
                    AS = p2g.tile([P, CHM, HID], BF16, tag="AS")
                    nc.vector.tensor_mul(
                        out=AS[:, 0:VT, :], in0=G[:, 0:VT, :],
                        in1=asmm_sb[:, None, :].to_broadcast([P, VT, HID]),
                    )
                    AL = p2s.tile([P, CHM, H], BF16, tag="AL")
                    with nc.allow_low_precision(
                            reason="16-elem head dot, 2e-2 tolerance"):
                        nc.vector.tensor_reduce(
                            out=AL[:, 0:VT, :, None],
                            in_=AS[:, 0:VT, :].rearrange(
                                "p c (h d) -> p c h d", d=D),
                            axis=mybir.AxisListType.X, op=mybir.AluOpType.add,
                        )
                    # OHT[j, c, p] = (dl[p, c] == j); AD lookup via matmul
                    OHT = p2g.tile([P, CHM, P], BF16, tag="OHT")
                    nc.vector.tensor_tensor(
                        out=OHT[:, 0:VT, :],
                        in0=iotc_sb[:, :, None].to_broadcast([P, VT, P]),
                        in1=dlR[:, 0:VT * P].rearrange("j (c p) -> j c p", p=P),
                        op=mybir.AluOpType.is_equal,
                    )
                    adp = p2a.tile([P, CHM, H], F32, tag="adp")
                    for c in range(VT):
                        nc.tensor.matmul(
                            out=adp[:, c, :], lhsT=OHT[:, c, :],
                            rhs=ADm_sb[:, t, :], start=True, stop=True,
                        )
                    nc.vector.tensor_add(
                        out=AL[:, 0:VT, :], in0=AL[:, 0:VT, :],
                        in1=adp[:, 0:VT, :])
                    nc.vector.scalar_tensor_tensor(
                        out=AL[:, 0:VT, :], in0=AL[:, 0:VT, :], scalar=NEG,
                        in1=AL[:, 0:VT, :],
                        op0=mybir.AluOpType.mult, op1=mybir.AluOpType.max,
                    )
                    Gw = p2g.tile([P, CHM, 136], BF16, tag="Gw")
                    nc.scalar.activation(
                        out=Gw[:, 0:VT, 128:136], in_=AL[:, 0:VT, :],
                        func=mybir.ActivationFunctionType.Exp,
                    )
                    nc.vector.tensor_mul(
                        out=Gw[:, 0:VT, 0:128].rearrange(
                            "p c (e s) -> p c e s", s=D),
                        in0=G[:, 0:VT, :].rearrange("p c (e s) -> p c e s", s=D),
                        in1=Gw[:, 0:VT, 128:136, None].to_broadcast(
                            [P, VT, H, D]),
                    )
                    OH = p2g.tile([P, CHM, P], BF16, tag="OH")
                    nc.vector.tensor_tensor(
                        out=OH[:, 0:VT, :],
                        in0=iot_sb[:, None, :].to_broadcast([P, VT, P]),
                        in1=dlb[:, 0:VT, None].to_broadcast([P, VT, P]),
                        op=mybir.AluOpType.is_equal,
                    )
                    pmm = p2p.tile([P, 136], F32, tag="mm")
                    for c in range(VT):
                        nc.tensor.matmul(
                            out=pmm[:],
                            lhsT=OH[:, c, :],
                            rhs=Gw[:, c, :],
                            start=(c == 0),
                            stop=(c == VT - 1),
                        )
                    # ---- em dense ----
                    pem = p2p.tile([P, H, 32], F32, tag="em")
                    if do_em:
                        ct = p2s.tile([N_ELEM, P], BF16, tag="ct")
                        szM = N_ELEM * P
                        nc.sync.dma_start(
                            out=ct[:],
                            in_=pk_seg(M0 + t * szM, szM).rearrange(
                                "(s d) -> s d", d=P).bitcast(BF16))
                        adE = p2s.tile([N_ELEM, H, P], F32, tag="adE")
                        nc.sync.dma_start(
                            out=adE[:],
                            in_=ADemR[t:t + 1, :].to_broadcast(
                                [N_ELEM, H * P]).rearrange(
                                "s (h d) -> s h d", h=H),
                        )
                        alE = p2s.tile([N_ELEM, H, P], F32, tag="alE")
                        nc.vector.tensor_add(
                            out=alE[:], in0=adE[:],
                            in1=Sem_sb[:, :, None].to_broadcast([N_ELEM, H, P]),
                        )
                        nc.vector.scalar_tensor_tensor(
                            out=alE[:], in0=alE[:], scalar=NEG, in1=alE[:],
                            op0=mybir.AluOpType.mult, op1=mybir.AluOpType.max,
                        )
                        WE = p2s.tile([N_ELEM, H, P], BF16, tag="WE")
                        nc.scalar.activation(
                            out=WE[:], in_=alE[:],
                            func=mybir.ActivationFunctionType.Exp,
                        )
                        nc.vector.tensor_mul(
                            out=WE[:], in0=WE[:],
                            in1=ct[:, None, :].to_broadcast([N_ELEM, H, P]),
                        )
                        for h in range(H):
                            nc.tensor.matmul(
                                out=pem[:, h, 0:17], lhsT=WE[:, h, :],
                                rhs=Rem_sb[:, h, :], start=True, stop=True,
                            )
                    # ---- per-metapath epilogue ----
                    for mp, oTd in ((0, oem_sb), (1, omm_sb)):
                        den = p2s.tile([P, 8], F32, tag=f"den{mp}")
                        if mp == 0:
                            if not do_em:
                                continue
                            nc.vector.tensor_scalar_add(
                                out=den[:], in0=pem[:, :, 16], scalar1=1e-16
                            )
                        else:
                            nc.vector.tensor_scalar_add(
                                out=den[:], in0=pmm[:, 128:136], scalar1=1e-16
                            )
                        nc.vector.reciprocal(out=den[:], in_=den[:])
                        o_sb = p2s.tile([P, 128], F32, tag=f"o{mp}")
                        num_ap = (pem[:, :, 0:16] if mp == 0
                                  else pmm[:, 0:128].rearrange(
                                      "p (e s) -> p e s", s=D))
                        nc.vector.tensor_mul(
                            out=o_sb[:].rearrange("p (e s) -> p e s", s=D),
                            in0=num_ap,
                            in1=den[:, :, None].to_broadcast([P, H, D]),
                        )
                        ptr = p2t.tile([P, P], F32, tag="tr")
                        nc.tensor.transpose(
                            out=ptr[:], in_=o_sb[:], identity=ident[:]
                        )
                        nc.scalar.activation(
                            out=oTd[:, t * P:(t + 1) * P],
                            in_=ptr[:],
                            func=mybir.ActivationFunctionType.Relu,
                        )
                        nw = ND - (NT - 1) * P if t == NT - 1 else P
                        pk = p2t.tile([P, P], F32, tag="k")
                        nc.tensor.matmul(
                            out=pk[:, 0:nw], lhsT=wkT_sb,
                            rhs=oTd[:, t * P:t * P + nw],
                            start=True, stop=True,
                        )
                        tanh_sb = p2s.tile([P, P], F32, tag="tanh")
                        s_col = p2s.tile([P, 1], F32, tag="scol")
                        nc.scalar.activation(
                            out=tanh_sb[:, 0:nw],
                            in_=pk[:, 0:nw],
                            func=mybir.ActivationFunctionType.Tanh,
                            bias=bkc_sb[:, 0:1],
                            accum_out=s_col[:],
                        )
                        nc.vector.tensor_add(
                            out=S_sb[:, mp:mp + 1],
                            in0=S_sb[:, mp:mp + 1],
                            in1=s_col[:],
                        )

            # ================= P3: semantic attention + final =================
            if variant in ("full", "nop2"):
              with (
                tc.tile_pool(name="p3s", bufs=3) as p3s,
                tc.tile_pool(name="p3p", bufs=2, space="PSUM") as p3p,
              ):
                nc.sync.dma_start(out=Sin_d[:], in_=S_sb[:])
                nc.gpsimd.collective_compute(
                    "AllReduce",
                    mybir.AluOpType.add,
                    replica_groups=[list(range(NCORES))],
                    ins=[Sin_d.opt()],
                    outs=[Sout_d.opt()],
                )
                Sr_sb = p3s.tile([HID, 2], F32, tag="Sr")
                nc.sync.dma_start(out=Sr_sb[:], in_=Sout_d[:])
                ps_s = p3p.tile([P, 2], F32, tag="s")
                nc.tensor.matmul(
                    out=ps_s[0:1, :], lhsT=qc_sb[:, 0:1], rhs=Sr_sb[:],
                    start=True, stop=True,
                )
                es = p3s.tile([P, 2], F32, tag="es")
                nc.scalar.activation(
                    out=es[0:1, :], in_=ps_s[0:1, :],
                    func=mybir.ActivationFunctionType.Exp,
                )
                ds = p3s.tile([P, 1], F32, tag="ds")
                nc.vector.tensor_reduce(
                    out=ds[0:1, :], in_=es[0:1, :],
                    axis=mybir.AxisListType.X, op=mybir.AluOpType.add,
                )
                nc.vector.reciprocal(out=ds[0:1, :], in_=ds[0:1, :])
                at = p3s.tile([P, 2], F32, tag="at")
                nc.vector.tensor_scalar_mul(
                    out=at[0:1, :], in0=es[0:1, :], scalar1=ds[0:1, 0:1]
                )
                pb = p3p.tile([P, 2], F32, tag="b")
                nc.tensor.matmul(
                    out=pb[:], lhsT=ones_sb[:], rhs=at[0:1, :],
                    start=True, stop=True,
                )
                ab = p3s.tile([P, 2], F32, tag="ab")
                nc.vector.tensor_copy(out=ab[:], in_=pb[:])
                # process tiles in groups of 4: one combine over [128, 512],
                # 4 matmuls, one batched y store
                TG = 4
                for g in range(0, NT, TG):
                    ng = min(TG, NT - g)
                    gw = ng * P
                    comb = p3s.tile([P, TG * P], BF16, tag="comb")
                    nc.vector.tensor_scalar_mul(
                        out=comb[:, 0:gw],
                        in0=oem_sb[:, g * P:g * P + gw], scalar1=ab[:, 0:1]
                    )
                    nc.vector.scalar_tensor_tensor(
                        out=comb[:, 0:gw],
                        in0=omm_sb[:, g * P:g * P + gw],
                        scalar=ab[:, 1:2],
                        in1=comb[:, 0:gw],
                        op0=mybir.AluOpType.mult,
                        op1=mybir.AluOpType.add,
                    )
                    y_sb = p3s.tile([P, TG, OUT], F32, tag="ysb")
                    for j in range(ng):
                        py_ = p3p.tile([P, OUT], F32, tag="y")
                        nc.tensor.matmul(
                            out=py_[:], lhsT=comb[:, j * P:(j + 1) * P],
                            rhs=wlT_sb, start=True, stop=True,
                        )
                        nc.vector.tensor_add(
                            out=y_sb[:, j, :], in0=py_[:], in1=blb_sb)
                    nc.sync.dma_start(
                        out=y[g * P:g * P + gw, :].rearrange(
                            "(a p) o -> p a o", p=P),
                        in_=y_sb[:, 0:ng, :])

    nc.compile()
    return nc


_CACHE = {}


def prep_all(inputs):
    host = build_host_tensors(inputs)
    e16, dlr, CmT, c16, chr_ = bucket_edges_v3(inputs)
    xTb = host.pop("xT")
    CHM = NWIN * chr_
    _, _, _, _, PK_TOT = pk_offsets(CHM)
    in_maps = []
    for c in range(NCORES):
        xl = np.ascontiguousarray(xTb[:, c * ND:(c + 1) * ND])
        pk = np.concatenate([
            e16[c].ravel(),
            CmT[c].view(np.int16).ravel(),
            dlr[c].view(np.int16).ravel(),
            xl.view(np.int16).ravel(),
        ])[None, :]
        assert pk.shape[1] == PK_TOT, (pk.shape, PK_TOT)
        in_maps.append(dict(host, pk=np.ascontiguousarray(pk)))
    return in_maps, c16, chr_


def kernel(**inputs):
    in_maps, c16, chr_ = prep_all(inputs)
    key = (c16, chr_)
    if key not in _CACHE:
        _CACHE[key] = build_program(c16, chr_)
    nc = _CACHE[key]
    res = run_bass_kernel_spmd(nc, in_maps, core_ids=list(range(NCORES)))
    out = np.empty((N_MAT, OUT), np.float32)
    for c in range(NCORES):
        out[c * ND:(c + 1) * ND] = res.results[c]["y"][:ND]
    return out


# revision 31
# speedup vs baseline: 3.8051x; 1.0689x over previous
"""HAN kernel v3 — dense-em + matmul-AD, mm-only gathers, dst-partitioned.

Strategy (dst-partitioned across 8 cores, core c owns mat rows
[c*12500, (c+1)*12500)):
  - P1 projects all mat nodes into 4 bf16 window tables Tm0..3 (dma_gather
    indices are int16); elem nodes (118) stay SBUF-resident. Per-dst
    attention dots: aD_mm lands directly in an SBUF table ADm_sb[128,98,8];
    aD_em is PE-transposed per 128-dst block and written to DRAM rows
    ADemR[98, 8*128] (f32) for later broadcast-loads. S_em[118,8] =
    <h_elem, a_src_em> via transpose+matmul; Rem[118,8,17] packs h_elem
    head-columns plus a ones column (for the softmax denominator).
  - P2 mm (edge-parallel): edges bucketed host-side by (core, dst_tile,
    src_window); per (tile,window) the gather count is the max over cores
    (static, SPMD-shared), shorter cores padded with idx=0 / dl=300
    sentinel edges; slots beyond the count are skipped by the DMA
    (descriptor savings). Per tile: 4 dma_gathers fetch h_src rows;
    alpha = <h,a_src_mm> (DVE mult+reduce) + aD_mm[dst] where the dst
    lookup is a one-hot-transpose matmul: OHT[j,c,p] = (dl[p,c]==j) built
    from a broadcast-DMA'd dl row, then OHT_c^T @ ADm_sb[:,t,:] in PSUM.
    leakyrelu+exp; weighted rows + exp cols scatter-add into PSUM via the
    one-hot matmul per chunk (numerator | denominator at once).
  - P2 em (dense): only 118 sources, so per tile the full [118 src x 128
    dst] attention matrix is computed densely: alpha = S_em + aD_em
    (broadcast-DMA row), lrelu, exp, times the host-built edge-count
    matrix CmT[t]; num|den via 8 per-head matmuls against Rem.
  - o = relu(num/den), PE-transpose, bf16 oT tables; tanh/semantic partial
    sums accumulate; P3: 1KB AllReduce, softmax over 2 metapaths, weighted
    combine, final linear to fp32.
"""

import numpy as np
import ml_dtypes

import concourse.bacc as bacc
import concourse.bass as bass
import concourse.mybir as mybir
import concourse.tile as tile
from concourse.bass_utils import run_bass_kernel_spmd
from concourse.masks import make_identity

P = 128
N_MAT = 100000
N_ELEM = 118
F_MAT = 128
F_ELEM = 64
HID = 128
H = 8
D = 16
OUT = 64
NCORES = 8
ND = N_MAT // NCORES          # 12500 dst rows per core
NT = (ND + P - 1) // P        # 98 dst tiles per core
NDP = NT * P                  # 12544 padded dst rows
NEG = 0.2
WIN = 32768                   # int16 index window
NWIN = 4
SLABS_PER_WIN = WIN // 1024   # 32
NTM_FULL = N_MAT // 1024      # 97 full 1024-row projection slabs
LAST_SLAB = N_MAT - NTM_FULL * 1024  # 672
NDL_FULL = ND // 1024         # 12 full local slabs
LAST_DL = ND - NDL_FULL * 1024       # 212
F32 = mybir.dt.float32
BF16 = mybir.dt.bfloat16
I16 = mybir.dt.int16

BF = ml_dtypes.bfloat16

# consolidated-constant column layouts (shared by host packing and program)
def _mkseg(widths):
    seg, c = {}, 0
    for n, w in widths:
        seg[n] = (c, c + w)
        c += w
    return seg, c


CB_SEG, CB_COLS = _mkseg([
    ("whT", HID), ("weT", HID), ("wad", 16), ("asmm", HID), ("aembd", H),
    ("wkT", HID), ("wlT", OUT), ("iot", P), ("iotc", 1), ("xeT", N_ELEM),
])
CF_SEG, CF_COLS = _mkseg([
    ("bh", HID), ("be", HID), ("bad", 16), ("bkc", 1), ("qc", 1),
    ("blb", OUT),
])


def pk_offsets(CHM):
    E0 = 0
    M0 = E0 + NT * P * 9 * CHM
    D0 = M0 + NT * N_ELEM * P
    X0 = D0 + NT * CHM * P
    TOT = X0 + F_MAT * ND
    return E0, M0, D0, X0, TOT


def _bf(a):
    return np.asarray(a, dtype=BF)


def _blockdiag(a):
    """a [H, D] -> [HID, H] block diagonal so h @ A = per-head <h, a>."""
    A = np.zeros((HID, H), np.float32)
    for h in range(H):
        A[h * D:(h + 1) * D, h] = a[h]
    return A


def build_host_tensors(inputs):
    x_mat = inputs["x_mat"]
    WpT = np.ascontiguousarray(inputs["W_proj_mat"].T)       # [128f, 128k]
    WpeT = np.ascontiguousarray(inputs["W_proj_elem"].T)     # [64f, 128k]
    A_dem = _blockdiag(inputs["a_dst_em"])
    A_dmm = _blockdiag(inputs["a_dst_mm"])
    wad = np.concatenate([WpT @ A_dem, WpT @ A_dmm], axis=1)  # [128, 16]
    bad_row = np.concatenate(
        [inputs["b_proj_mat"] @ A_dem, inputs["b_proj_mat"] @ A_dmm])
    asmm = np.tile(inputs["a_src_mm"].reshape(1, HID), (P, 1))

    bfp = dict(
        whT=_bf(WpT),                                        # [128, 128]
        weT=_bf(WpeT),                                       # [64, 128]
        wad=_bf(wad),                                        # [128, 16]
        asmm=_bf(asmm),
        aembd=_bf(_blockdiag(inputs["a_src_em"])),           # [128, 8]
        wkT=_bf(np.ascontiguousarray(inputs["Wk"].T)),
        wlT=_bf(np.ascontiguousarray(inputs["Wl"].T)),
        iot=_bf(np.tile(np.arange(P, dtype=np.float32), (P, 1))),
        iotc=_bf(np.arange(P, dtype=np.float32)[:, None]),
        xeT=_bf(inputs["x_elem"].T),                         # [64, 118]
    )
    f32p = dict(
        bh=np.tile(inputs["b_proj_mat"].astype(np.float32), (P, 1)),
        be=np.tile(inputs["b_proj_elem"].astype(np.float32), (P, 1)),
        bad=np.tile(bad_row.astype(np.float32), (P, 1)),     # [128, 16]
        bkc=inputs["bk"].astype(np.float32)[:, None],
        qc=(inputs["q"] / float(N_MAT)).astype(np.float32)[:, None],
        blb=np.tile(inputs["bl"].astype(np.float32), (P, 1)),
    )
    cb_arr = np.zeros((P, CB_COLS), BF)
    for n, a in bfp.items():
        c0, c1 = CB_SEG[n]
        cb_arr[0:a.shape[0], c0:c1] = a
    cf_arr = np.zeros((P, CF_COLS), np.float32)
    for n, a in f32p.items():
        c0, c1 = CF_SEG[n]
        cf_arr[0:a.shape[0], c0:c1] = a
    return dict(cb=cb_arr, cf=cf_arr, xT=_bf(x_mat.T))


def bucket_edges_v3(inputs):
    """mm edges only: bucket by (core, dst_tile, src_window). Static per
    (tile, window) gather count = max over cores (>=16); shorter cores are
    padded with idx=0 / dl=300. Returns the int16 pack [NCORES, NT, P,
    9*CHM], the dl row tensor [NCORES, NT, CHM*128], counts c16 [NT, 4],
    and CHR. Also builds the em count matrices CmT [NCORES, NT, 118, 128]."""
    src = inputs["src_mm"].astype(np.int64)
    dst = inputs["dst_mm"].astype(np.int64)
    core = dst // ND
    rem = dst - core * ND
    tl = rem // P
    dl = rem % P
    # gather-table row in the AllGather'd padded layout (NDP rows per core)
    src = (src // ND) * NDP + (src % ND)
    w = src // WIN
    key = ((core * NT + tl) * NWIN + w)
    order = np.argsort(key, kind="stable")
    counts = np.bincount(key, minlength=NCORES * NT * NWIN)
    c4 = counts.reshape(NCORES, NT, NWIN)
    c16 = np.maximum(c4.max(axis=0), 16)                     # [NT, NWIN]
    CHR = int(-(-c16.max() // P))
    CHM = NWIN * CHR

    starts = np.zeros(NCORES * NT * NWIN, np.int64)
    starts[1:] = np.cumsum(counts)[:-1]
    ks = key[order]
    rank = np.arange(len(order)) - starts[ks]

    # valid-chunk repacking: window w of tile t occupies chunk slots
    # [voff[t][w], voff[t][w]+vc[t][w]) so downstream per-tile work only
    # covers vtot[t] = sum_w vc chunks (instead of the global max CHM).
    vc = -(-c16 // P)                                # [NT, NWIN]
    voff = np.zeros((NT, NWIN), np.int64)
    voff[:, 1:] = np.cumsum(vc, axis=1)[:, :-1]
    vtot = vc.sum(axis=1)                            # [NT]

    ct_ = ks // NWIN                       # core*NT + tl
    ws = ks % NWIN
    tls = ct_ % NT
    slot = (ct_ * CHM + voff[tls, ws]) * P + rank

    idxv = np.full(NCORES * NT * CHM * P, -1, np.int16)
    idxv[slot] = (src[order] - ws * WIN).astype(np.int16)
    dlv = np.full(NCORES * NT * CHM * P, 300.0, np.float32)
    dlv[slot] = dl[order]

    # pad [count, c16) with idx=0 (valid row, excluded via dl sentinel)
    idxv = idxv.reshape(NCORES, NT, CHM * P)
    for t in range(NT):
        for wi in range(NWIN):
            base = int(voff[t, wi]) * P
            tgt = int(c16[t, wi])
            for c in range(NCORES):
                n = int(c4[c, t, wi])
                if n < tgt:
                    idxv[c, t, base + n:base + tgt] = 0
    dlv = dlv.reshape(NCORES, NT, CHM * P)

    def wrap(a):
        # [..., n] -> [..., 128, n//16] (16-part wrap, replicated x8)
        sh = a.shape[:-1]
        n = a.shape[-1]
        w_ = a.reshape(*sh, n // 16, 16)
        w_ = np.swapaxes(w_, -1, -2)                 # [..., 16, n//16]
        w_ = np.broadcast_to(w_[..., None, :, :], (*sh, 8, 16, n // 16))
        return w_.reshape(*sh, P, n // 16)

    # per-window wrapped idx blocks at col offset 8*voff (width 8*vc)
    iw = np.full((NCORES, NT, P, 8 * CHM), -1, np.int16)
    for t in range(NT):
        for wi in range(NWIN):
            b, n = int(voff[t, wi]), int(vc[t, wi])
            iw[:, t, :, 8 * b:8 * (b + n)] = wrap(
                idxv[:, t, b * P:(b + n) * P])
    # dlb: [p, CHM] with col = global chunk slot
    dlb = _bf(dlv).view(np.int16).reshape(NCORES, NT, CHM, P)
    dlb = np.swapaxes(dlb, 2, 3)                          # [.., 128, CHM]
    e16 = np.concatenate([iw, dlb], axis=3)
    assert e16.shape == (NCORES, NT, P, 9 * CHM)

    dlr = _bf(dlv).reshape(NCORES, NT, CHM * P)

    # em dense count matrices: CmT[c, t, s, p] = #em edges (s -> dst c*ND+t*128+p)
    src_e = inputs["src_em"].astype(np.int64)
    dst_e = inputs["dst_em"].astype(np.int64)
    core_e = dst_e // ND
    rem_e = dst_e - core_e * ND
    flat = (core_e * NT + rem_e // P) * (N_ELEM * P) + src_e * P + rem_e % P
    cm = np.bincount(flat, minlength=NCORES * NT * N_ELEM * P)
    CmT = _bf(cm.reshape(NCORES, NT, N_ELEM, P))

    c16_t = tuple(tuple(int(x) for x in row) for row in c16)
    return (np.ascontiguousarray(e16), np.ascontiguousarray(dlr),
            np.ascontiguousarray(CmT), c16_t, CHR)


def build_program(c16, chr_, dbg=False, variant="full"):
    vparts = variant.split("-")
    vflags = set(vparts[1:])
    variant = vparts[0]
    CHM = NWIN * chr_
    nc = bacc.Bacc(
        "TRN2",
        target_bir_lowering=False,
        debug=False,
        enable_asserts=False,
        num_devices=NCORES,
    )

    # three consolidated inputs (per-execution buffer binding costs ~55us
    # per input through the axon PJRT path, so pack everything):
    #   cb: shared bf16 constants [128, CB_COLS], column segments
    #   cf: shared f32 constants [128, CF_COLS]
    #   pk: per-core flat i16 blob (e16 | CmT | dlr | xlT bit patterns)
    cb = nc.dram_tensor("cb", [P, CB_COLS], BF16, kind="ExternalInput").ap()
    cf = nc.dram_tensor("cf", [P, CF_COLS], F32, kind="ExternalInput").ap()
    E0, M0, D0, X0, PK_TOT = pk_offsets(CHM)
    pkin = nc.dram_tensor("pk", [1, PK_TOT], I16, kind="ExternalInput").ap()
    y = nc.dram_tensor("y", [NDP, OUT], F32, kind="ExternalOutput").ap()

    def pk_seg(a, n):
        return pkin[0, a:a + n]

    xlT = pk_seg(X0, F_MAT * ND).rearrange("(f n) -> f n", n=ND).bitcast(BF16)

    with tile.TileContext(nc) as tc:
        with (
            tc.tile_pool(name="const", bufs=1) as cp,
            tc.tile_pool(name="dram", bufs=1, space="DRAM") as dp,
        ):
            # ---- persistent DRAM tables ----
            AGin = dp.tile([NDP, HID], BF16)
            TmAll = dp.tile([NCORES * NDP, HID], BF16, addr_space="Shared")
            # gather windows are int16-addressable row-slices of TmAll
            Tm = [TmAll[WIN * r: min(WIN * (r + 1), NCORES * NDP), :]
                  for r in range(NWIN)]
            ADemR = dp.tile([NT, H * P], F32)
            Sin_d = dp.tile([HID, 2], F32)
            Sout_d = dp.tile([HID, 2], F32)

            # ---- constants in SBUF: two bulk loads, then slices ----
            cb_sb = cp.tile([P, CB_COLS], BF16, tag="cb")
            nc.sync.dma_start(out=cb_sb[:], in_=cb[:])
            cf_sb = cp.tile([P, CF_COLS], F32, tag="cf")
            nc.sync.dma_start(out=cf_sb[:], in_=cf[:])

            def cbs(name, rows=P):
                a, b = CB_SEG[name]
                return cb_sb[0:rows, a:b]

            def cfs(name, rows=P):
                a, b = CF_SEG[name]
                return cf_sb[0:rows, a:b]

            whT_sb = cbs("whT")
            weT_sb = cbs("weT", F_ELEM)
            wad_sb = cbs("wad")
            asmm_sb = cbs("asmm")
            aembd_sb = cbs("aembd")
            wkT_sb = cbs("wkT")
            wlT_sb = cbs("wlT")
            iot_sb = cbs("iot")
            iotc_sb = cbs("iotc")
            xeT_sb = cbs("xeT", F_ELEM)
            bh_sb = cfs("bh")
            be_sb = cfs("be")
            bad_sb = cfs("bad")
            bkc_sb = cfs("bkc")
            qc_sb = cfs("qc")
            blb_sb = cfs("blb")
            ones_sb = cp.tile([1, P], F32, tag="ones")
            nc.vector.memset(ones_sb[:], 1.0)
            ident = cp.tile([P, P], F32, tag="ident")
            make_identity(nc, ident[:])
            identb = cp.tile([P, P], BF16, tag="identb")
            nc.vector.tensor_copy(out=identb[:], in_=ident[:])
            S_sb = cp.tile([HID, 2], F32, tag="S")
            nc.gpsimd.memset(S_sb[:], 0.0)
            # persistent SBUF tables
            ADm_sb = cp.tile([P, NT, H], BF16, tag="ADm")
            nc.gpsimd.memset(ADm_sb[:], 0.0)
            Sem_sb = cp.tile([N_ELEM, H], F32, tag="Sem")
            Rem_sb = cp.tile([N_ELEM, H, 17], BF16, tag="Rem")
            # per-metapath outputs, transposed [HID, dst], SBUF-resident
            oem_sb = cp.tile([HID, NDP], BF16, tag="oem")
            omm_sb = cp.tile([HID, NDP], BF16, tag="omm")
            nc.vector.memset(oem_sb[:], 0.0)
            nc.vector.memset(omm_sb[:], 0.0)

            # ================= P1: projections =================
            # Distributed: each core projects only its local ND rows into
            # AGin, then an AllGather assembles the full node table TmAll
            # [NCORES*NDP, HID] every core gathers from. The per-dst
            # attention dots share the same xlT slab pass (one load, two
            # matmuls into one psum tile).
            do_p1 = "nop1" not in vflags
            with (
                tc.tile_pool(name="p1s", bufs=3) as p1s,
                tc.tile_pool(name="p1p", bufs=2, space="PSUM") as p1p,
            ):
                for s in range(NDL_FULL + 1 if do_p1 else 0):
                    w = 1024 if s < NDL_FULL else LAST_DL
                    xsl = p1s.tile([P, 1024], BF16, tag="xsl")
                    nc.sync.dma_start(
                        out=xsl[:, 0:w], in_=xlT[:, s * 1024: s * 1024 + w]
                    )
                    ev = p1s.tile([P, 8, HID], BF16, tag="ev")
                    evT = p1s.tile([H, 8, P], F32, tag="evT")
                    ntile = (w + P - 1) // P
                    for j in range(ntile):
                        m = min(P, w - j * P)
                        t_abs = s * 8 + j
                        ps = p1p.tile([P, HID + 16], F32, tag="ps")
                        nc.tensor.matmul(
                            out=ps[0:m, 0:HID],
                            lhsT=xsl[:, j * P: j * P + m],
                            rhs=whT_sb,
                            start=True,
                            stop=True,
                        )
                        nc.tensor.matmul(
                            out=ps[0:m, HID:HID + 16],
                            lhsT=xsl[:, j * P: j * P + m],
                            rhs=wad_sb,
                            start=True,
                            stop=True,
                        )
                        nc.vector.tensor_add(
                            out=ev[0:m, j, :], in0=ps[0:m, 0:HID],
                            in1=bh_sb[0:m, :]
                        )
                        ev2 = p1s.tile([P, 16], F32, tag="ev2")
                        nc.vector.tensor_add(
                            out=ev2[0:m, :], in0=ps[0:m, HID:HID + 16],
                            in1=bad_sb[0:m, :]
                        )
                        # mm half -> SBUF table (bf16)
                        nc.vector.tensor_copy(
                            out=ADm_sb[0:m, t_abs, :], in_=ev2[0:m, 8:16]
                        )
                        # em half -> transpose -> ADemR rows
                        ptr = p1p.tile([P, P], F32, tag="ptr")
                        nc.tensor.transpose(
                            out=ptr[0:16, 0:m], in_=ev2[0:m, 0:16],
                            identity=ident[0:m, 0:m],
                        )
                        nc.vector.tensor_copy(
                            out=evT[:, j, 0:m], in_=ptr[0:H, 0:m]
                        )
                        if m < P:
                            nc.gpsimd.memset(evT[:, j, m:P], 0.0)
                    if s < NDL_FULL:
                        nc.sync.dma_start(
                            out=AGin[s * 1024:(s + 1) * 1024, :].rearrange(
                                "(a p) e -> p a e", p=P
                            ),
                            in_=ev[:, :, :],
                        )
                    else:
                        for j in range(ntile):
                            m = min(P, w - j * P)
                            r0 = s * 1024 + j * P
                            nc.sync.dma_start(
                                out=AGin[r0: r0 + m, :], in_=ev[0:m, j, :]
                            )
                    nc.sync.dma_start(
                        out=ADemR[s * 8: s * 8 + ntile, :].rearrange(
                            "t (h d) -> h t d", h=H),
                        in_=evT[:, 0:ntile, :],
                    )
                if do_p1:
                    nc.gpsimd.collective_compute(
                        "AllGather",
                        mybir.AluOpType.bypass,
                        replica_groups=[list(range(NCORES))],
                        ins=[AGin.opt()],
                        outs=[TmAll.opt()],
                    )
                # ---- elem projection + S_em + Rem ----
                pse = p1p.tile([P, HID], F32, tag="ps")
                nc.tensor.matmul(
                    out=pse[0:N_ELEM, :],
                    lhsT=xeT_sb,
                    rhs=weT_sb,
                    start=True,
                    stop=True,
                )
                eve = p1s.tile([P, HID], BF16, tag="eve")
                nc.vector.tensor_add(
                    out=eve[0:N_ELEM, :],
                    in0=pse[0:N_ELEM, :],
                    in1=be_sb[0:N_ELEM, :],
                )
                # Rem[s, h, 0:16] = h_elem head cols; [s, h, 16] = 1
                nc.vector.tensor_copy(
                    out=Rem_sb[:, :, 0:16],
                    in_=eve[0:N_ELEM, :].rearrange("s (h k) -> s h k", k=D),
                )
                nc.gpsimd.memset(Rem_sb[:, :, 16:17], 1.0)
                # S_em = h_elem @ blockdiag(a_src_em): transpose h_elem first
                ptr_e = p1p.tile([P, N_ELEM], BF16, tag="ptre")
                nc.tensor.transpose(
                    out=ptr_e[:, 0:N_ELEM], in_=eve[0:N_ELEM, 0:P],
                    identity=identb[0:N_ELEM, 0:N_ELEM],
                )
                heT = p1s.tile([P, N_ELEM], BF16, tag="heT")
                nc.vector.tensor_copy(out=heT[:], in_=ptr_e[:, 0:N_ELEM])
                ps_s = p1p.tile([P, H], F32, tag="pss")
                nc.tensor.matmul(
                    out=ps_s[0:N_ELEM, :], lhsT=heT[:], rhs=aembd_sb,
                    start=True, stop=True,
                )
                nc.vector.tensor_copy(
                    out=Sem_sb[:], in_=ps_s[0:N_ELEM, :]
                )

            # ================= P2: edge aggregation =================
            do_p2 = variant != "nop2"
            g_mm = variant in ("full", "nomem") or "gonly" in vflags
            do_em = variant in ("full", "nomm_g")
            if do_p2:
              with (
                tc.tile_pool(name="p2s", bufs=3) as p2s,
                tc.tile_pool(name="p2g", bufs=2) as p2g,
                tc.tile_pool(name="p2p", bufs=2, space="PSUM") as p2p,
                tc.tile_pool(name="p2a", bufs=2, space="PSUM") as p2a,
                tc.tile_pool(name="p2t", bufs=1, space="PSUM") as p2t,
              ):
                # per-tile valid-chunk geometry from the static counts
                vc_ = [[-(-c16[t][w] // P) for w in range(NWIN)]
                       for t in range(NT)]
                # zero both G rotation buffers once: slots skipped by the
                # gather read stale SBUF; stale must be finite bf16
                for _ in range(2):
                    Gz = p2g.tile([P, CHM, HID], BF16, tag="G")
                    nc.gpsimd.memset(Gz[:], 0.0)
                for t in range(4 if "nt4" in vflags else NT):
                    offs = [0]
                    for w in range(NWIN):
                        offs.append(offs[-1] + vc_[t][w])
                    VT = offs[-1]
                    i16 = p2s.tile([P, 9 * CHM], I16, tag="i16")
                    szE = P * 9 * CHM
                    nc.sync.dma_start(
                        out=i16[:],
                        in_=pk_seg(E0 + t * szE, szE).rearrange(
                            "(p w) -> p w", w=9 * CHM))
                    dlb = i16[:, 8 * CHM:9 * CHM].bitcast(BF16)
                    dlR = p2s.tile([P, CHM * P], BF16, tag="dlR")
                    nc.sync.dma_start(
                        out=dlR[:, 0:VT * P],
                        in_=pk_seg(D0 + t * CHM * P, VT * P).rearrange(
                            "(o n) -> o n", o=1).bitcast(BF16).to_broadcast(
                            [P, VT * P]),
                    )
                    G = p2g.tile([P, CHM, HID], BF16, tag="G")
                    if g_mm:
                        for r in range(NWIN):
                            b, n = offs[r], vc_[t][r]
                            nc.gpsimd.dma_gather(
                                G[:, b:b + n, :], Tm[r],
                                i16[:, 8 * b:8 * (b + n)],
                                n * P, c16[t][r], HID, single_packet=True,
                            )
                    else:
                        nc.gpsimd.memset(G[:], 0.0)
                    # alpha = <h_src, a_src_mm> + aD_mm[dst]
                    AS = p2g.tile([P, CHM, HID], BF16, tag="AS")
                    nc.vector.tensor_mul(
                        out=AS[:, 0:VT, :], in0=G[:, 0:VT, :],
                        in1=asmm_sb[:, None, :].to_broadcast([P, VT, HID]),
                    )
                    AL = p2s.tile([P, CHM, H], BF16, tag="AL")
                    with nc.allow_low_precision(
                            reason="16-elem head dot, 2e-2 tolerance"):
                        nc.vector.tensor_reduce(
                            out=AL[:, 0:VT, :, None],
                            in_=AS[:, 0:VT, :].rearrange(
                                "p c (h d) -> p c h d", d=D),
                            axis=mybir.AxisListType.X, op=mybir.AluOpType.add,
                        )
                    # OHT[j, c, p] = (dl[p, c] == j); AD lookup via matmul
                    OHT = p2g.tile([P, CHM, P], BF16, tag="OHT")
                    nc.vector.tensor_tensor(
                        out=OHT[:, 0:VT, :],
                        in0=iotc_sb[:, :, None].to_broadcast([P, VT, P]),
                        in1=dlR[:, 0:VT * P].rearrange("j (c p) -> j c p", p=P),
                        op=mybir.AluOpType.is_equal,
                    )
                    adp = p2a.tile([P, CHM, H], F32, tag="adp")
                    for c in range(VT):
                        nc.tensor.matmul(
                            out=adp[:, c, :], lhsT=OHT[:, c, :],
                            rhs=ADm_sb[:, t, :], start=True, stop=True,
                        )
                    nc.vector.tensor_add(
                        out=AL[:, 0:VT, :], in0=AL[:, 0:VT, :],
                        in1=adp[:, 0:VT, :])
                    nc.vector.scalar_tensor_tensor(
                        out=AL[:, 0:VT, :], in0=AL[:, 0:VT, :], scalar=NEG,
                        in1=AL[:, 0:VT, :],
                        op0=mybir.AluOpType.mult, op1=mybir.AluOpType.max,
                    )
                    Gw = p2g.tile([P, CHM, 136], BF16, tag="Gw")
                    nc.scalar.activation(
                        out=Gw[:, 0:VT, 128:136], in_=AL[:, 0:VT, :],
                        func=mybir.ActivationFunctionType.Exp,
                    )
                    nc.vector.tensor_mul(
                        out=Gw[:, 0:VT, 0:128].rearrange(
                            "p c (e s) -> p c e s", s=D),
                        in0=G[:, 0:VT, :].rearrange("p c (e s) -> p c e s", s=D),
                        in1=Gw[:, 0:VT, 128:136, None].to_broadcast(
                            [P, VT, H, D]),
                    )
                    OH = p2g.tile([P, CHM, P], BF16, tag="OH")
                    nc.vector.tensor_tensor(
                        out=OH[:, 0:VT, :],
                        in0=iot_sb[:, None, :].to_broadcast([P, VT, P]),
                        in1=dlb[:, 0:VT, None].to_broadcast([P, VT, P]),
                        op=mybir.AluOpType.is_equal,
                    )
                    pmm = p2p.tile([P, 136], F32, tag="mm")
                    for c in range(VT):
                        nc.tensor.matmul(
                            out=pmm[:],
                            lhsT=OH[:, c, :],
                            rhs=Gw[:, c, :],
                            start=(c == 0),
                            stop=(c == VT - 1),
                        )
                    # ---- em dense ----
                    pem = p2p.tile([P, H, 32], F32, tag="em")
                    if do_em:
                        ct = p2s.tile([N_ELEM, P], BF16, tag="ct")
                        szM = N_ELEM * P
                        nc.sync.dma_start(
                            out=ct[:],
                            in_=pk_seg(M0 + t * szM, szM).rearrange(
                                "(s d) -> s d", d=P).bitcast(BF16))
                        adE = p2s.tile([N_ELEM, H, P], F32, tag="adE")
                        nc.sync.dma_start(
                            out=adE[:],
                            in_=ADemR[t:t + 1, :].to_broadcast(
                                [N_ELEM, H * P]).rearrange(
                                "s (h d) -> s h d", h=H),
                        )
                        alE = p2s.tile([N_ELEM, H, P], F32, tag="alE")
                        nc.vector.tensor_add(
                            out=alE[:], in0=adE[:],
                            in1=Sem_sb[:, :, None].to_broadcast([N_ELEM, H, P]),
                        )
                        nc.vector.scalar_tensor_tensor(
                            out=alE[:], in0=alE[:], scalar=NEG, in1=alE[:],
                            op0=mybir.AluOpType.mult, op1=mybir.AluOpType.max,
                        )
                        WE = p2s.tile([N_ELEM, H, P], BF16, tag="WE")
                        nc.scalar.activation(
                            out=WE[:], in_=alE[:],
                            func=mybir.ActivationFunctionType.Exp,
                        )
                        nc.vector.tensor_mul(
                            out=WE[:], in0=WE[:],
                            in1=ct[:, None, :].to_broadcast([N_ELEM, H, P]),
                        )
                        for h in range(H):
                            nc.tensor.matmul(
                                out=pem[:, h, 0:17], lhsT=WE[:, h, :],
                                rhs=Rem_sb[:, h, :], start=True, stop=True,
                            )
                    # ---- per-metapath epilogue ----
                    for mp, oTd in ((0, oem_sb), (1, omm_sb)):
                        den = p2s.tile([P, 8], F32, tag=f"den{mp}")
                        if mp == 0:
                            if not do_em:
                                continue
                            nc.vector.tensor_scalar_add(
                                out=den[:], in0=pem[:, :, 16], scalar1=1e-16
                            )
                        else:
                            nc.vector.tensor_scalar_add(
                                out=den[:], in0=pmm[:, 128:136], scalar1=1e-16
                            )
                        nc.vector.reciprocal(out=den[:], in_=den[:])
                        o_sb = p2s.tile([P, 128], F32, tag=f"o{mp}")
                        num_ap = (pem[:, :, 0:16] if mp == 0
                                  else pmm[:, 0:128].rearrange(
                                      "p (e s) -> p e s", s=D))
                        nc.vector.tensor_mul(
                            out=o_sb[:].rearrange("p (e s) -> p e s", s=D),
                            in0=num_ap,
                            in1=den[:, :, None].to_broadcast([P, H, D]),
                        )
                        ptr = p2t.tile([P, P], F32, tag="tr")
                        nc.tensor.transpose(
                            out=ptr[:], in_=o_sb[:], identity=ident[:]
                        )
                        nc.scalar.activation(
                            out=oTd[:, t * P:(t + 1) * P],
                            in_=ptr[:],
                            func=mybir.ActivationFunctionType.Relu,
                        )
                        nw = ND - (NT - 1) * P if t == NT - 1 else P
                        pk = p2t.tile([P, P], F32, tag="k")
                        nc.tensor.matmul(
                            out=pk[:, 0:nw], lhsT=wkT_sb,
                            rhs=oTd[:, t * P:t * P + nw],
                            start=True, stop=True,
                        )
                        tanh_sb = p2s.tile([P, P], F32, tag="tanh")
                        s_col = p2s.tile([P, 1], F32, tag="scol")
                        nc.scalar.activation(
                            out=tanh_sb[:, 0:nw],
                            in_=pk[:, 0:nw],
                            func=mybir.ActivationFunctionType.Tanh,
                            bias=bkc_sb[:, 0:1],
                            accum_out=s_col[:],
                        )
                        nc.vector.tensor_add(
                            out=S_sb[:, mp:mp + 1],
                            in0=S_sb[:, mp:mp + 1],
                            in1=s_col[:],
                        )

            # ================= P3: semantic attention + final =================
            if variant in ("full", "nop2"):
              with (
                tc.tile_pool(name="p3s", bufs=3) as p3s,
                tc.tile_pool(name="p3p", bufs=2, space="PSUM") as p3p,
              ):
                nc.sync.dma_start(out=Sin_d[:], in_=S_sb[:])
                nc.gpsimd.collective_compute(
                    "AllReduce",
                    mybir.AluOpType.add,
                    replica_groups=[list(range(NCORES))],
                    ins=[Sin_d.opt()],
                    outs=[Sout_d.opt()],
                )
                Sr_sb = p3s.tile([HID, 2], F32, tag="Sr")
                nc.sync.dma_start(out=Sr_sb[:], in_=Sout_d[:])
                ps_s = p3p.tile([P, 2], F32, tag="s")
                nc.tensor.matmul(
                    out=ps_s[0:1, :], lhsT=qc_sb[:, 0:1], rhs=Sr_sb[:],
                    start=True, stop=True,
                )
                es = p3s.tile([P, 2], F32, tag="es")
                nc.scalar.activation(
                    out=es[0:1, :], in_=ps_s[0:1, :],
                    func=mybir.ActivationFunctionType.Exp,
                )
                ds = p3s.tile([P, 1], F32, tag="ds")
                nc.vector.tensor_reduce(
                    out=ds[0:1, :], in_=es[0:1, :],
                    axis=mybir.AxisListType.X, op=mybir.AluOpType.add,
                )
                nc.vector.reciprocal(out=ds[0:1, :], in_=ds[0:1, :])
                at = p3s.tile([P, 2], F32, tag="at")
                nc.vector.tensor_scalar_mul(
                    out=at[0:1, :], in0=es[0:1, :], scalar1=ds[0:1, 0:1]
                )
                pb = p3p.tile([P, 2], F32, tag="b")
                nc.tensor.matmul(
                    out=pb[:], lhsT=ones_sb[:], rhs=at[0:1, :],
                    start=True, stop=True,
                )
                ab = p3s.tile([P, 2], F32, tag="ab")
                nc.vector.tensor_copy(out=ab[:], in_=pb[:])
                # process tiles in groups of 4: one combine over [128, 512],
                # 4 matmuls, one batched y store
                TG = 4
                for g in range(0, NT, TG):
                    ng = min(TG, NT - g)
                    gw = ng * P
                    comb = p3s.tile([P, TG * P], BF16, tag="comb")
                    nc.vector.tensor_scalar_mul(
                        out=comb[:, 0:gw],
                        in0=oem_sb[:, g * P:g * P + gw], scalar1=ab[:, 0:1]
                    )
                    nc.vector.scalar_tensor_tensor(
                        out=comb[:, 0:gw],
                        in0=omm_sb[:, g * P:g * P + gw],
                        scalar=ab[:, 1:2],
                        in1=comb[:, 0:gw],
                        op0=mybir.AluOpType.mult,
                        op1=mybir.AluOpType.add,
                    )
                    y_sb = p3s.tile([P, TG, OUT], F32, tag="ysb")
                    for j in range(ng):
                        py_ = p3p.tile([P, OUT], F32, tag="y")
                        nc.tensor.matmul(
                            out=py_[:], lhsT=comb[:, j * P:(j + 1) * P],
                            rhs=wlT_sb, start=True, stop=True,
                        )
                        nc.vector.tensor_add(
                            out=y_sb[:, j, :], in0=py_[:], in1=blb_sb)
                    nc.sync.dma_start(
                        out=y[g * P:g * P + gw, :].rearrange(
                            "(a p) o -> p a o", p=P),
                        in_=y_sb[:, 0:ng, :])

    nc.compile()
    return nc


_CACHE = {}


def prep_all(inputs):
    host = build_host_tensors(inputs)
    e16, dlr, CmT, c16, chr_ = bucket_edges_v3(inputs)
    xTb = host.pop("xT")
    CHM = NWIN * chr_
    _, _, _, _, PK_TOT = pk_offsets(CHM)
    in_maps = []
    for c in range(NCORES):
        xl = np.ascontiguousarray(xTb[:, c * ND:(c + 1) * ND])
        pk = np.concatenate([
            e16[c].ravel(),
            CmT[c].view(np.int16).ravel(),
            dlr[c].view(np.int16).ravel(),
            xl.view(np.int16).ravel(),
        ])[None, :]
        assert pk.shape[1] == PK_TOT, (pk.shape, PK_TOT)
        in_maps.append(dict(host, pk=np.ascontiguousarray(pk)))
    return in_maps, c16, chr_


def kernel(**inputs):
    in_maps, c16, chr_ = prep_all(inputs)
    key = (c16, chr_)
    if key not in _CACHE:
        _CACHE[key] = build_program(c16, chr_)
    nc = _CACHE[key]
    res = run_bass_kernel_spmd(nc, in_maps, core_ids=list(range(NCORES)))
    out = np.empty((N_MAT, OUT), np.float32)
    for c in range(NCORES):
        out[c * ND:(c + 1) * ND] = res.results[c]["y"][:ND]
    return out
